# revision 1
# baseline (speedup 1.0000x reference)
"""GATWithSentenceEmbedding Trainium2 kernel (8 NeuronCores, SPMD + collectives).

Sharding:
  - fcl [E,E] / fce [BERT,E] column-sharded (each core computes a 1024-chunk of
    g1/g2); fc2 [2E,E] row-sharded with matching rows; one AllReduce yields the
    full orig_edge_logits on every core.
  - GAT: edges sorted by dst; core c owns dst nodes [256c, 256c+256) and their
    incoming edges (incl. self-loops). Segment softmax/aggregation via one-hot
    matmul into PSUM. xp2 / h2 chunks are AllGathered between layers.
  - Edge MLP: same dst-based edge partition (real edges only); masked-BN stats
    combined with two tiny AllReduces.
"""

import numpy as np
from contextlib import ExitStack

import concourse.bass as bass
import concourse.mybir as mybir
import concourse.tile as tile
from concourse import bacc
from concourse.bass_utils import run_bass_kernel_spmd
from concourse.masks import make_identity

N, F, HC, S, H, E, BERT = 2048, 256, 256, 512, 4, 8192, 768
NCORES = 8
P = 128
NCHUNK = N // NCORES          # 256 dst nodes per core
ECH = E // NCORES             # 1024 g1/g2 columns per core
XP1W = H * HC + 2 * H         # 1032 = xp1 | al_s | al_d
XP2W = F + 2                  # 258  = xp2 | al_s | al_d
HC2 = HC // 2                 # 128
BIG = 1.0e9

dt = mybir.dt
AF = mybir.ActivationFunctionType
ALU = mybir.AluOpType
RG = [list(range(NCORES))]

_cache = {}
last_in_maps = None
DEBUG = False
TRACE = False
last_results = None


def _build(nt_g: int, nt_m: int, debug: bool = False, stage: int = 4):
    pad_g = nt_g * P
    pad_m = nt_m * P
    nc = bacc.Bacc("TRN2", target_bir_lowering=False, debug=False)

    def inp(name, shape, dtype=dt.float32):
        return nc.dram_tensor(name, shape, dtype, kind="ExternalInput")

    # shared inputs
    x_in = inp("x", [N, F])
    sent_in = inp("sent_emb", [BERT])
    elp_in = inp("elp", [E])
    fc0_w = inp("fc0_w", [BERT, S]); fc0_b = inp("fc0_b", [S])
    fc1_w = inp("fc1_w", [F, S]); fc1_b = inp("fc1_b", [S])
    c1w = inp("conv1_W", [S, H * HC]); c1a = inp("conv1_a", [2 * H * HC])
    c1b = inp("conv1_b", [H * HC])
    c2w = inp("conv2_W", [H * HC, F]); c2a = inp("conv2_a", [2 * F])
    c2b = inp("conv2_b", [F])
    m1w = inp("mlp1_w", [4 * F, HC]); m1b = inp("mlp1_b", [HC])
    bn1g = inp("bn1_g", [HC]); bn1b = inp("bn1_b", [HC])
    m2w = inp("mlp2_w", [HC, HC2]); m2b = inp("mlp2_b", [HC2])
    bn2g = inp("bn2_g", [HC2]); bn2b = inp("bn2_b", [HC2])
    m3w = inp("mlp3_w", [HC2, 1]); m3b = inp("mlp3_b", [1])
    fc2_b = inp("fc2_b", [E])
    # per-core inputs
    fclw_sh = inp("fclw_sh", [E, ECH]); fclb_sh = inp("fclb_sh", [ECH])
    fcew_sh = inp("fcew_sh", [BERT, ECH]); fceb_sh = inp("fceb_sh", [ECH])
    fc2w_sh = inp("fc2w_sh", [2 * ECH, E])
    g_src = inp("g_src", [pad_g], dt.int32)
    g_dst = inp("g_dst", [pad_g], dt.int32)
    g_lidx = inp("g_lidx", [pad_g], dt.int32)
    g_oh = inp("g_oh", [pad_g, NCHUNK], dt.float16)
    m_src = inp("m_src", [pad_m], dt.int32)
    m_dst = inp("m_dst", [pad_m], dt.int32)
    m_lidx = inp("m_lidx", [pad_m], dt.int32)
    # outputs
    orig_out = nc.dram_tensor("orig_out", [E], dt.float32, kind="ExternalOutput")
    score_out = nc.dram_tensor("score_out", [pad_m], dt.float32,
                               kind="ExternalOutput")
    dbg = {}
    if debug:
        for nm, shp in [("h_dbg", [N, S]), ("xp1_dbg", [N, XP1W]),
                        ("h1_dbg", [NCHUNK, H * HC]), ("xp2_dbg", [N, XP2W]),
                        ("h2_dbg", [N, F]), ("z1_dbg", [pad_m, HC]),
                        ("st1_dbg", [520]),
                        ("xs_dbg", [pad_g, XP1W]), ("ad_dbg", [pad_g, 2 * H]),
                        ("ex_dbg", [pad_g, H]), ("den_dbg", [2 * P, H]),
                        ("msum_dbg", [2 * P, H * HC])]:
            dbg[nm] = nc.dram_tensor(nm, shp, dt.float32, kind="ExternalOutput")

    def bcast(dram_handle, cols, offset=0):
        """AP reading a [1, cols] DRAM row replicated over 128 partitions."""
        return bass.AP(tensor=dram_handle.ap().tensor, offset=offset,
                       ap=[[0, P], [1, cols]])

    def bcast_ap(ap_tile, cols, offset=0):
        a = ap_tile[:] if not isinstance(ap_tile, bass.AP) else ap_tile
        return bass.AP(tensor=a.tensor, offset=a.offset + offset,
                       ap=[[0, P], [1, cols]])

    with tile.TileContext(nc) as tc:
        with (
            tc.tile_pool(name="dram", bufs=1, space="DRAM") as dram,
            tc.tile_pool(name="single", bufs=1) as single,
            tc.tile_pool(name="sb", bufs=2) as sb,
            tc.tile_pool(name="psum2", bufs=2, space="PSUM") as psum2,
            tc.tile_pool(name="keep", bufs=1) as keep,
        ):
            ident = single.tile([P, P], dt.float32)
            make_identity(nc, ident[:])
            ident_h = single.tile([P, P], dt.float16)
            nc.vector.tensor_copy(ident_h[:], ident[:])

            # internal DRAM
            xp1_dram = dram.tile([N, XP1W], dt.float16)
            al1d_dram = dram.tile([N, 2 * H], dt.float16)
            fc2part = dram.tile([E], dt.float32)
            logits_dram = dram.tile([E], dt.float32, addr_space="Shared")
            lext_dram = dram.tile([E + 2, 1], dt.float32)
            sent_dram = dram.tile([S], dt.float32)
            g12_dram = dram.tile([2 * ECH], dt.float32)
            xp2_in = dram.tile([NCHUNK, XP2W], dt.float16)
            xp2_dram = dram.tile([N, XP2W], dt.float16, addr_space="Shared")
            h2_in = dram.tile([NCHUNK, F], dt.float16)
            h2_dram = dram.tile([N, F], dt.float16, addr_space="Shared")
            st1_in = dram.tile([520], dt.float32)
            st1_out = dram.tile([520], dt.float32, addr_space="Shared")
            st2_in = dram.tile([2 * HC2], dt.float32)
            st2_out = dram.tile([2 * HC2], dt.float32, addr_space="Shared")
            row_dram = dram.tile([4 * HC], dt.float32)  # scratch rows for bcast

            # ======== phases A (h/xp1) + B (g1/g2/fc2) — scoped pools ========
            esA = ExitStack()
            sbA = esA.enter_context(tc.tile_pool(name="sbA", bufs=2))
            psA = esA.enter_context(tc.tile_pool(name="psA", bufs=1, space="PSUM"))

            # sent = relu(sent_emb @ fc0_w + fc0_b), weights-stationary chunks
            sent_sb = single.tile([P, BERT // P], dt.float32)
            nc.sync.dma_start(sent_sb[:], sent_in.ap().rearrange("(k p) -> p k", p=P))
            fc0w_t = [sbA.tile([P, S], dt.float32, tag=f"fc0w{k}", bufs=1,
                               name=f"fc0w{k}")
                      for k in range(BERT // P)]
            for k in range(BERT // P):
                nc.sync.dma_start(fc0w_t[k][:], fc0_w[k * P:(k + 1) * P, :])
            for j in range(S // P):
                ps_v = psA.tile([P, 1], dt.float32, space="PSUM", tag="vec",
                                bufs=2, name="ps_v")
                for k in range(BERT // P):
                    nc.tensor.matmul(ps_v[:], lhsT=fc0w_t[k][:, j * P:(j + 1) * P],
                                     rhs=sent_sb[:, k:k + 1],
                                     start=(k == 0), stop=(k == BERT // P - 1))
                bcol = sbA.tile([P, 1], dt.float32, tag="bcol")
                nc.sync.dma_start(bcol[:], fc0_b[j * P:(j + 1) * P][:, None])
                sc = sbA.tile([P, 1], dt.float32, tag="scol")
                nc.vector.tensor_add(sc[:], ps_v[:], bcol[:])
                nc.scalar.activation(sc[:], sc[:], AF.Relu)
                nc.sync.dma_start(sent_dram[j * P:(j + 1) * P][:, None], sc[:])
            sent_bc = single.tile([P, S], dt.float32)
            nc.sync.dma_start(sent_bc[:], bcast_ap(sent_dram, S))

            # W1aug = [conv1_W | W@a_src | W@a_dst] as 4 k-tiles [128, 1032]
            c1a_bc = sbA.tile([P, 2 * H * HC], dt.float32, tag="c1abc", bufs=1)
            nc.sync.dma_start(c1a_bc[:], bcast(c1a, 2 * H * HC))
            w1aug = [sbA.tile([P, XP1W], dt.float32, tag=f"w1aug{k}", bufs=1,
                              name=f"w1aug{k}")
                     for k in range(S // P)]
            for k in range(S // P):
                nc.sync.dma_start(w1aug[k][:, 0:H * HC],
                                  c1w[k * P:(k + 1) * P, :])
                tmp = sbA.tile([P, H * HC], dt.float32, tag="scratch4k")
                nc.vector.tensor_mul(tmp[:], w1aug[k][:, 0:H * HC],
                                     c1a_bc[:, 0:H * HC])
                for h in range(H):
                    nc.vector.reduce_sum(
                        w1aug[k][:, H * HC + h:H * HC + h + 1],
                        tmp[:, h * HC:(h + 1) * HC], axis=mybir.AxisListType.X)
                nc.vector.tensor_mul(tmp[:], w1aug[k][:, 0:H * HC],
                                     c1a_bc[:, H * HC:2 * H * HC])
                for h in range(H):
                    nc.vector.reduce_sum(
                        w1aug[k][:, H * HC + H + h:H * HC + H + h + 1],
                        tmp[:, h * HC:(h + 1) * HC], axis=mybir.AxisListType.X)

            # h = relu(x @ fc1_w + fc1_b) + sent ; xp1aug = h @ W1aug
            fc1w_t = [sbA.tile([P, S], dt.float32, tag=f"fc1w{k}", bufs=1,
                               name=f"fc1w{k}")
                      for k in range(F // P)]
            for k in range(F // P):
                nc.sync.dma_start(fc1w_t[k][:], fc1_w[k * P:(k + 1) * P, :])
            w1aug_h = [sbA.tile([P, XP1W], dt.float16, tag=f"w1augh{k}", bufs=1,
                               name=f"w1augh{k}")
                       for k in range(S // P)]
            for k in range(S // P):
                nc.vector.tensor_copy(w1aug_h[k][:], w1aug[k][:])
            fc1b_bc = sbA.tile([P, S], dt.float32, tag="fc1bbc", bufs=1)
            nc.sync.dma_start(fc1b_bc[:], bcast(fc1_b, S))
            for nt in range(N // P):
                x_t = sbA.tile([P, F], dt.float32, tag="x")
                nc.sync.dma_start(x_t[:], x_in[nt * P:(nt + 1) * P, :])
                ps_h = psA.tile([P, S], dt.float32, space="PSUM", tag="ps_h")
                for k in range(F // P):
                    ps_xt = psum2.tile([P, P], dt.float32, space="PSUM", tag="ps_xt")
                    nc.tensor.transpose(ps_xt[:], x_t[:, k * P:(k + 1) * P], ident[:])
                    xT = sb.tile([P, P], dt.float32, tag="xT")
                    nc.vector.tensor_copy(xT[:], ps_xt[:])
                    nc.tensor.matmul(ps_h[:], lhsT=xT[:], rhs=fc1w_t[k][:],
                                     start=(k == 0), stop=(k == F // P - 1))
                h_t = sbA.tile([P, S], dt.float32, tag="h")
                nc.vector.tensor_add(h_t[:], ps_h[:], fc1b_bc[:])
                nc.scalar.activation(h_t[:], h_t[:], AF.Relu)
                nc.vector.tensor_add(h_t[:], h_t[:], sent_bc[:])
                if debug:
                    nc.sync.dma_start(dbg["h_dbg"][nt * P:(nt + 1) * P, :], h_t[:])
                ps_xp1 = psA.tile([P, XP1W], dt.float32, space="PSUM", tag="ps_xp1")
                for k in range(S // P):
                    ps_ht = psum2.tile([P, P], dt.float32, space="PSUM", tag="ps_xt")
                    nc.tensor.transpose(ps_ht[:], h_t[:, k * P:(k + 1) * P], ident[:])
                    hT = sb.tile([P, P], dt.float16, tag="xTh")
                    nc.vector.tensor_copy(hT[:], ps_ht[:])
                    for s0, s1 in ((0, 512), (512, 1024), (1024, XP1W)):
                        nc.tensor.matmul(ps_xp1[:, s0:s1], lhsT=hT[:],
                                         rhs=w1aug_h[k][:, s0:s1],
                                         start=(k == 0), stop=(k == S // P - 1))
                xp1_t = sbA.tile([P, XP1W], dt.float16, tag="xp1")
                nc.vector.tensor_copy(xp1_t[:], ps_xp1[:])
                nc.sync.dma_start(xp1_dram[nt * P:(nt + 1) * P, :], xp1_t[:])
                nc.sync.dma_start(al1d_dram[nt * P:(nt + 1) * P, :],
                                  xp1_t[:, H * HC:H * HC + 2 * H])
                if debug:
                    nc.sync.dma_start(dbg["xp1_dbg"][nt * P:(nt + 1) * P, :],
                                      xp1_t[:])

            # g1/g2/fc2 via weights-stationary vector-matmuls (PE ~= LS rate).
            elp_sb = single.tile([P, E // P], dt.float32)
            nc.sync.dma_start(elp_sb[:], elp_in.ap().rearrange("(k p) -> p k", p=P))
            g_sb = single.tile([P, 2 * ECH // P], dt.float32)
            KQ = 16  # k-tiles per column-slice DMA
            for j in range(ECH // P):       # g1 output chunks
                ps_v = psA.tile([P, 1], dt.float32, space="PSUM", tag="vec",
                                bufs=2, name="ps_v")
                for q in range(E // P // KQ):
                    wq = sbA.tile([P, KQ, P], dt.float32, tag="wcol", bufs=3,
                                  name="wq")
                    nc.sync.dma_start(
                        wq[:],
                        fclw_sh[q * KQ * P:(q + 1) * KQ * P,
                                j * P:(j + 1) * P].rearrange(
                                    "(k p) j -> p k j", p=P))
                    for kk in range(KQ):
                        k = q * KQ + kk
                        nc.tensor.matmul(ps_v[:], lhsT=wq[:, kk, :],
                                         rhs=elp_sb[:, k:k + 1],
                                         start=(k == 0), stop=(k == E // P - 1))
                bcol = sbA.tile([P, 1], dt.float32, tag="bcol")
                nc.sync.dma_start(bcol[:], fclb_sh[j * P:(j + 1) * P][:, None])
                nc.vector.tensor_add(g_sb[:, j:j + 1], ps_v[:], bcol[:])
                nc.scalar.activation(g_sb[:, j:j + 1], g_sb[:, j:j + 1], AF.Relu)
            for j in range(ECH // P):       # g2 output chunks
                ps_v = psA.tile([P, 1], dt.float32, space="PSUM", tag="vec",
                                bufs=2, name="ps_v")
                wq = sbA.tile([P, BERT // P, P], dt.float32, tag="wcol",
                              bufs=3, name="wq")
                nc.sync.dma_start(
                    wq[:],
                    fcew_sh[:, j * P:(j + 1) * P].rearrange(
                        "(k p) j -> p k j", p=P))
                for k in range(BERT // P):
                    nc.tensor.matmul(ps_v[:], lhsT=wq[:, k, :],
                                     rhs=sent_sb[:, k:k + 1],
                                     start=(k == 0), stop=(k == BERT // P - 1))
                bcol = sbA.tile([P, 1], dt.float32, tag="bcol")
                nc.sync.dma_start(bcol[:], fceb_sh[j * P:(j + 1) * P][:, None])
                jo = ECH // P + j
                nc.vector.tensor_add(g_sb[:, jo:jo + 1], ps_v[:], bcol[:])
                nc.scalar.activation(g_sb[:, jo:jo + 1], g_sb[:, jo:jo + 1],
                                     AF.Relu)
            # fc2 partial: 64 output chunks, k = 16 g-columns
            for j in range(E // P):
                ps_v = psA.tile([P, 1], dt.float32, space="PSUM", tag="vec",
                                bufs=2, name="ps_v")
                wq = sbA.tile([P, 2 * ECH // P, P], dt.float32, tag="wcol",
                              bufs=3, name="wq")
                nc.sync.dma_start(
                    wq[:],
                    fc2w_sh[:, j * P:(j + 1) * P].rearrange(
                        "(k p) j -> p k j", p=P))
                for k in range(2 * ECH // P):
                    nc.tensor.matmul(ps_v[:], lhsT=wq[:, k, :],
                                     rhs=g_sb[:, k:k + 1],
                                     start=(k == 0), stop=(k == 2 * ECH // P - 1))
                f2c = sbA.tile([P, 1], dt.float32, tag="f2c")
                nc.vector.tensor_copy(f2c[:], ps_v[:])
                nc.sync.dma_start(fc2part[j * P:(j + 1) * P][:, None], f2c[:])
            esA.close()
            if stage >= 2:

                nc.gpsimd.collective_compute(
                    "AllReduce", ALU.add, replica_groups=RG,
                    ins=[fc2part[:]], outs=[logits_dram[:]])
                # logits += fc2_b ; orig_out ; logits_ext
                lg_pf = single.tile([P, E // P], dt.float32)
                nc.sync.dma_start(lg_pf[:], logits_dram[:].rearrange("(p f) -> p f", p=P))
                f2b_pf = single.tile([P, E // P], dt.float32)
                nc.sync.dma_start(f2b_pf[:], fc2_b.ap().rearrange("(p f) -> p f", p=P))
                nc.vector.tensor_add(lg_pf[:], lg_pf[:], f2b_pf[:])
                nc.sync.dma_start(orig_out.ap().rearrange("(p f) -> p f", p=P), lg_pf[:])
                nc.sync.dma_start(
                    lext_dram[0:E, :].rearrange("(p f) x -> p (f x)", p=P), lg_pf[:])
                big_t = single.tile([1, 2], dt.float32)
                nc.vector.memset(big_t[:, 0:1], BIG)
                nc.vector.memset(big_t[:, 1:2], -BIG)
                nc.sync.dma_start(lext_dram[E:E + 2, 0][None, :], big_t[:])

                # ============ conv1 aggregation (per dst-chunk) ============
                gsrc_sb = single.tile([P, nt_g], dt.int32)
                nc.sync.dma_start(gsrc_sb[:], g_src.ap().rearrange("(t p) -> p t", p=P))
                gdst_sb = single.tile([P, nt_g], dt.int32)
                nc.sync.dma_start(gdst_sb[:], g_dst.ap().rearrange("(t p) -> p t", p=P))
                glidx_sb = single.tile([P, nt_g], dt.int32)
                nc.sync.dma_start(glidx_sb[:], g_lidx.ap().rearrange("(t p) -> p t", p=P))
                valid_t = [keep.tile([P, 1], dt.float32, tag=f"valid{t}",
                                     name=f"valid{t}")
                           for t in range(nt_g)]
                esC = ExitStack()
                sbC = esC.enter_context(tc.tile_pool(name="sbC", bufs=2))
                psC = esC.enter_context(tc.tile_pool(name="psC", bufs=1, space="PSUM"))
                ps_msg = [psC.tile([P, H * HC], dt.float32, space="PSUM",
                                   tag=f"ps_msg{d}", name=f"ps_msg{d}")
                          for d in range(2)]
                ps_den = [psC.tile([P, H], dt.float32, space="PSUM",
                                   tag=f"ps_den{d}", name=f"ps_den{d}")
                          for d in range(2)]
                for t in range(nt_g):
                    oh_t = sbC.tile([P, NCHUNK], dt.float16, tag="oh", bufs=6)
                    nc.sync.dma_start(oh_t[:], g_oh[t * P:(t + 1) * P, :])
                    xs = sbC.tile([P, XP1W], dt.float16, tag="gxs", bufs=6)
                    nc.gpsimd.indirect_dma_start(
                        out=xs[:], out_offset=None, in_=xp1_dram[:],
                        in_offset=bass.IndirectOffsetOnAxis(
                            ap=gsrc_sb[:, t:t + 1], axis=0))
                    ad = sbC.tile([P, 2 * H], dt.float16, tag="gad", bufs=6)
                    nc.gpsimd.indirect_dma_start(
                        out=ad[:], out_offset=None, in_=al1d_dram[:],
                        in_offset=bass.IndirectOffsetOnAxis(
                            ap=gdst_sb[:, t:t + 1], axis=0))
                    lg = sbC.tile([P, 1], dt.float32, tag="glg", bufs=6)
                    nc.gpsimd.indirect_dma_start(
                        out=lg[:], out_offset=None, in_=lext_dram[:],
                        in_offset=bass.IndirectOffsetOnAxis(
                            ap=glidx_sb[:, t:t + 1], axis=0))
                    nc.vector.tensor_scalar(valid_t[t][:], lg[:], 0.0, None,
                                            op0=ALU.is_gt)
                    alpha = sbC.tile([P, H], dt.float32, tag="alpha")
                    nc.vector.tensor_add(alpha[:], xs[:, H * HC:H * HC + H],
                                         ad[:, H:2 * H])
                    nc.vector.scalar_tensor_tensor(alpha[:], alpha[:], 0.2, alpha[:],
                                                   op0=ALU.mult, op1=ALU.max)
                    ex = sbC.tile([P, H], dt.float32, tag="ex")
                    nc.scalar.activation(ex[:], alpha[:], AF.Exp)
                    nc.vector.tensor_mul(ex[:], ex[:],
                                         valid_t[t][:].to_broadcast([P, H]))
                    if debug:
                        nc.sync.dma_start(dbg["xs_dbg"][t * P:(t + 1) * P, :], xs[:])
                        nc.sync.dma_start(dbg["ad_dbg"][t * P:(t + 1) * P, :], ad[:])
                        nc.sync.dma_start(dbg["ex_dbg"][t * P:(t + 1) * P, :], ex[:])
                    msg = sbC.tile([P, H * HC + H], dt.float16, tag="msg")
                    for h in range(H):
                        nc.vector.tensor_tensor(
                            msg[:, h * HC:(h + 1) * HC], xs[:, h * HC:(h + 1) * HC],
                            ex[:, h:h + 1].to_broadcast([P, HC]), op=ALU.mult)
                    nc.vector.tensor_copy(msg[:, H * HC:H * HC + H], ex[:])
                    for d in range(2):
                        lhsT = oh_t[:, d * P:(d + 1) * P]
                        st, sp = (t == 0), (t == nt_g - 1)
                        nc.tensor.matmul(ps_msg[d][:, 0:512], lhsT=lhsT,
                                         rhs=msg[:, 0:512], start=st, stop=sp)
                        nc.tensor.matmul(ps_msg[d][:, 512:1024], lhsT=lhsT,
                                         rhs=msg[:, 512:1024], start=st, stop=sp)
                        nc.tensor.matmul(ps_den[d][:], lhsT=lhsT,
                                         rhs=msg[:, H * HC:H * HC + H],
                                         start=st, stop=sp)
                # finalize conv1 + xp2aug
                c1b_bc = sbC.tile([P, H * HC], dt.float32, tag="c1bbc", bufs=1)
                nc.sync.dma_start(c1b_bc[:], bcast(c1b, H * HC))
                c2a_bc = sbC.tile([P, 2 * F], dt.float32, tag="c2abc", bufs=1)
                nc.sync.dma_start(c2a_bc[:], bcast(c2a, 2 * F))
                w2aug = [keep.tile([P, XP2W], dt.float32, tag=f"w2aug{k}",
                                   name=f"w2aug{k}")
                         for k in range(H * HC // P)]
                for k in range(H * HC // P):
                    nc.sync.dma_start(w2aug[k][:, 0:F], c2w[k * P:(k + 1) * P, :])
                    tmp = sbC.tile([P, F], dt.float32, tag="w2tmp")
                    nc.vector.tensor_mul(tmp[:], w2aug[k][:, 0:F], c2a_bc[:, 0:F])
                    nc.vector.reduce_sum(w2aug[k][:, F:F + 1], tmp[:],
                                         axis=mybir.AxisListType.X)
                    nc.vector.tensor_mul(tmp[:], w2aug[k][:, 0:F], c2a_bc[:, F:2 * F])
                    nc.vector.reduce_sum(w2aug[k][:, F + 1:F + 2], tmp[:],
                                         axis=mybir.AxisListType.X)
                if debug:
                    for d in range(2):
                        dd = sbC.tile([P, H], dt.float32, tag="dendbg")
                        nc.vector.tensor_copy(dd[:], ps_den[d][:])
                        nc.sync.dma_start(dbg["den_dbg"][d * P:(d + 1) * P, :], dd[:])
                        dm = sbC.tile([P, H * HC], dt.float32, tag="msumdbg")
                        nc.vector.tensor_copy(dm[:], ps_msg[d][:])
                        nc.sync.dma_start(dbg["msum_dbg"][d * P:(d + 1) * P, :], dm[:])
                h1_keep = [keep.tile([P, H * HC], dt.float16, tag=f"h1k{d}",
                                     name=f"h1k{d}")
                           for d in range(2)]
                for d in range(2):
                    denr = sbC.tile([P, H], dt.float32, tag="denr")
                    nc.vector.reciprocal(denr[:], ps_den[d][:])
                    h1_t = h1_keep[d]
                    for h in range(H):
                        nc.vector.scalar_tensor_tensor(
                            h1_t[:, h * HC:(h + 1) * HC],
                            ps_msg[d][:, h * HC:(h + 1) * HC],
                            denr[:, h:h + 1],
                            c1b_bc[:, h * HC:(h + 1) * HC],
                            op0=ALU.mult, op1=ALU.add)
                    # elu = relu(x) + exp(min(x,0)) - 1
                    relu_t = sbC.tile([P, H * HC], dt.float32, tag="elu_r")
                    nc.scalar.activation(relu_t[:], h1_t[:], AF.Relu)
                    nc.vector.tensor_scalar_min(h1_t[:], h1_t[:], 0.0)
                    nc.scalar.activation(h1_t[:], h1_t[:], AF.Exp)
                    nc.vector.scalar_tensor_tensor(h1_t[:], h1_t[:], -1.0,
                                                   relu_t[:],
                                                   op0=ALU.add, op1=ALU.add)
                    if debug:
                        nc.sync.dma_start(dbg["h1_dbg"][d * P:(d + 1) * P, :], h1_t[:])
                esC.close()
                esD = ExitStack()
                sbD = esD.enter_context(tc.tile_pool(name="sbD", bufs=2))
                psD = esD.enter_context(tc.tile_pool(name="psD", bufs=1, space="PSUM"))
                w2aug_h = [sbD.tile([P, XP2W], dt.float16, tag=f"w2augh{k}", bufs=1,
                                    name=f"w2augh{k}")
                           for k in range(H * HC // P)]
                for k in range(H * HC // P):
                    nc.vector.tensor_copy(w2aug_h[k][:], w2aug[k][:])
                for d in range(2):
                    h1_t = h1_keep[d]
                    ps_xp2 = psD.tile([P, XP2W], dt.float32, space="PSUM", tag="ps_xp2")
                    for k in range(H * HC // P):
                        ps_h1t = psD.tile([P, P], dt.float16, space="PSUM",
                                          tag="ps_xth", bufs=2)
                        nc.tensor.transpose(ps_h1t[:], h1_t[:, k * P:(k + 1) * P],
                                            ident_h[:])
                        h1T = sb.tile([P, P], dt.float16, tag="xTh")
                        nc.vector.tensor_copy(h1T[:], ps_h1t[:])
                        nc.tensor.matmul(ps_xp2[:], lhsT=h1T[:], rhs=w2aug_h[k][:],
                                         start=(k == 0), stop=(k == H * HC // P - 1))
                    xp2_t = sbD.tile([P, XP2W], dt.float16, tag="xp2")
                    nc.vector.tensor_copy(xp2_t[:], ps_xp2[:])
                    nc.sync.dma_start(xp2_in[d * P:(d + 1) * P, :], xp2_t[:])
                esD.close()
                nc.gpsimd.collective_compute(
                    "AllGather", ALU.bypass, replica_groups=RG,
                    ins=[xp2_in[:]], outs=[xp2_dram[:]])
                if debug:
                    for nt in range(N // P):
                        dtmp = sb.tile([P, XP2W], dt.float32, tag="dbg1")
                        nc.sync.dma_start(dtmp[:], xp2_dram[nt * P:(nt + 1) * P, :])
                        nc.sync.dma_start(dbg["xp2_dbg"][nt * P:(nt + 1) * P, :],
                                          dtmp[:])

            if stage >= 3:
                # ============ conv2 aggregation ============
                esE = ExitStack()
                sbE = esE.enter_context(tc.tile_pool(name="sbE", bufs=2))
                psE = esE.enter_context(tc.tile_pool(name="psE", bufs=1, space="PSUM"))
                ps_m2 = [psE.tile([P, F + 1], dt.float32, space="PSUM",
                                  tag=f"ps_m2{d}", name=f"ps_m2{d}")
                         for d in range(2)]
                for t in range(nt_g):
                    oh_t = sbE.tile([P, NCHUNK], dt.float16, tag="oh2", bufs=4)
                    nc.sync.dma_start(oh_t[:], g_oh[t * P:(t + 1) * P, :])
                    xs2 = sbE.tile([P, XP2W], dt.float16, tag="xs2", bufs=4)
                    nc.gpsimd.indirect_dma_start(
                        out=xs2[:], out_offset=None, in_=xp2_dram[:],
                        in_offset=bass.IndirectOffsetOnAxis(
                            ap=gsrc_sb[:, t:t + 1], axis=0))
                    xd2 = sbE.tile([P, XP2W], dt.float16, tag="xd2", bufs=4)
                    nc.gpsimd.indirect_dma_start(
                        out=xd2[:], out_offset=None, in_=xp2_dram[:],
                        in_offset=bass.IndirectOffsetOnAxis(
                            ap=gdst_sb[:, t:t + 1], axis=0))
                    alpha2 = sbE.tile([P, 1], dt.float32, tag="alpha2")
                    nc.vector.tensor_add(alpha2[:], xs2[:, F:F + 1],
                                         xd2[:, F + 1:F + 2])
                    nc.vector.scalar_tensor_tensor(alpha2[:], alpha2[:], 0.2, alpha2[:],
                                                   op0=ALU.mult, op1=ALU.max)
                    ex2 = sbE.tile([P, 1], dt.float32, tag="ex2")
                    nc.scalar.activation(ex2[:], alpha2[:], AF.Exp)
                    nc.vector.tensor_mul(ex2[:], ex2[:], valid_t[t][:])
                    msg2 = sbE.tile([P, F + 1], dt.float16, tag="msg2")
                    nc.vector.tensor_tensor(msg2[:, 0:F], xs2[:, 0:F],
                                            ex2[:].to_broadcast([P, F]), op=ALU.mult)
                    nc.vector.tensor_copy(msg2[:, F:F + 1], ex2[:])
                    for d in range(2):
                        lhsT = oh_t[:, d * P:(d + 1) * P]
                        st, sp = (t == 0), (t == nt_g - 1)
                        nc.tensor.matmul(ps_m2[d][:], lhsT=lhsT, rhs=msg2[:],
                                         start=st, stop=sp)
                c2b_bc = sbE.tile([P, F], dt.float32, tag="c2bbc", bufs=1)
                nc.sync.dma_start(c2b_bc[:], bcast(c2b, F))
                for d in range(2):
                    d2r = sbE.tile([P, 1], dt.float32, tag="d2r")
                    nc.vector.reciprocal(d2r[:], ps_m2[d][:, F:F + 1])
                    h2_t = sbE.tile([P, F], dt.float16, tag="h2")
                    nc.vector.tensor_tensor(h2_t[:], ps_m2[d][:, 0:F],
                                            d2r[:].to_broadcast([P, F]), op=ALU.mult)
                    nc.vector.tensor_add(h2_t[:], h2_t[:], c2b_bc[:])
                    nc.sync.dma_start(h2_in[d * P:(d + 1) * P, :], h2_t[:])
                esE.close()
                nc.gpsimd.collective_compute(
                    "AllGather", ALU.bypass, replica_groups=RG,
                    ins=[h2_in[:]], outs=[h2_dram[:]])
                if debug:
                    for nt in range(N // P):
                        dtmp2 = sb.tile([P, F], dt.float32, tag="dbg2")
                        nc.sync.dma_start(dtmp2[:], h2_dram[nt * P:(nt + 1) * P, :])
                        nc.sync.dma_start(dbg["h2_dbg"][nt * P:(nt + 1) * P, :],
                                          dtmp2[:])

            if stage >= 4:
                # ============ edge MLP ============
                msrc_sb = single.tile([P, nt_m], dt.int32)
                nc.sync.dma_start(msrc_sb[:], m_src.ap().rearrange("(t p) -> p t", p=P))
                mdst_sb = single.tile([P, nt_m], dt.int32)
                nc.sync.dma_start(mdst_sb[:], m_dst.ap().rearrange("(t p) -> p t", p=P))
                mlidx_sb = single.tile([P, nt_m], dt.int32)
                nc.sync.dma_start(mlidx_sb[:], m_lidx.ap().rearrange("(t p) -> p t", p=P))
                m1w_t = [keep.tile([P, HC], dt.float32, tag=f"m1w{k}", name=f"m1w{k}")
                         for k in range(4 * F // P)]
                m1w_h = [keep.tile([P, HC], dt.float16, tag=f"m1wh{k}",
                                   name=f"m1wh{k}")
                         for k in range(4 * F // P)]
                for k in range(4 * F // P):
                    nc.sync.dma_start(m1w_t[k][:], m1w[k * P:(k + 1) * P, :])
                    nc.vector.tensor_copy(m1w_h[k][:], m1w_t[k][:])
                m1b_bc = single.tile([P, HC], dt.float32)
                nc.sync.dma_start(m1b_bc[:], bcast(m1b, HC))
                mask_f = [keep.tile([P, 1], dt.float32, tag=f"maskf{t}",
                                    name=f"maskf{t}")
                          for t in range(nt_m)]
                mask_u8 = [keep.tile([P, 1], dt.uint8, tag=f"masku{t}",
                                     name=f"masku{t}")
                           for t in range(nt_m)]
                z1_t = [keep.tile([P, HC], dt.float32, tag=f"z1_{t}", name=f"z1_{t}")
                        for t in range(nt_m)]
                esF = ExitStack()
                sbF = esF.enter_context(tc.tile_pool(name="sbF", bufs=2))
                psF = esF.enter_context(tc.tile_pool(name="psF", bufs=1, space="PSUM"))
                ps_s1 = psF.tile([1, HC], dt.float32, space="PSUM", tag="ps_s1")
                ps_q1 = psF.tile([1, HC], dt.float32, space="PSUM", tag="ps_q1")
                ps_cnt = psF.tile([1, 1], dt.float32, space="PSUM", tag="ps_cnt")
                for t in range(nt_m):
                    xi = sbF.tile([P, F], dt.float16, tag="xi", bufs=4)
                    nc.gpsimd.indirect_dma_start(
                        out=xi[:], out_offset=None, in_=h2_dram[:],
                        in_offset=bass.IndirectOffsetOnAxis(
                            ap=msrc_sb[:, t:t + 1], axis=0))
                    xj = sbF.tile([P, F], dt.float16, tag="xj", bufs=4)
                    nc.gpsimd.indirect_dma_start(
                        out=xj[:], out_offset=None, in_=h2_dram[:],
                        in_offset=bass.IndirectOffsetOnAxis(
                            ap=mdst_sb[:, t:t + 1], axis=0))
                    lg = sbF.tile([P, 1], dt.float32, tag="mlg", bufs=6)
                    nc.gpsimd.indirect_dma_start(
                        out=lg[:], out_offset=None, in_=lext_dram[:],
                        in_offset=bass.IndirectOffsetOnAxis(
                            ap=mlidx_sb[:, t:t + 1], axis=0))
                    nc.vector.tensor_scalar(mask_f[t][:], lg[:], 0.0, None,
                                            op0=ALU.is_gt)
                    nc.vector.tensor_copy(mask_u8[t][:], mask_f[t][:])
                    dsub = sbF.tile([P, F], dt.float16, tag="dsub")
                    nc.vector.tensor_sub(dsub[:], xi[:], xj[:])
                    nc.scalar.activation(dsub[:], dsub[:], AF.Abs)
                    pmul = sbF.tile([P, F], dt.float16, tag="pmul")
                    nc.vector.tensor_mul(pmul[:], xi[:], xj[:])
                    ps_z1 = psF.tile([P, HC], dt.float32, space="PSUM", tag="ps_z1")
                    for pi, piece in enumerate((xi, xj, dsub, pmul)):
                        for hf in range(2):
                            ps_t = psF.tile([P, P], dt.float16, space="PSUM",
                                            tag="ps_xth", bufs=2)
                            nc.tensor.transpose(ps_t[:], piece[:, hf * P:(hf + 1) * P],
                                                ident_h[:])
                            efT = sb.tile([P, P], dt.float16, tag="xTh")
                            nc.vector.tensor_copy(efT[:], ps_t[:])
                            k = pi * 2 + hf
                            nc.tensor.matmul(ps_z1[:], lhsT=efT[:],
                                             rhs=m1w_h[k][:],
                                             start=(k == 0), stop=(k == 7))
                    nc.vector.tensor_add(z1_t[t][:], ps_z1[:], m1b_bc[:])
                    if debug:
                        nc.sync.dma_start(dbg["z1_dbg"][t * P:(t + 1) * P, :],
                                          z1_t[t][:])
                    zsq = sbF.tile([P, HC], dt.float32, tag="zsq")
                    nc.vector.tensor_mul(zsq[:], z1_t[t][:], z1_t[t][:])
                    st, sp = (t == 0), (t == nt_m - 1)
                    nc.tensor.matmul(ps_s1[:], lhsT=mask_f[t][:], rhs=z1_t[t][:],
                                     start=st, stop=sp)
                    nc.tensor.matmul(ps_q1[:], lhsT=mask_f[t][:], rhs=zsq[:],
                                     start=st, stop=sp)
                    nc.tensor.matmul(ps_cnt[:], lhsT=mask_f[t][:], rhs=mask_f[t][:],
                                     start=st, stop=sp)
                # pack stats1, AllReduce
                s_sb = sbF.tile([1, HC], dt.float32, tag="stat")
                nc.vector.tensor_copy(s_sb[:], ps_s1[:])
                nc.sync.dma_start(st1_in[None, 0:HC], s_sb[:])
                q_sb = sbF.tile([1, HC], dt.float32, tag="stat")
                nc.vector.tensor_copy(q_sb[:], ps_q1[:])
                nc.sync.dma_start(st1_in[None, HC:2 * HC], q_sb[:])
                c_sb = sbF.tile([1, 1], dt.float32, tag="statc")
                nc.vector.tensor_copy(c_sb[:], ps_cnt[:])
                nc.sync.dma_start(st1_in[None, 2 * HC:2 * HC + 1], c_sb[:])
                zpad = sbF.tile([1, 7], dt.float32, tag="statz")
                nc.vector.memset(zpad[:], 0.0)
                nc.sync.dma_start(st1_in[None, 2 * HC + 1:520], zpad[:])
                esF.close()
                nc.gpsimd.collective_compute(
                    "AllReduce", ALU.add, replica_groups=RG,
                    ins=[st1_in[:]], outs=[st1_out[:]])
                if debug:
                    dstat = sb.tile([1, 520], dt.float32, tag="dbg3")
                    nc.sync.dma_start(dstat[:], st1_out[None, :])
                    nc.sync.dma_start(dbg["st1_dbg"][None, :], dstat[:])

                esG = ExitStack()
                sbG = esG.enter_context(tc.tile_pool(name="sbG", bufs=2))
                psG = esG.enter_context(tc.tile_pool(name="psG", bufs=1, space="PSUM"))

                def bn_rows(st_out, nch, g_in, b_in, row_off):
                    s_row = sbG.tile([1, nch], dt.float32, tag="bnrow_s_row", name="s_row")
                    nc.sync.dma_start(s_row[:], st_out[None, 0:nch])
                    q_row = sbG.tile([1, nch], dt.float32, tag="bnrow_q_row", name="q_row")
                    nc.sync.dma_start(q_row[:], st_out[None, nch:2 * nch])
                    cnt_row = sbG.tile([1, 1], dt.float32, tag="bnrow_cnt_row", name="cnt_row")
                    nc.sync.dma_start(cnt_row[:], st1_out[None, 2 * HC:2 * HC + 1])
                    nc.vector.tensor_scalar_max(cnt_row[:], cnt_row[:], 1.0)
                    cr = sbG.tile([1, 1], dt.float32, tag="bnrow_cr", name="cr")
                    nc.vector.reciprocal(cr[:], cnt_row[:])
                    mean = sbG.tile([1, nch], dt.float32, tag="bnrow_mean", name="mean")
                    nc.vector.tensor_tensor(mean[:], s_row[:],
                                            cr[:].to_broadcast([1, nch]), op=ALU.mult)
                    var = sbG.tile([1, nch], dt.float32, tag="bnrow_var", name="var")
                    nc.vector.tensor_tensor(var[:], q_row[:],
                                            cr[:].to_broadcast([1, nch]), op=ALU.mult)
                    msq = sbG.tile([1, nch], dt.float32, tag="bnrow_msq", name="msq")
                    nc.vector.tensor_mul(msq[:], mean[:], mean[:])
                    nc.vector.tensor_sub(var[:], var[:], msq[:])
                    nc.vector.tensor_scalar_add(var[:], var[:], 1e-5)
                    nc.scalar.activation(var[:], var[:], AF.Sqrt)
                    rstd = sbG.tile([1, nch], dt.float32, tag="bnrow_rstd", name="rstd")
                    nc.vector.reciprocal(rstd[:], var[:])
                    g_row = sbG.tile([1, nch], dt.float32, tag="bnrow_g_row", name="g_row")
                    nc.sync.dma_start(g_row[:], g_in[None, :])
                    b_row = sbG.tile([1, nch], dt.float32, tag="bnrow_b_row", name="b_row")
                    nc.sync.dma_start(b_row[:], b_in[None, :])
                    gs = sbG.tile([1, nch], dt.float32, tag="bnrow_gs", name="gs")
                    nc.vector.tensor_mul(gs[:], g_row[:], rstd[:])
                    gb = sbG.tile([1, nch], dt.float32, tag="bnrow_gb", name="gb")
                    nc.vector.tensor_mul(gb[:], mean[:], gs[:])
                    nc.vector.tensor_sub(gb[:], b_row[:], gb[:])
                    nc.sync.dma_start(row_dram[None, row_off:row_off + nch], gs[:])
                    nc.sync.dma_start(row_dram[None, row_off + nch:row_off + 2 * nch],
                                      gb[:])

                bn_rows(st1_out, HC, bn1g, bn1b, 0)
                gs1_bc = single.tile([P, HC], dt.float32)
                nc.sync.dma_start(gs1_bc[:], bcast_ap(row_dram, HC, 0))
                gb1_bc = single.tile([P, HC], dt.float32)
                nc.sync.dma_start(gb1_bc[:], bcast_ap(row_dram, HC, HC))
                m2w_t = [keep.tile([P, HC2], dt.float32, tag=f"m2w{k}", name=f"m2w{k}")
                         for k in range(HC // P)]
                m2w_h = [keep.tile([P, HC2], dt.float16, tag=f"m2wh{k}",
                                   name=f"m2wh{k}")
                         for k in range(HC // P)]
                for k in range(HC // P):
                    nc.sync.dma_start(m2w_t[k][:], m2w[k * P:(k + 1) * P, :])
                    nc.vector.tensor_copy(m2w_h[k][:], m2w_t[k][:])
                m2b_bc = single.tile([P, HC2], dt.float32)
                nc.sync.dma_start(m2b_bc[:], bcast(m2b, HC2))
                z2_t = [keep.tile([P, HC2], dt.float32, tag=f"z2_{t}", name=f"z2_{t}")
                        for t in range(nt_m)]
                ps_s2 = psG.tile([1, HC2], dt.float32, space="PSUM", tag="ps_s2")
                ps_q2 = psG.tile([1, HC2], dt.float32, space="PSUM", tag="ps_q2")
                for t in range(nt_m):
                    zn = sbG.tile([P, HC], dt.float16, tag="zn")
                    nc.vector.tensor_mul(zn[:], z1_t[t][:], gs1_bc[:])
                    nc.vector.tensor_add(zn[:], zn[:], gb1_bc[:])
                    nc.scalar.activation(zn[:], zn[:], AF.Relu)
                    ps_z2 = psG.tile([P, HC2], dt.float32, space="PSUM", tag="ps_z2")
                    for k in range(HC // P):
                        ps_t = psG.tile([P, P], dt.float16, space="PSUM",
                                        tag="ps_xth", bufs=2)
                        nc.tensor.transpose(ps_t[:], zn[:, k * P:(k + 1) * P],
                                            ident_h[:])
                        znT = sb.tile([P, P], dt.float16, tag="xTh")
                        nc.vector.tensor_copy(znT[:], ps_t[:])
                        nc.tensor.matmul(ps_z2[:], lhsT=znT[:], rhs=m2w_h[k][:],
                                         start=(k == 0), stop=(k == HC // P - 1))
                    nc.vector.tensor_add(z2_t[t][:], ps_z2[:], m2b_bc[:])
                    zsq2 = sbG.tile([P, HC2], dt.float32, tag="zsq2")
                    nc.vector.tensor_mul(zsq2[:], z2_t[t][:], z2_t[t][:])
                    st, sp = (t == 0), (t == nt_m - 1)
                    nc.tensor.matmul(ps_s2[:], lhsT=mask_f[t][:], rhs=z2_t[t][:],
                                     start=st, stop=sp)
                    nc.tensor.matmul(ps_q2[:], lhsT=mask_f[t][:], rhs=zsq2[:],
                                     start=st, stop=sp)
                s2_sb = sbG.tile([1, HC2], dt.float32, tag="stat2")
                nc.vector.tensor_copy(s2_sb[:], ps_s2[:])
                nc.sync.dma_start(st2_in[None, 0:HC2], s2_sb[:])
                q2_sb = sbG.tile([1, HC2], dt.float32, tag="stat2")
                nc.vector.tensor_copy(q2_sb[:], ps_q2[:])
                nc.sync.dma_start(st2_in[None, HC2:2 * HC2], q2_sb[:])
                nc.gpsimd.collective_compute(
                    "AllReduce", ALU.add, replica_groups=RG,
                    ins=[st2_in[:]], outs=[st2_out[:]])
                bn_rows(st2_out, HC2, bn2g, bn2b, 2 * HC)
                gs2_bc = single.tile([P, HC2], dt.float32)
                nc.sync.dma_start(gs2_bc[:], bcast_ap(row_dram, HC2, 2 * HC))
                gb2_bc = single.tile([P, HC2], dt.float32)
                nc.sync.dma_start(gb2_bc[:], bcast_ap(row_dram, HC2, 2 * HC + HC2))
                m3w_sb = single.tile([P, 1], dt.float32)
                nc.sync.dma_start(m3w_sb[:], m3w[:, :])
                m3w_h = single.tile([P, 1], dt.float16)
                nc.vector.tensor_copy(m3w_h[:], m3w_sb[:])
                m3w_h = single.tile([P, 1], dt.float16)
                nc.vector.tensor_copy(m3w_h[:], m3w_sb[:])

                m3b_bc = single.tile([P, 1], dt.float32)
                nc.sync.dma_start(m3b_bc[:], bcast(m3b, 1))
                neg25 = single.tile([P, 1], dt.float32)
                nc.vector.memset(neg25[:], -2.5)
                for t in range(nt_m):
                    zn2 = sbG.tile([P, HC2], dt.float16, tag="zn2")
                    nc.vector.tensor_mul(zn2[:], z2_t[t][:], gs2_bc[:])
                    nc.vector.tensor_add(zn2[:], zn2[:], gb2_bc[:])
                    nc.scalar.activation(zn2[:], zn2[:], AF.Relu)
                    ps_t = psG.tile([P, P], dt.float16, space="PSUM",
                                    tag="ps_xth", bufs=2)
                    nc.tensor.transpose(ps_t[:], zn2[:], ident_h[:])
                    znT2 = sb.tile([P, P], dt.float16, tag="xTh")
                    nc.vector.tensor_copy(znT2[:], ps_t[:])
                    ps_sc = psG.tile([P, 1], dt.float32, space="PSUM", tag="ps_sc")
                    nc.tensor.matmul(ps_sc[:], lhsT=znT2[:], rhs=m3w_h[:],
                                     start=True, stop=True)
                    score = sbG.tile([P, 1], dt.float32, tag="score")
                    nc.vector.tensor_add(score[:], ps_sc[:], m3b_bc[:])
                    sel = sbG.tile([P, 1], dt.float32, tag="sel")
                    nc.vector.select(sel[:], mask_u8[t][:], score[:], neg25[:])
                    nc.scalar.activation(sel[:], sel[:], AF.Sigmoid)
                    nc.sync.dma_start(score_out[t * P:(t + 1) * P][:, None], sel[:])
                esG.close()

    nc.compile()
    return nc


def kernel(**inputs):
    inputs = {k: np.asarray(v) for k, v in inputs.items()}
    src = inputs["edge_index"][0].astype(np.int64)
    dst = inputs["edge_index"][1].astype(np.int64)

    # --- edge partition by dst chunk (GAT set includes self loops) ---
    all_src = np.concatenate([src, np.arange(N, dtype=np.int64)])
    all_dst = np.concatenate([dst, np.arange(N, dtype=np.int64)])
    lidx_all = np.concatenate(
        [np.arange(E, dtype=np.int64), np.full(N, E, dtype=np.int64)])
    chunk_g = all_dst // NCHUNK
    gids = [np.where(chunk_g == c)[0] for c in range(NCORES)]
    nt_g = int(np.ceil(max(len(i) for i in gids) / P))
    pad_g = nt_g * P
    chunk_m = dst // NCHUNK
    mids = [np.where(chunk_m == c)[0] for c in range(NCORES)]
    nt_m = int(np.ceil(max(len(i) for i in mids) / P))
    pad_m = nt_m * P

    key = (nt_g, nt_m, DEBUG)
    if key not in _cache:
        _cache[key] = _build(nt_g, nt_m, debug=DEBUG)
    nc = _cache[key]

    shared = dict(
        x=inputs["x"],
        sent_emb=inputs["sent_emb"],
        elp=inputs["edge_logits_param"],
        fc0_w=inputs["fc0_w"], fc0_b=inputs["fc0_b"],
        fc1_w=inputs["fc1_w"], fc1_b=inputs["fc1_b"],
        conv1_W=inputs["conv1_W"],
        conv1_a=np.concatenate([inputs["conv1_asrc"].reshape(-1),
                                inputs["conv1_adst"].reshape(-1)]),
        conv1_b=inputs["conv1_b"],
        conv2_W=inputs["conv2_W"],
        conv2_a=np.concatenate([inputs["conv2_asrc"].reshape(-1),
                                inputs["conv2_adst"].reshape(-1)]),
        conv2_b=inputs["conv2_b"],
        mlp1_w=inputs["mlp1_w"], mlp1_b=inputs["mlp1_b"],
        bn1_g=inputs["bn1_g"], bn1_b=inputs["bn1_b"],
        mlp2_w=inputs["mlp2_w"], mlp2_b=inputs["mlp2_b"],
        bn2_g=inputs["bn2_g"], bn2_b=inputs["bn2_b"],
        mlp3_w=inputs["mlp3_w"], mlp3_b=inputs["mlp3_b"],
        fc2_b=inputs["fc2_b"],
    )
    shared = {k: np.ascontiguousarray(v, dtype=np.float32)
              for k, v in shared.items()}

    fcl_w, fce_w, fc2_w = inputs["fcl_w"], inputs["fce_w"], inputs["fc2_w"]
    in_maps = []
    for c in range(NCORES):
        gi = gids[c]
        mi = mids[c]
        gsrc = np.zeros(pad_g, np.int32); gsrc[:len(gi)] = all_src[gi]
        gdst = np.zeros(pad_g, np.int32); gdst[:len(gi)] = all_dst[gi]
        glidx = np.full(pad_g, E + 1, np.int32); glidx[:len(gi)] = lidx_all[gi]
        goh = np.zeros((pad_g, NCHUNK), np.float16)
        goh[np.arange(len(gi)), all_dst[gi] - c * NCHUNK] = 1.0
        msrc = np.zeros(pad_m, np.int32); msrc[:len(mi)] = src[mi]
        mdst = np.zeros(pad_m, np.int32); mdst[:len(mi)] = dst[mi]
        mlidx = np.full(pad_m, E + 1, np.int32); mlidx[:len(mi)] = mi
        m = dict(shared)
        m.update(
            fclw_sh=np.ascontiguousarray(fcl_w[:, c * ECH:(c + 1) * ECH],
                                         dtype=np.float32),
            fclb_sh=np.ascontiguousarray(inputs["fcl_b"][c * ECH:(c + 1) * ECH],
                                         dtype=np.float32),
            fcew_sh=np.ascontiguousarray(fce_w[:, c * ECH:(c + 1) * ECH],
                                         dtype=np.float32),
            fceb_sh=np.ascontiguousarray(inputs["fce_b"][c * ECH:(c + 1) * ECH],
                                         dtype=np.float32),
            fc2w_sh=np.ascontiguousarray(np.concatenate(
                [fc2_w[c * ECH:(c + 1) * ECH],
                 fc2_w[E + c * ECH:E + (c + 1) * ECH]], axis=0),
                dtype=np.float32),
            g_src=gsrc, g_dst=gdst, g_lidx=glidx, g_oh=goh,
            m_src=msrc, m_dst=mdst, m_lidx=mlidx,
        )
        in_maps.append(m)

    global last_results, last_in_maps
    last_in_maps = in_maps
    res = run_bass_kernel_spmd(nc, in_maps, core_ids=list(range(NCORES)),
                               trace=TRACE)
    last_results = res
    orig = res.results[0]["orig_out"].reshape(E).astype(np.float32)
    sig = np.empty(E, np.float32)
    for c in range(NCORES):
        mi = mids[c]
        sig[mi] = res.results[c]["score_out"].reshape(pad_m)[:len(mi)]
    return sig, orig



# revision 24
# speedup vs baseline: 1.0186x; 1.0186x over previous
"""GATWithSentenceEmbedding Trainium2 kernel (8 NeuronCores, SPMD + collectives).

V2 restructure vs baseline:
  - Phase B (fcl/fce/fc2 streaming) uses rhs-side weight tiles with [1,N]
    matvec psums: weights stream row-major straight from DRAM (no rearrange,
    no per-j Ldweights storm).  Stream DMAs dispatch on the Pool engine so
    they never stall behind compute-gated dispatches.
  - Phase A: host stages x^T, so h^T and xp1 are computed with zero on-device
    transposes.  Small weights are host-cast to fp16 (halves DMA + removes
    DVE convert passes).
  - conv1 gather tiles (xs/ad/oh) are prefetched into SBUF during the weight
    stream; only the mask-dependent work runs after the logits AllReduce.
  - BN stats use AllGather + local ones-matmul combine instead of AllReduce
    (15.4us vs 28.2us each).
  - conv2 reuses conv1's one-hot tiles.
"""

import numpy as np
from contextlib import ExitStack

import concourse.bass as bass
import concourse.mybir as mybir
import concourse.tile as tile
from concourse import bacc
from concourse.bass_utils import run_bass_kernel_spmd
from concourse.masks import make_identity

N, F, HC, S, H, E, BERT = 2048, 256, 256, 512, 4, 8192, 768
NCORES = 8
P = 128
NCHUNK = N // NCORES          # 256 dst nodes per core
ECH = E // NCORES             # 1024 g1/g2 columns per core
XP1W = H * HC + 2 * H         # 1032 = xp1 | al_s | al_d
XP2W = F + 2                  # 258  = xp2 | al_s | al_d
HC2 = HC // 2                 # 128
BIG = 1.0e9

dt = mybir.dt
AF = mybir.ActivationFunctionType
ALU = mybir.AluOpType
RG = [list(range(NCORES))]

_cache = {}
last_in_maps = None
DEBUG = False
TRACE = False
last_results = None


def _build(nt_g: int, nt_m: int, debug: bool = False, stage: int = 4):
    pad_g = nt_g * P
    pad_m = nt_m * P
    nc = bacc.Bacc("TRN2", target_bir_lowering=False, debug=False)

    def inp(name, shape, dtype=dt.float32):
        return nc.dram_tensor(name, shape, dtype, kind="ExternalInput")

    # shared inputs (host-cast fp16 where precision allows)
    xT_in = inp("xT", [F, N], dt.float16)
    sent_in = inp("sent_emb", [BERT])
    elp_in = inp("elp", [E])
    fc0_w = inp("fc0_w", [BERT, S], dt.float16)
    fc0_b = inp("fc0_b", [S])
    fc1_w = inp("fc1_w", [F, S], dt.float16)
    fc1_b = inp("fc1_b", [S])
    c1w = inp("conv1_W", [S, H * HC], dt.float16)
    c1a = inp("conv1_a", [2 * H * HC], dt.float16)
    c1b = inp("conv1_b", [H * HC])
    c2w = inp("conv2_W", [H * HC, F], dt.float16)
    c2a = inp("conv2_a", [2 * F], dt.float16)
    c2b = inp("conv2_b", [F])
    m1w = inp("mlp1_w", [4 * F, HC], dt.float16)
    m1b = inp("mlp1_b", [HC])
    bn1g = inp("bn1_g", [HC]); bn1b = inp("bn1_b", [HC])
    m2w = inp("mlp2_w", [HC, HC2], dt.float16)
    m2b = inp("mlp2_b", [HC2])
    bn2g = inp("bn2_g", [HC2]); bn2b = inp("bn2_b", [HC2])
    m3w = inp("mlp3_w", [HC2, 1], dt.float16)
    m3b = inp("mlp3_b", [1])
    fc2_b = inp("fc2_b", [E])
    # per-core inputs (row-major, streamed as rhs tiles).  Each big matrix is
    # host-split into fp16 hi + fp16 lo (lo = x - fp16(x)); the matvec runs
    # hi*Whi + lo*Whi + hi*Wlo at fp16 PE rate (4x the fp32 rate) with
    # ~2^-22 effective precision (lo*Wlo dropped).
    fclw_hl = inp("fclw_hl", [E, 2 * ECH], dt.float16)
    fclb_sh = inp("fclb_sh", [ECH])
    fcew_hl = inp("fcew_hl", [BERT, 2 * ECH], dt.float16)
    fceb_sh = inp("fceb_sh", [ECH])
    fc2w_hl = inp("fc2w_hl", [2 * ECH, 2 * E], dt.float16)
    elp_hi = inp("elp_hi", [E], dt.float16)
    elp_lo = inp("elp_lo", [E], dt.float16)
    semb_hi = inp("semb_hi", [BERT], dt.float16)
    semb_lo = inp("semb_lo", [BERT], dt.float16)
    g_src = inp("g_src", [pad_g], dt.int32)
    g_dst = inp("g_dst", [pad_g], dt.int32)
    g_lidx = inp("g_lidx", [pad_g], dt.int32)
    g_oh = inp("g_oh", [pad_g, NCHUNK], dt.float16)
    m_src = inp("m_src", [pad_m], dt.int32)
    m_dst = inp("m_dst", [pad_m], dt.int32)
    m_lidx = inp("m_lidx", [pad_m], dt.int32)
    # outputs
    orig_out = nc.dram_tensor("orig_out", [E], dt.float32, kind="ExternalOutput")
    score_out = nc.dram_tensor("score_out", [pad_m], dt.float32,
                               kind="ExternalOutput")

    def bcast(dram_handle, cols, offset=0):
        """AP reading a [1, cols] DRAM row replicated over 128 partitions."""
        return bass.AP(tensor=dram_handle.ap().tensor, offset=offset,
                       ap=[[0, P], [1, cols]])

    def bcast_ap(ap_tile, cols, offset=0):
        a = ap_tile[:] if not isinstance(ap_tile, bass.AP) else ap_tile
        return bass.AP(tensor=a.tensor, offset=a.offset + offset,
                       ap=[[0, P], [1, cols]])

    with tile.TileContext(nc) as tc:
        with (
            tc.tile_pool(name="dram", bufs=1, space="DRAM") as dram,
            tc.tile_pool(name="single", bufs=1) as single,
            tc.tile_pool(name="sb", bufs=2) as sb,
            tc.tile_pool(name="keep", bufs=1) as keep,
        ):
            ident_h = single.tile([P, P], dt.float16)
            make_identity(nc, ident_h[:])

            # internal DRAM
            xp1_dram = dram.tile([N, XP1W], dt.float16)
            al1d_dram = dram.tile([N, 2 * H], dt.float16)
            fc2part = dram.tile([E], dt.float32)
            logits_dram = dram.tile([E], dt.float32, addr_space="Shared")
            lext_dram = dram.tile([E + 2, 1], dt.float32)
            sent_dram = dram.tile([S], dt.float32)
            gd_dram = dram.tile([2 * ECH], dt.float32)
            xp2_in = dram.tile([NCHUNK, XP2W], dt.float16)
            xp2_dram = dram.tile([N, XP2W], dt.float16, addr_space="Shared")
            h2_in = dram.tile([NCHUNK, F], dt.float16)
            h2_dram = dram.tile([N, F], dt.float16, addr_space="Shared")
            st1_in = dram.tile([520], dt.float32)
            st1_all = dram.tile([8 * 520], dt.float32, addr_space="Shared")
            st2_in = dram.tile([2 * HC2], dt.float32)
            st2_all = dram.tile([8 * 2 * HC2], dt.float32, addr_space="Shared")
            row_dram = dram.tile([4 * HC], dt.float32)  # gs/gb rows for bcast

            # =============== phase A: inputs + hT + xp1 (SP engine DMAs) ====
            # Phases A and B overlap at runtime, sharing 8 PSUM banks via two
            # tags: ps_big [128,2048] (sent-free; hT then xp1 cycle it) and
            # ps_row [1,2048] (sent, g1, g2, then fc2 blocks cycle it).
            esA = ExitStack()
            sbA = esA.enter_context(tc.tile_pool(name="sbA", bufs=2))
            psAB = esA.enter_context(
                tc.tile_pool(name="psAB", bufs=1, space="PSUM"))

            def row_ps():
                return psAB.tile([1, 2048], dt.float32, space="PSUM",
                                 tag="ps_row", bufs=1, name="ps_row")

            def big_ps():
                return psAB.tile([P, N], dt.float32, space="PSUM",
                                 tag="ps_big", bufs=1, name="ps_big")

            xT_t = [sbA.tile([P, N], dt.float16, tag=f"xT{k}", bufs=1,
                             name=f"xT{k}") for k in range(F // P)]
            for k in range(F // P):
                nc.sync.dma_start(xT_t[k][:], xT_in[k * P:(k + 1) * P, :])
            fc1w_t = [sbA.tile([P, S], dt.float16, tag=f"fc1w{k}", bufs=1,
                               name=f"fc1w{k}") for k in range(F // P)]
            for k in range(F // P):
                nc.sync.dma_start(fc1w_t[k][:], fc1_w[k * P:(k + 1) * P, :])
            fc0w_t = [sbA.tile([P, S], dt.float16, tag=f"fc0w{k}", bufs=1,
                               name=f"fc0w{k}") for k in range(BERT // P)]
            for k in range(BERT // P):
                nc.sync.dma_start(fc0w_t[k][:], fc0_w[k * P:(k + 1) * P, :])
            semb32 = single.tile([P, BERT // P], dt.float32)
            nc.sync.dma_start(semb32[:],
                              sent_in.ap().rearrange("(k p) -> p k", p=P))
            semb16 = single.tile([P, BERT // P], dt.float16)
            nc.vector.tensor_copy(semb16[:], semb32[:])
            elp_hc = single.tile([P, E // P], dt.float16)
            nc.sync.dma_start(elp_hc[:],
                              elp_hi.ap().rearrange("(k p) -> p k", p=P))
            elp_lc = single.tile([P, E // P], dt.float16)
            nc.sync.dma_start(elp_lc[:],
                              elp_lo.ap().rearrange("(k p) -> p k", p=P))
            semb_hc = single.tile([P, BERT // P], dt.float16)
            nc.sync.dma_start(semb_hc[:],
                              semb_hi.ap().rearrange("(k p) -> p k", p=P))
            semb_lc = single.tile([P, BERT // P], dt.float16)
            nc.sync.dma_start(semb_lc[:],
                              semb_lo.ap().rearrange("(k p) -> p k", p=P))

            # sent = relu(sent_emb @ fc0_w + fc0_b) as a [1, S] row
            ps_sent = row_ps()
            for k in range(BERT // P):
                nc.tensor.matmul(ps_sent[:, 0:S], lhsT=semb16[:, k:k + 1],
                                 rhs=fc0w_t[k][:],
                                 start=(k == 0), stop=(k == BERT // P - 1))
            sent_row = sbA.tile([1, S], dt.float32, tag="sentrow", bufs=1)
            b_row = sbA.tile([1, S], dt.float32, tag="fc0brow", bufs=1)
            nc.sync.dma_start(b_row[:], fc0_b[None, :])
            nc.vector.tensor_add(sent_row[:], ps_sent[:, 0:S], b_row[:])
            nc.scalar.activation(sent_row[:], sent_row[:], AF.Relu)
            nc.sync.dma_start(sent_dram[:][None, :], sent_row[:])
            sent_col = single.tile([P, S // P], dt.float32)
            nc.sync.dma_start(sent_col[:],
                              sent_dram[:].rearrange("(k p) -> p k", p=P))
            fc1b_col = single.tile([P, S // P], dt.float32)
            nc.sync.dma_start(fc1b_col[:],
                              fc1_b.ap().rearrange("(k p) -> p k", p=P))

            # hT[s, n] = relu(xT^T-free GEMM) + sent, fully transpose-free
            hT_t = [sbA.tile([P, N], dt.float16, tag=f"hT{s}", bufs=1,
                             name=f"hT{s}") for s in range(S // P)]
            for si in range(S // P):
                ps_hT = big_ps()
                for k in range(F // P):
                    for q0 in range(0, N, 512):
                        nc.tensor.matmul(
                            ps_hT[:, q0:q0 + 512],
                            lhsT=fc1w_t[k][:, si * P:(si + 1) * P],
                            rhs=xT_t[k][:, q0:q0 + 512],
                            start=(k == 0), stop=(k == F // P - 1))
                nc.vector.tensor_tensor(
                    hT_t[si][:], ps_hT[:],
                    fc1b_col[:, si:si + 1].to_broadcast([P, N]), op=ALU.add)
                nc.scalar.activation(hT_t[si][:], hT_t[si][:], AF.Relu)
                nc.vector.tensor_tensor(
                    hT_t[si][:], hT_t[si][:],
                    sent_col[:, si:si + 1].to_broadcast([P, N]), op=ALU.add)

            # W1aug = [conv1_W | W@a_src | W@a_dst] as 4 k-tiles [128, 1032] f16
            c1a_bc = sbA.tile([P, 2 * H * HC], dt.float16, tag="c1abc", bufs=1)
            nc.sync.dma_start(c1a_bc[:], bcast(c1a, 2 * H * HC))
            w1aug_h = [sbA.tile([P, XP1W], dt.float16, tag=f"w1aug{k}", bufs=1,
                                name=f"w1aug{k}") for k in range(S // P)]
            lp = nc.allow_low_precision(
                "fp16 a-vector projections only shape GAT softmax logits")
            lp.__enter__()
            for k in range(S // P):
                nc.sync.dma_start(w1aug_h[k][:, 0:H * HC],
                                  c1w[k * P:(k + 1) * P, :])
                tmp = sbA.tile([P, H * HC], dt.float16, tag="scratch4k")
                nc.vector.tensor_mul(tmp[:], w1aug_h[k][:, 0:H * HC],
                                     c1a_bc[:, 0:H * HC])
                for h in range(H):
                    nc.vector.reduce_sum(
                        w1aug_h[k][:, H * HC + h:H * HC + h + 1],
                        tmp[:, h * HC:(h + 1) * HC], axis=mybir.AxisListType.X)
                nc.vector.tensor_mul(tmp[:], w1aug_h[k][:, 0:H * HC],
                                     c1a_bc[:, H * HC:2 * H * HC])
                for h in range(H):
                    nc.vector.reduce_sum(
                        w1aug_h[k][:, H * HC + H + h:H * HC + H + h + 1],
                        tmp[:, h * HC:(h + 1) * HC], axis=mybir.AxisListType.X)

            # xp1 = h @ W1aug per node-tile; lhsT = hT slice (no transposes)

            def emit_xp1_tile(nt):
                ps_xp1 = big_ps()
                for si in range(S // P):
                    for s0, s1 in ((0, 512), (512, 1024), (1024, XP1W)):
                        nc.tensor.matmul(
                            ps_xp1[:, s0:s1],
                            lhsT=hT_t[si][:, nt * P:(nt + 1) * P],
                            rhs=w1aug_h[si][:, s0:s1],
                            start=(si == 0), stop=(si == S // P - 1))
                xp1_t = sbA.tile([P, XP1W], dt.float16, tag="xp1")
                nc.vector.tensor_copy(xp1_t[:], ps_xp1[:, 0:XP1W])
                nc.sync.dma_start(xp1_dram[nt * P:(nt + 1) * P, :], xp1_t[:])
                nc.sync.dma_start(al1d_dram[nt * P:(nt + 1) * P, :],
                                  xp1_t[:, H * HC:H * HC + 2 * H])

            # ======== phase B: weight streaming (Pool engine DMAs) ========
            sbB = esA.enter_context(tc.tile_pool(name="sbB", bufs=2))

            # g1 = relu(elp @ fcl_w + b): [1, ECH] psum row, hi/lo passes
            ps_g1 = row_ps()
            for k in range(E // P):
                wt = sbB.tile([P, 2 * ECH], dt.float16, tag="wfcl", bufs=3,
                              name="wfcl")
                nc.gpsimd.dma_start(wt[:], fclw_hl[k * P:(k + 1) * P, :])
                for q0 in range(0, ECH, 512):
                    nc.tensor.matmul(ps_g1[:, q0:q0 + 512],
                                     lhsT=elp_hc[:, k:k + 1],
                                     rhs=wt[:, q0:q0 + 512],
                                     start=(k == 0), stop=False)
                    nc.tensor.matmul(ps_g1[:, q0:q0 + 512],
                                     lhsT=elp_lc[:, k:k + 1],
                                     rhs=wt[:, q0:q0 + 512],
                                     start=False, stop=False)
                    nc.tensor.matmul(ps_g1[:, q0:q0 + 512],
                                     lhsT=elp_hc[:, k:k + 1],
                                     rhs=wt[:, ECH + q0:ECH + q0 + 512],
                                     start=False, stop=(k == E // P - 1))
            g1_row = sbB.tile([1, ECH], dt.float32, tag="g1row", bufs=1,
                              name="g1row")
            gb_row = sbB.tile([1, ECH], dt.float32, tag="gbrow", bufs=1)
            nc.sync.dma_start(gb_row[:], fclb_sh[None, :])
            nc.vector.tensor_add(g1_row[:], ps_g1[:, 0:ECH], gb_row[:])
            nc.scalar.activation(g1_row[:], g1_row[:], AF.Relu)
            nc.sync.dma_start(gd_dram[0:ECH][None, :], g1_row[:])

            # g2 = relu(sent_emb @ fce_w + b)
            ps_g2 = row_ps()
            for k in range(BERT // P):
                wt = sbB.tile([P, 2 * ECH], dt.float16, tag="wfcl", bufs=3,
                              name="wfce")
                nc.gpsimd.dma_start(wt[:], fcew_hl[k * P:(k + 1) * P, :])
                for q0 in range(0, ECH, 512):
                    nc.tensor.matmul(ps_g2[:, q0:q0 + 512],
                                     lhsT=semb_hc[:, k:k + 1],
                                     rhs=wt[:, q0:q0 + 512],
                                     start=(k == 0), stop=False)
                    nc.tensor.matmul(ps_g2[:, q0:q0 + 512],
                                     lhsT=semb_lc[:, k:k + 1],
                                     rhs=wt[:, q0:q0 + 512],
                                     start=False, stop=False)
                    nc.tensor.matmul(ps_g2[:, q0:q0 + 512],
                                     lhsT=semb_hc[:, k:k + 1],
                                     rhs=wt[:, ECH + q0:ECH + q0 + 512],
                                     start=False, stop=(k == BERT // P - 1))
            g2_row = sbB.tile([1, ECH], dt.float32, tag="g2row", bufs=1,
                              name="g2row")
            gb2_row = sbB.tile([1, ECH], dt.float32, tag="gbrow", bufs=1)
            nc.sync.dma_start(gb2_row[:], fceb_sh[None, :])
            nc.vector.tensor_add(g2_row[:], ps_g2[:, 0:ECH], gb2_row[:])
            nc.scalar.activation(g2_row[:], g2_row[:], AF.Relu)
            nc.sync.dma_start(gd_dram[ECH:2 * ECH][None, :], g2_row[:])

            # g hi/lo split done in tiny [128, 16] column space
            g_col32 = single.tile([P, 2 * ECH // P], dt.float32)
            nc.sync.dma_start(g_col32[:],
                              gd_dram[:].rearrange("(k p) -> p k", p=P))
            g_colh = single.tile([P, 2 * ECH // P], dt.float16)
            nc.vector.tensor_copy(g_colh[:], g_col32[:])
            g_colh32 = single.tile([P, 2 * ECH // P], dt.float32)
            nc.vector.tensor_copy(g_colh32[:], g_colh[:])
            g_coll32 = single.tile([P, 2 * ECH // P], dt.float32)
            nc.vector.tensor_sub(g_coll32[:], g_col32[:], g_colh32[:])
            g_coll = single.tile([P, 2 * ECH // P], dt.float16)
            nc.vector.tensor_copy(g_coll[:], g_coll32[:])

            # fc2 partial: column-block streaming, xp1 tiles interleaved so
            # the PE fills DMA-pacing slack without stalling the stream.
            CB = 2048
            NXB = (N // P) // (E // CB)
            for c in range(E // CB):
                ps_f2 = row_ps()
                for k in range(2 * ECH // P):
                    wt = sbB.tile([P, 2 * CB], dt.float16, tag="wfc2", bufs=3,
                                  name="wfc2")
                    nc.gpsimd.dma_start(
                        wt[:], fc2w_hl[k * P:(k + 1) * P,
                                       2 * c * CB:2 * (c + 1) * CB])
                    for q0 in range(0, CB, 512):
                        nc.tensor.matmul(ps_f2[:, q0:q0 + 512],
                                         lhsT=g_colh[:, k:k + 1],
                                         rhs=wt[:, q0:q0 + 512],
                                         start=(k == 0), stop=False)
                        nc.tensor.matmul(ps_f2[:, q0:q0 + 512],
                                         lhsT=g_coll[:, k:k + 1],
                                         rhs=wt[:, q0:q0 + 512],
                                         start=False, stop=False)
                        nc.tensor.matmul(ps_f2[:, q0:q0 + 512],
                                         lhsT=g_colh[:, k:k + 1],
                                         rhs=wt[:, CB + q0:CB + q0 + 512],
                                         start=False,
                                         stop=(k == 2 * ECH // P - 1))
                f2row = sbB.tile([1, CB], dt.float32, tag="f2row", bufs=1)
                nc.vector.tensor_copy(f2row[:], ps_f2[:, 0:CB])
                nc.sync.dma_start(fc2part[c * CB:(c + 1) * CB][None, :],
                                  f2row[:])
                for nt in range(c * NXB, (c + 1) * NXB):
                    emit_xp1_tile(nt)

            # ======== conv1 gather prefetch (during fc2 stream) ========
            gsrc_sb = single.tile([P, nt_g], dt.int32)
            nc.sync.dma_start(gsrc_sb[:],
                              g_src.ap().rearrange("(t p) -> p t", p=P))
            gdst_sb = single.tile([P, nt_g], dt.int32)
            nc.sync.dma_start(gdst_sb[:],
                              g_dst.ap().rearrange("(t p) -> p t", p=P))
            glidx_sb = single.tile([P, nt_g], dt.int32)
            nc.sync.dma_start(glidx_sb[:],
                              g_lidx.ap().rearrange("(t p) -> p t", p=P))
            oh_t = [keep.tile([P, NCHUNK], dt.float16, tag=f"oh{t}",
                              name=f"oh{t}") for t in range(nt_g)]
            xs_t = [keep.tile([P, XP1W], dt.float16, tag=f"xs{t}",
                              name=f"xs{t}") for t in range(nt_g)]
            ad_t = [keep.tile([P, 2 * H], dt.float16, tag=f"ad{t}",
                              name=f"ad{t}") for t in range(nt_g)]
            for t in range(nt_g):
                nc.sync.dma_start(oh_t[t][:], g_oh[t * P:(t + 1) * P, :])
                nc.gpsimd.indirect_dma_start(
                    out=xs_t[t][:], out_offset=None, in_=xp1_dram[:],
                    in_offset=bass.IndirectOffsetOnAxis(
                        ap=gsrc_sb[:, t:t + 1], axis=0))
                nc.gpsimd.indirect_dma_start(
                    out=ad_t[t][:], out_offset=None, in_=al1d_dram[:],
                    in_offset=bass.IndirectOffsetOnAxis(
                        ap=gdst_sb[:, t:t + 1], axis=0))
            esA.close()

            if stage >= 2:
                nc.gpsimd.collective_compute(
                    "AllReduce", ALU.add, replica_groups=RG,
                    ins=[fc2part[:]], outs=[logits_dram[:]])
                # logits += fc2_b ; orig_out ; logits_ext
                lg_pf = single.tile([P, E // P], dt.float32)
                nc.sync.dma_start(lg_pf[:],
                                  logits_dram[:].rearrange("(p f) -> p f", p=P))
                f2b_pf = single.tile([P, E // P], dt.float32)
                nc.sync.dma_start(f2b_pf[:],
                                  fc2_b.ap().rearrange("(p f) -> p f", p=P))
                nc.vector.tensor_add(lg_pf[:], lg_pf[:], f2b_pf[:])
                nc.sync.dma_start(orig_out.ap().rearrange("(p f) -> p f", p=P),
                                  lg_pf[:])
                nc.sync.dma_start(
                    lext_dram[0:E, :].rearrange("(p f) x -> p (f x)", p=P),
                    lg_pf[:])
                big_t = single.tile([1, 2], dt.float32)
                nc.vector.memset(big_t[:, 0:1], BIG)
                nc.vector.memset(big_t[:, 1:2], -BIG)
                nc.sync.dma_start(lext_dram[E:E + 2, 0][None, :], big_t[:])

                # ============ conv1 aggregation (mask-dependent part) =======
                valid_t = [keep.tile([P, 1], dt.float32, tag=f"valid{t}",
                                     name=f"valid{t}") for t in range(nt_g)]
                esC = ExitStack()
                sbC = esC.enter_context(tc.tile_pool(name="sbC", bufs=2))
                psC = esC.enter_context(
                    tc.tile_pool(name="psC", bufs=1, space="PSUM"))
                ps_msg = [psC.tile([P, H * HC], dt.float32, space="PSUM",
                                   tag=f"ps_msg{d}", name=f"ps_msg{d}")
                          for d in range(2)]
                ps_den = [psC.tile([P, H], dt.float32, space="PSUM",
                                   tag=f"ps_den{d}", name=f"ps_den{d}")
                          for d in range(2)]
                for t in range(nt_g):
                    lg = sbC.tile([P, 1], dt.float32, tag="glg", bufs=6)
                    nc.gpsimd.indirect_dma_start(
                        out=lg[:], out_offset=None, in_=lext_dram[:],
                        in_offset=bass.IndirectOffsetOnAxis(
                            ap=glidx_sb[:, t:t + 1], axis=0))
                    nc.vector.tensor_scalar(valid_t[t][:], lg[:], 0.0, None,
                                            op0=ALU.is_gt)
                    alpha = sbC.tile([P, H], dt.float32, tag="alpha", bufs=4)
                    nc.vector.tensor_add(alpha[:],
                                         xs_t[t][:, H * HC:H * HC + H],
                                         ad_t[t][:, H:2 * H])
                    nc.vector.scalar_tensor_tensor(alpha[:], alpha[:], 0.2,
                                                   alpha[:],
                                                   op0=ALU.mult, op1=ALU.max)
                    ex = sbC.tile([P, H], dt.float32, tag="ex", bufs=4)
                    nc.scalar.activation(ex[:], alpha[:], AF.Exp)
                    nc.vector.tensor_mul(ex[:], ex[:],
                                         valid_t[t][:].to_broadcast([P, H]))
                    msg = sbC.tile([P, H * HC + H], dt.float16, tag="msg",
                                   bufs=4)
                    for h in range(H):
                        nc.vector.tensor_tensor(
                            msg[:, h * HC:(h + 1) * HC],
                            xs_t[t][:, h * HC:(h + 1) * HC],
                            ex[:, h:h + 1].to_broadcast([P, HC]), op=ALU.mult)
                    nc.vector.tensor_copy(msg[:, H * HC:H * HC + H], ex[:])
                    for d in range(2):
                        lhsT = oh_t[t][:, d * P:(d + 1) * P]
                        st, sp = (t == 0), (t == nt_g - 1)
                        nc.tensor.matmul(ps_msg[d][:, 0:512], lhsT=lhsT,
                                         rhs=msg[:, 0:512], start=st, stop=sp)
                        nc.tensor.matmul(ps_msg[d][:, 512:1024], lhsT=lhsT,
                                         rhs=msg[:, 512:1024], start=st,
                                         stop=sp)
                        nc.tensor.matmul(ps_den[d][:], lhsT=lhsT,
                                         rhs=msg[:, H * HC:H * HC + H],
                                         start=st, stop=sp)
                # finalize conv1 (+elu) and xp2aug weights
                c1b_bc = sbC.tile([P, H * HC], dt.float32, tag="c1bbc", bufs=1)
                nc.sync.dma_start(c1b_bc[:], bcast(c1b, H * HC))
                c2a_bc = sbC.tile([P, 2 * F], dt.float16, tag="c2abc", bufs=1)
                nc.sync.dma_start(c2a_bc[:], bcast(c2a, 2 * F))
                w2aug_h = [keep.tile([P, XP2W], dt.float16, tag=f"w2aug{k}",
                                     name=f"w2aug{k}")
                           for k in range(H * HC // P)]
                for k in range(H * HC // P):
                    nc.sync.dma_start(w2aug_h[k][:, 0:F],
                                      c2w[k * P:(k + 1) * P, :])
                    tmp = sbC.tile([P, F], dt.float16, tag="w2tmp")
                    nc.vector.tensor_mul(tmp[:], w2aug_h[k][:, 0:F],
                                         c2a_bc[:, 0:F])
                    nc.vector.reduce_sum(w2aug_h[k][:, F:F + 1], tmp[:],
                                         axis=mybir.AxisListType.X)
                    nc.vector.tensor_mul(tmp[:], w2aug_h[k][:, 0:F],
                                         c2a_bc[:, F:2 * F])
                    nc.vector.reduce_sum(w2aug_h[k][:, F + 1:F + 2], tmp[:],
                                         axis=mybir.AxisListType.X)
                h1_keep = [keep.tile([P, H * HC], dt.float16, tag=f"h1k{d}",
                                     name=f"h1k{d}") for d in range(2)]
                for d in range(2):
                    denr = sbC.tile([P, H], dt.float32, tag="denr")
                    nc.vector.reciprocal(denr[:], ps_den[d][:])
                    h1_t = h1_keep[d]
                    h1f = sbC.tile([P, H * HC], dt.float32, tag="h1f")
                    for h in range(H):
                        nc.vector.scalar_tensor_tensor(
                            h1f[:, h * HC:(h + 1) * HC],
                            ps_msg[d][:, h * HC:(h + 1) * HC],
                            denr[:, h:h + 1],
                            c1b_bc[:, h * HC:(h + 1) * HC],
                            op0=ALU.mult, op1=ALU.add)
                    # elu = relu(x) + exp(min(x,0)) - 1
                    relu_t = sbC.tile([P, H * HC], dt.float32, tag="elu_r")
                    nc.scalar.activation(relu_t[:], h1f[:], AF.Relu)
                    nc.vector.tensor_scalar_min(h1f[:], h1f[:], 0.0)
                    nc.scalar.activation(h1f[:], h1f[:], AF.Exp)
                    nc.vector.scalar_tensor_tensor(h1_t[:], h1f[:], -1.0,
                                                   relu_t[:],
                                                   op0=ALU.add, op1=ALU.add)
                esC.close()
                esD = ExitStack()
                sbD = esD.enter_context(tc.tile_pool(name="sbD", bufs=2))
                psD = esD.enter_context(
                    tc.tile_pool(name="psD", bufs=1, space="PSUM"))
                for d in range(2):
                    h1_t = h1_keep[d]
                    ps_xp2 = psD.tile([P, XP2W], dt.float32, space="PSUM",
                                      tag="ps_xp2")
                    for k in range(H * HC // P):
                        ps_h1t = psD.tile([P, P], dt.float16, space="PSUM",
                                          tag="ps_xth", bufs=2)
                        nc.tensor.transpose(ps_h1t[:],
                                            h1_t[:, k * P:(k + 1) * P],
                                            ident_h[:])
                        h1T = sb.tile([P, P], dt.float16, tag="xTh")
                        nc.vector.tensor_copy(h1T[:], ps_h1t[:])
                        nc.tensor.matmul(ps_xp2[:], lhsT=h1T[:],
                                         rhs=w2aug_h[k][:],
                                         start=(k == 0),
                                         stop=(k == H * HC // P - 1))
                    xp2_t = sbD.tile([P, XP2W], dt.float16, tag="xp2")
                    nc.vector.tensor_copy(xp2_t[:], ps_xp2[:])
                    nc.sync.dma_start(xp2_in[d * P:(d + 1) * P, :], xp2_t[:])
                esD.close()
                nc.gpsimd.collective_compute(
                    "AllGather", ALU.bypass, replica_groups=RG,
                    ins=[xp2_in[:]], outs=[xp2_dram[:]])

            if stage >= 3:
                # ============ conv2 aggregation (reuses oh tiles) ============
                esE = ExitStack()
                sbE = esE.enter_context(tc.tile_pool(name="sbE", bufs=2))
                psE = esE.enter_context(
                    tc.tile_pool(name="psE", bufs=1, space="PSUM"))
                ps_m2 = [psE.tile([P, F + 1], dt.float32, space="PSUM",
                                  tag=f"ps_m2{d}", name=f"ps_m2{d}")
                         for d in range(2)]
                for t in range(nt_g):
                    xs2 = sbE.tile([P, XP2W], dt.float16, tag="xs2", bufs=4)
                    nc.gpsimd.indirect_dma_start(
                        out=xs2[:], out_offset=None, in_=xp2_dram[:],
                        in_offset=bass.IndirectOffsetOnAxis(
                            ap=gsrc_sb[:, t:t + 1], axis=0))
                    xd2 = sbE.tile([P, XP2W], dt.float16, tag="xd2", bufs=4)
                    nc.gpsimd.indirect_dma_start(
                        out=xd2[:], out_offset=None, in_=xp2_dram[:],
                        in_offset=bass.IndirectOffsetOnAxis(
                            ap=gdst_sb[:, t:t + 1], axis=0))
                    alpha2 = sbE.tile([P, 1], dt.float32, tag="alpha2", bufs=4)
                    nc.vector.tensor_add(alpha2[:], xs2[:, F:F + 1],
                                         xd2[:, F + 1:F + 2])
                    nc.vector.scalar_tensor_tensor(alpha2[:], alpha2[:], 0.2,
                                                   alpha2[:],
                                                   op0=ALU.mult, op1=ALU.max)
                    ex2 = sbE.tile([P, 1], dt.float32, tag="ex2", bufs=4)
                    nc.scalar.activation(ex2[:], alpha2[:], AF.Exp)
                    nc.vector.tensor_mul(ex2[:], ex2[:], valid_t[t][:])
                    msg2 = sbE.tile([P, F + 1], dt.float16, tag="msg2", bufs=4)
                    nc.vector.tensor_tensor(msg2[:, 0:F], xs2[:, 0:F],
                                            ex2[:].to_broadcast([P, F]),
                                            op=ALU.mult)
                    nc.vector.tensor_copy(msg2[:, F:F + 1], ex2[:])
                    for d in range(2):
                        lhsT = oh_t[t][:, d * P:(d + 1) * P]
                        st, sp = (t == 0), (t == nt_g - 1)
                        nc.tensor.matmul(ps_m2[d][:], lhsT=lhsT, rhs=msg2[:],
                                         start=st, stop=sp)
                c2b_bc = sbE.tile([P, F], dt.float32, tag="c2bbc", bufs=1)
                nc.sync.dma_start(c2b_bc[:], bcast(c2b, F))
                for d in range(2):
                    d2r = sbE.tile([P, 1], dt.float32, tag="d2r")
                    nc.vector.reciprocal(d2r[:], ps_m2[d][:, F:F + 1])
                    h2_t = sbE.tile([P, F], dt.float16, tag="h2")
                    nc.vector.tensor_tensor(h2_t[:], ps_m2[d][:, 0:F],
                                            d2r[:].to_broadcast([P, F]),
                                            op=ALU.mult)
                    nc.vector.tensor_add(h2_t[:], h2_t[:], c2b_bc[:])
                    nc.sync.dma_start(h2_in[d * P:(d + 1) * P, :], h2_t[:])
                esE.close()
                nc.gpsimd.collective_compute(
                    "AllGather", ALU.bypass, replica_groups=RG,
                    ins=[h2_in[:]], outs=[h2_dram[:]])

            if stage >= 4:
                # ============ edge MLP ============
                msrc_sb = single.tile([P, nt_m], dt.int32)
                nc.sync.dma_start(msrc_sb[:],
                                  m_src.ap().rearrange("(t p) -> p t", p=P))
                mdst_sb = single.tile([P, nt_m], dt.int32)
                nc.sync.dma_start(mdst_sb[:],
                                  m_dst.ap().rearrange("(t p) -> p t", p=P))
                mlidx_sb = single.tile([P, nt_m], dt.int32)
                nc.sync.dma_start(mlidx_sb[:],
                                  m_lidx.ap().rearrange("(t p) -> p t", p=P))
                m1w_h = [keep.tile([P, HC], dt.float16, tag=f"m1w{k}",
                                   name=f"m1w{k}") for k in range(4 * F // P)]
                for k in range(4 * F // P):
                    nc.sync.dma_start(m1w_h[k][:], m1w[k * P:(k + 1) * P, :])
                m1b_bc = single.tile([P, HC], dt.float32)
                nc.sync.dma_start(m1b_bc[:], bcast(m1b, HC))
                mask_f = [keep.tile([P, 1], dt.float32, tag=f"maskf{t}",
                                    name=f"maskf{t}") for t in range(nt_m)]
                mask_u8 = [keep.tile([P, 1], dt.uint8, tag=f"masku{t}",
                                     name=f"masku{t}") for t in range(nt_m)]
                z1_t = [keep.tile([P, HC], dt.float32, tag=f"z1_{t}",
                                  name=f"z1_{t}") for t in range(nt_m)]
                esF = ExitStack()
                sbF = esF.enter_context(tc.tile_pool(name="sbF", bufs=2))
                psF = esF.enter_context(
                    tc.tile_pool(name="psF", bufs=1, space="PSUM"))
                ps_s1 = psF.tile([1, HC], dt.float32, space="PSUM", tag="ps_s1")
                ps_q1 = psF.tile([1, HC], dt.float32, space="PSUM", tag="ps_q1")
                ps_cnt = psF.tile([1, 1], dt.float32, space="PSUM",
                                  tag="ps_cnt")
                for t in range(nt_m):
                    xi = sbF.tile([P, F], dt.float16, tag="xi", bufs=4)
                    nc.gpsimd.indirect_dma_start(
                        out=xi[:], out_offset=None, in_=h2_dram[:],
                        in_offset=bass.IndirectOffsetOnAxis(
                            ap=msrc_sb[:, t:t + 1], axis=0))
                    xj = sbF.tile([P, F], dt.float16, tag="xj", bufs=4)
                    nc.gpsimd.indirect_dma_start(
                        out=xj[:], out_offset=None, in_=h2_dram[:],
                        in_offset=bass.IndirectOffsetOnAxis(
                            ap=mdst_sb[:, t:t + 1], axis=0))
                    lg = sbF.tile([P, 1], dt.float32, tag="mlg", bufs=6)
                    nc.gpsimd.indirect_dma_start(
                        out=lg[:], out_offset=None, in_=lext_dram[:],
                        in_offset=bass.IndirectOffsetOnAxis(
                            ap=mlidx_sb[:, t:t + 1], axis=0))
                    nc.vector.tensor_scalar(mask_f[t][:], lg[:], 0.0, None,
                                            op0=ALU.is_gt)
                    nc.vector.tensor_copy(mask_u8[t][:], mask_f[t][:])
                    dsub = sbF.tile([P, F], dt.float16, tag="dsub", bufs=4)
                    nc.vector.tensor_sub(dsub[:], xi[:], xj[:])
                    nc.scalar.activation(dsub[:], dsub[:], AF.Abs)
                    pmul = sbF.tile([P, F], dt.float16, tag="pmul", bufs=4)
                    nc.vector.tensor_mul(pmul[:], xi[:], xj[:])
                    ps_z1 = psF.tile([P, HC], dt.float32, space="PSUM",
                                     tag="ps_z1", bufs=2)
                    for pi, piece in enumerate((xi, xj, dsub, pmul)):
                        for hf in range(2):
                            ps_t = psF.tile([P, P], dt.float16, space="PSUM",
                                            tag="ps_xth", bufs=2)
                            nc.tensor.transpose(ps_t[:],
                                                piece[:, hf * P:(hf + 1) * P],
                                                ident_h[:])
                            efT = sb.tile([P, P], dt.float16, tag="xTh")
                            nc.vector.tensor_copy(efT[:], ps_t[:])
                            k = pi * 2 + hf
                            nc.tensor.matmul(ps_z1[:], lhsT=efT[:],
                                             rhs=m1w_h[k][:],
                                             start=(k == 0), stop=(k == 7))
                    nc.vector.tensor_add(z1_t[t][:], ps_z1[:], m1b_bc[:])
                    zsq = sbF.tile([P, HC], dt.float32, tag="zsq", bufs=4)
                    nc.vector.tensor_mul(zsq[:], z1_t[t][:], z1_t[t][:])
                    st, sp = (t == 0), (t == nt_m - 1)
                    nc.tensor.matmul(ps_s1[:], lhsT=mask_f[t][:],
                                     rhs=z1_t[t][:], start=st, stop=sp)
                    nc.tensor.matmul(ps_q1[:], lhsT=mask_f[t][:], rhs=zsq[:],
                                     start=st, stop=sp)
                    nc.tensor.matmul(ps_cnt[:], lhsT=mask_f[t][:],
                                     rhs=mask_f[t][:], start=st, stop=sp)
                # pack stats1, AllGather + local combine
                s_sb = sbF.tile([1, HC], dt.float32, tag="stat")
                nc.vector.tensor_copy(s_sb[:], ps_s1[:])
                nc.sync.dma_start(st1_in[None, 0:HC], s_sb[:])
                q_sb = sbF.tile([1, HC], dt.float32, tag="stat")
                nc.vector.tensor_copy(q_sb[:], ps_q1[:])
                nc.sync.dma_start(st1_in[None, HC:2 * HC], q_sb[:])
                c_sb = sbF.tile([1, 1], dt.float32, tag="statc")
                nc.vector.tensor_copy(c_sb[:], ps_cnt[:])
                nc.sync.dma_start(st1_in[None, 2 * HC:2 * HC + 1], c_sb[:])
                zpad = sbF.tile([1, 7], dt.float32, tag="statz")
                nc.vector.memset(zpad[:], 0.0)
                nc.sync.dma_start(st1_in[None, 2 * HC + 1:520], zpad[:])
                esF.close()
                nc.gpsimd.collective_compute(
                    "AllGather", ALU.bypass, replica_groups=RG,
                    ins=[st1_in[:]], outs=[st1_all[:]])

                esG = ExitStack()
                sbG = esG.enter_context(tc.tile_pool(name="sbG", bufs=2))
                psG = esG.enter_context(
                    tc.tile_pool(name="psG", bufs=1, space="PSUM"))

                ones8 = single.tile([8, 1], dt.float32)
                nc.vector.memset(ones8[:], 1.0)

                def combine_stats(st_all, width, tagn):
                    st8 = sbG.tile([8, width], dt.float32, tag=f"st8{tagn}",
                                   name=f"st8{tagn}")
                    nc.sync.dma_start(
                        st8[:], st_all[:].rearrange("(r c) -> r c", r=8))
                    ps_c = psG.tile([1, 520], dt.float32, space="PSUM",
                                    tag="pscmb", bufs=1, name=f"pscmb{tagn}")
                    for q0 in range(0, width, 512):
                        q1 = min(q0 + 512, width)
                        nc.tensor.matmul(ps_c[:, q0:q1], lhsT=ones8[:],
                                         rhs=st8[:, q0:q1], start=True,
                                         stop=True)
                    row = sbG.tile([1, width], dt.float32, tag=f"strow{tagn}",
                                   name=f"strow{tagn}")
                    nc.vector.tensor_copy(row[:], ps_c[:, 0:width])
                    return row

                st1_row = combine_stats(st1_all, 520, "a")

                def bn_rows(st_row, nch, g_in, b_in, row_off):
                    cnt_row = sbG.tile([1, 1], dt.float32, tag="bn_cnt",
                                       name="bn_cnt")
                    nc.vector.tensor_scalar_max(cnt_row[:],
                                                st1_row[:, 512:513], 1.0)
                    cr = sbG.tile([1, 1], dt.float32, tag="bn_cr", name="bn_cr")
                    nc.vector.reciprocal(cr[:], cnt_row[:])
                    mean = sbG.tile([1, nch], dt.float32, tag="bn_mean",
                                    name="bn_mean")
                    nc.vector.tensor_tensor(mean[:], st_row[:, 0:nch],
                                            cr[:].to_broadcast([1, nch]),
                                            op=ALU.mult)
                    var = sbG.tile([1, nch], dt.float32, tag="bn_var",
                                   name="bn_var")
                    nc.vector.tensor_tensor(var[:], st_row[:, nch:2 * nch],
                                            cr[:].to_broadcast([1, nch]),
                                            op=ALU.mult)
                    msq = sbG.tile([1, nch], dt.float32, tag="bn_msq",
                                   name="bn_msq")
                    nc.vector.tensor_mul(msq[:], mean[:], mean[:])
                    nc.vector.tensor_sub(var[:], var[:], msq[:])
                    nc.vector.tensor_scalar_add(var[:], var[:], 1e-5)
                    nc.scalar.activation(var[:], var[:], AF.Sqrt)
                    rstd = sbG.tile([1, nch], dt.float32, tag="bn_rstd",
                                    name="bn_rstd")
                    nc.vector.reciprocal(rstd[:], var[:])
                    g_row = sbG.tile([1, nch], dt.float32, tag="bn_g",
                                     name="bn_g")
                    nc.sync.dma_start(g_row[:], g_in[None, :])
                    b_row2 = sbG.tile([1, nch], dt.float32, tag="bn_b",
                                      name="bn_b")
                    nc.sync.dma_start(b_row2[:], b_in[None, :])
                    gs = sbG.tile([1, nch], dt.float32, tag="bn_gs",
                                  name="bn_gs")
                    nc.vector.tensor_mul(gs[:], g_row[:], rstd[:])
                    gb = sbG.tile([1, nch], dt.float32, tag="bn_gb",
                                  name="bn_gb")
                    nc.vector.tensor_mul(gb[:], mean[:], gs[:])
                    nc.vector.tensor_sub(gb[:], b_row2[:], gb[:])
                    nc.sync.dma_start(row_dram[None, row_off:row_off + nch],
                                      gs[:])
                    nc.sync.dma_start(
                        row_dram[None, row_off + nch:row_off + 2 * nch], gb[:])

                bn_rows(st1_row, HC, bn1g, bn1b, 0)
                gs1_bc = single.tile([P, HC], dt.float32)
                nc.sync.dma_start(gs1_bc[:], bcast_ap(row_dram, HC, 0))
                gb1_bc = single.tile([P, HC], dt.float32)
                nc.sync.dma_start(gb1_bc[:], bcast_ap(row_dram, HC, HC))
                m2w_h = [keep.tile([P, HC2], dt.float16, tag=f"m2w{k}",
                                   name=f"m2w{k}") for k in range(HC // P)]
                for k in range(HC // P):
                    nc.sync.dma_start(m2w_h[k][:], m2w[k * P:(k + 1) * P, :])
                m2b_bc = single.tile([P, HC2], dt.float32)
                nc.sync.dma_start(m2b_bc[:], bcast(m2b, HC2))
                z2_t = [keep.tile([P, HC2], dt.float32, tag=f"z2_{t}",
                                  name=f"z2_{t}") for t in range(nt_m)]
                ps_s2 = psG.tile([1, HC2], dt.float32, space="PSUM",
                                 tag="ps_s2")
                ps_q2 = psG.tile([1, HC2], dt.float32, space="PSUM",
                                 tag="ps_q2")
                for t in range(nt_m):
                    zn = sbG.tile([P, HC], dt.float16, tag="zn", bufs=4)
                    nc.vector.tensor_mul(zn[:], z1_t[t][:], gs1_bc[:])
                    nc.vector.tensor_add(zn[:], zn[:], gb1_bc[:])
                    nc.scalar.activation(zn[:], zn[:], AF.Relu)
                    ps_z2 = psG.tile([P, HC2], dt.float32, space="PSUM",
                                     tag="ps_z2", bufs=1)
                    for k in range(HC // P):
                        ps_t = psG.tile([P, P], dt.float16, space="PSUM",
                                        tag="ps_xth", bufs=2)
                        nc.tensor.transpose(ps_t[:], zn[:, k * P:(k + 1) * P],
                                            ident_h[:])
                        znT = sb.tile([P, P], dt.float16, tag="xTh")
                        nc.vector.tensor_copy(znT[:], ps_t[:])
                        nc.tensor.matmul(ps_z2[:], lhsT=znT[:], rhs=m2w_h[k][:],
                                         start=(k == 0), stop=(k == HC // P - 1))
                    nc.vector.tensor_add(z2_t[t][:], ps_z2[:], m2b_bc[:])
                    zsq2 = sbG.tile([P, HC2], dt.float32, tag="zsq2", bufs=4)
                    nc.vector.tensor_mul(zsq2[:], z2_t[t][:], z2_t[t][:])
                    st, sp = (t == 0), (t == nt_m - 1)
                    nc.tensor.matmul(ps_s2[:], lhsT=mask_f[t][:],
                                     rhs=z2_t[t][:], start=st, stop=sp)
                    nc.tensor.matmul(ps_q2[:], lhsT=mask_f[t][:], rhs=zsq2[:],
                                     start=st, stop=sp)
                s2_sb = sbG.tile([1, HC2], dt.float32, tag="stat2")
                nc.vector.tensor_copy(s2_sb[:], ps_s2[:])
                nc.sync.dma_start(st2_in[None, 0:HC2], s2_sb[:])
                q2_sb = sbG.tile([1, HC2], dt.float32, tag="stat2")
                nc.vector.tensor_copy(q2_sb[:], ps_q2[:])
                nc.sync.dma_start(st2_in[None, HC2:2 * HC2], q2_sb[:])
                nc.gpsimd.collective_compute(
                    "AllGather", ALU.bypass, replica_groups=RG,
                    ins=[st2_in[:]], outs=[st2_all[:]])
                st2_row = combine_stats(st2_all, 2 * HC2, "b")
                bn_rows(st2_row, HC2, bn2g, bn2b, 2 * HC)
                gs2_bc = single.tile([P, HC2], dt.float32)
                nc.sync.dma_start(gs2_bc[:], bcast_ap(row_dram, HC2, 2 * HC))
                gb2_bc = single.tile([P, HC2], dt.float32)
                nc.sync.dma_start(gb2_bc[:],
                                  bcast_ap(row_dram, HC2, 2 * HC + HC2))
                m3w_h = single.tile([P, 1], dt.float16)
                nc.sync.dma_start(m3w_h[:], m3w[:, :])
                m3b_bc = single.tile([P, 1], dt.float32)
                nc.sync.dma_start(m3b_bc[:], bcast(m3b, 1))
                neg25 = single.tile([P, 1], dt.float32)
                nc.vector.memset(neg25[:], -2.5)
                for t in range(nt_m):
                    zn2 = sbG.tile([P, HC2], dt.float16, tag="zn2", bufs=4)
                    nc.vector.tensor_mul(zn2[:], z2_t[t][:], gs2_bc[:])
                    nc.vector.tensor_add(zn2[:], zn2[:], gb2_bc[:])
                    nc.scalar.activation(zn2[:], zn2[:], AF.Relu)
                    ps_t = psG.tile([P, P], dt.float16, space="PSUM",
                                    tag="ps_xth", bufs=2)
                    nc.tensor.transpose(ps_t[:], zn2[:], ident_h[:])
                    znT2 = sb.tile([P, P], dt.float16, tag="xTh")
                    nc.vector.tensor_copy(znT2[:], ps_t[:])
                    ps_sc = psG.tile([P, 1], dt.float32, space="PSUM",
                                     tag="ps_sc", bufs=1)
                    nc.tensor.matmul(ps_sc[:], lhsT=znT2[:], rhs=m3w_h[:],
                                     start=True, stop=True)
                    score = sbG.tile([P, 1], dt.float32, tag="score", bufs=4)
                    nc.vector.tensor_add(score[:], ps_sc[:], m3b_bc[:])
                    sel = sbG.tile([P, 1], dt.float32, tag="sel", bufs=4)
                    nc.vector.select(sel[:], mask_u8[t][:], score[:], neg25[:])
                    nc.scalar.activation(sel[:], sel[:], AF.Sigmoid)
                    nc.sync.dma_start(score_out[t * P:(t + 1) * P][:, None],
                                      sel[:])
                esG.close()

    nc.compile()
    return nc


def kernel(**inputs):
    inputs = {k: np.asarray(v) for k, v in inputs.items()}
    src = inputs["edge_index"][0].astype(np.int64)
    dst = inputs["edge_index"][1].astype(np.int64)

    # --- edge partition by dst chunk (GAT set includes self loops) ---
    all_src = np.concatenate([src, np.arange(N, dtype=np.int64)])
    all_dst = np.concatenate([dst, np.arange(N, dtype=np.int64)])
    lidx_all = np.concatenate(
        [np.arange(E, dtype=np.int64), np.full(N, E, dtype=np.int64)])
    chunk_g = all_dst // NCHUNK
    gids = [np.where(chunk_g == c)[0] for c in range(NCORES)]
    nt_g = int(np.ceil(max(len(i) for i in gids) / P))
    pad_g = nt_g * P
    chunk_m = dst // NCHUNK
    mids = [np.where(chunk_m == c)[0] for c in range(NCORES)]
    nt_m = int(np.ceil(max(len(i) for i in mids) / P))
    pad_m = nt_m * P

    key = (nt_g, nt_m, DEBUG)
    if key not in _cache:
        _cache[key] = _build(nt_g, nt_m, debug=DEBUG)
    nc = _cache[key]

    def f32(v):
        return np.ascontiguousarray(v, dtype=np.float32)

    def f16(v):
        return np.ascontiguousarray(v, dtype=np.float16)

    shared = dict(
        xT=f16(inputs["x"].T),
        sent_emb=f32(inputs["sent_emb"]),
        elp=f32(inputs["edge_logits_param"]),
        elp_hi=f16(inputs["edge_logits_param"]),
        elp_lo=f16(inputs["edge_logits_param"]
                   - inputs["edge_logits_param"].astype(np.float16)
                     .astype(np.float32)),
        semb_hi=f16(inputs["sent_emb"]),
        semb_lo=f16(inputs["sent_emb"]
                    - inputs["sent_emb"].astype(np.float16)
                      .astype(np.float32)),
        fc0_w=f16(inputs["fc0_w"]), fc0_b=f32(inputs["fc0_b"]),
        fc1_w=f16(inputs["fc1_w"]), fc1_b=f32(inputs["fc1_b"]),
        conv1_W=f16(inputs["conv1_W"]),
        conv1_a=f16(np.concatenate([inputs["conv1_asrc"].reshape(-1),
                                    inputs["conv1_adst"].reshape(-1)])),
        conv1_b=f32(inputs["conv1_b"]),
        conv2_W=f16(inputs["conv2_W"]),
        conv2_a=f16(np.concatenate([inputs["conv2_asrc"].reshape(-1),
                                    inputs["conv2_adst"].reshape(-1)])),
        conv2_b=f32(inputs["conv2_b"]),
        mlp1_w=f16(inputs["mlp1_w"]), mlp1_b=f32(inputs["mlp1_b"]),
        bn1_g=f32(inputs["bn1_g"]), bn1_b=f32(inputs["bn1_b"]),
        mlp2_w=f16(inputs["mlp2_w"]), mlp2_b=f32(inputs["mlp2_b"]),
        bn2_g=f32(inputs["bn2_g"]), bn2_b=f32(inputs["bn2_b"]),
        mlp3_w=f16(inputs["mlp3_w"]), mlp3_b=f32(inputs["mlp3_b"]),
        fc2_b=f32(inputs["fc2_b"]),
    )

    fcl_w, fce_w, fc2_w = inputs["fcl_w"], inputs["fce_w"], inputs["fc2_w"]
    in_maps = []
    for c in range(NCORES):
        gi = gids[c]
        mi = mids[c]
        gsrc = np.zeros(pad_g, np.int32); gsrc[:len(gi)] = all_src[gi]
        gdst = np.zeros(pad_g, np.int32); gdst[:len(gi)] = all_dst[gi]
        glidx = np.full(pad_g, E + 1, np.int32); glidx[:len(gi)] = lidx_all[gi]
        goh = np.zeros((pad_g, NCHUNK), np.float16)
        goh[np.arange(len(gi)), all_dst[gi] - c * NCHUNK] = 1.0
        msrc = np.zeros(pad_m, np.int32); msrc[:len(mi)] = src[mi]
        mdst = np.zeros(pad_m, np.int32); mdst[:len(mi)] = dst[mi]
        mlidx = np.full(pad_m, E + 1, np.int32); mlidx[:len(mi)] = mi
        m = dict(shared)
        def hilo(w, blk=None):
            w = np.asarray(w, np.float32)
            h = w.astype(np.float16)
            l = (w - h.astype(np.float32)).astype(np.float16)
            if blk is None:
                return np.ascontiguousarray(np.concatenate([h, l], axis=1))
            # interleave per column-block: [h0|l0|h1|l1|...]
            parts = []
            for b in range(w.shape[1] // blk):
                parts.append(h[:, b * blk:(b + 1) * blk])
                parts.append(l[:, b * blk:(b + 1) * blk])
            return np.ascontiguousarray(np.concatenate(parts, axis=1))

        fc2w_c = np.concatenate(
            [fc2_w[c * ECH:(c + 1) * ECH],
             fc2_w[E + c * ECH:E + (c + 1) * ECH]], axis=0)
        m.update(
            fclw_hl=hilo(fcl_w[:, c * ECH:(c + 1) * ECH]),
            fclb_sh=f32(inputs["fcl_b"][c * ECH:(c + 1) * ECH]),
            fcew_hl=hilo(fce_w[:, c * ECH:(c + 1) * ECH]),
            fceb_sh=f32(inputs["fce_b"][c * ECH:(c + 1) * ECH]),
            fc2w_hl=hilo(fc2w_c, blk=2048),
            g_src=gsrc, g_dst=gdst, g_lidx=glidx, g_oh=goh,
            m_src=msrc, m_dst=mdst, m_lidx=mlidx,
        )
        in_maps.append(m)

    global last_results, last_in_maps
    last_in_maps = in_maps
    res = run_bass_kernel_spmd(nc, in_maps, core_ids=list(range(NCORES)),
                               trace=TRACE)
    last_results = res
    orig = res.results[0]["orig_out"].reshape(E).astype(np.float32)
    sig = np.empty(E, np.float32)
    for c in range(NCORES):
        mi = mids[c]
        sig[mi] = res.results[c]["score_out"].reshape(pad_m)[:len(mi)]
    return sig, orig


# revision 32
# speedup vs baseline: 1.0741x; 1.0545x over previous
"""GATWithSentenceEmbedding Trainium2 kernel (8 NeuronCores, SPMD + collectives).

V2 restructure vs baseline:
  - Phase B (fcl/fce/fc2 streaming) uses rhs-side weight tiles with [1,N]
    matvec psums: weights stream row-major straight from DRAM (no rearrange,
    no per-j Ldweights storm).  Stream DMAs dispatch on the Pool engine so
    they never stall behind compute-gated dispatches.
  - Phase A: host stages x^T, so h^T and xp1 are computed with zero on-device
    transposes.  Small weights are host-cast to fp16 (halves DMA + removes
    DVE convert passes).
  - conv1 gather tiles (xs/ad/oh) are prefetched into SBUF during the weight
    stream; only the mask-dependent work runs after the logits AllReduce.
  - BN stats use AllGather + local ones-matmul combine instead of AllReduce
    (15.4us vs 28.2us each).
  - conv2 reuses conv1's one-hot tiles.
"""

import numpy as np
from contextlib import ExitStack

import concourse.bass as bass
import concourse.mybir as mybir
import concourse.tile as tile
from concourse import bacc
from concourse.bass_utils import run_bass_kernel_spmd
from concourse.masks import make_identity

N, F, HC, S, H, E, BERT = 2048, 256, 256, 512, 4, 8192, 768
NCORES = 8
P = 128
NCHUNK = N // NCORES          # 256 dst nodes per core
ECH = E // NCORES             # 1024 g1/g2 columns per core
XP1W = H * HC + 2 * H         # 1032 = xp1 | al_s | al_d
XP2W = F + 2                  # 258  = xp2 | al_s | al_d
HC2 = HC // 2                 # 128
BIG = 1.0e9

dt = mybir.dt
AF = mybir.ActivationFunctionType
ALU = mybir.AluOpType
RG = [list(range(NCORES))]

_cache = {}
last_in_maps = None
DEBUG = False
TRACE = False
last_results = None


def _build(nt_g: int, nt_m: int, debug: bool = False, stage: int = 4):
    pad_g = nt_g * P
    pad_m = nt_m * P
    nc = bacc.Bacc("TRN2", target_bir_lowering=False, debug=False)

    def inp(name, shape, dtype=dt.float32):
        return nc.dram_tensor(name, shape, dtype, kind="ExternalInput")

    # shared inputs (host-cast fp16 where precision allows)
    xT_in = inp("xT", [F, N], dt.float16)
    sent_in = inp("sent_emb", [BERT])
    elp_in = inp("elp", [E])
    fc0_w = inp("fc0_w", [BERT, S], dt.float16)
    fc0_b = inp("fc0_b", [S])
    fc1_w = inp("fc1_w", [F, S], dt.float16)
    fc1_b = inp("fc1_b", [S])
    c1w = inp("conv1_W", [S, H * HC], dt.float16)
    c1a = inp("conv1_a", [2 * H * HC], dt.float16)
    c1b = inp("conv1_b", [H * HC])
    c2w = inp("conv2_W", [H * HC, F], dt.float16)
    c2a = inp("conv2_a", [2 * F], dt.float16)
    c2b = inp("conv2_b", [F])
    m1w = inp("mlp1_w", [4 * F, HC], dt.float16)
    m1b = inp("mlp1_b", [HC])
    bn1g = inp("bn1_g", [HC]); bn1b = inp("bn1_b", [HC])
    m2w = inp("mlp2_w", [HC, HC2], dt.float16)
    m2b = inp("mlp2_b", [HC2])
    bn2g = inp("bn2_g", [HC2]); bn2b = inp("bn2_b", [HC2])
    m3w = inp("mlp3_w", [HC2, 1], dt.float16)
    m3b = inp("mlp3_b", [1])
    fc2_b = inp("fc2_b", [E])
    # per-core inputs (row-major, streamed as rhs tiles).  Each big matrix is
    # host-split into fp16 hi + fp16 lo (lo = x - fp16(x)); the matvec runs
    # hi*Whi + lo*Whi + hi*Wlo at fp16 PE rate (4x the fp32 rate) with
    # ~2^-22 effective precision (lo*Wlo dropped).
    fclw_hl = inp("fclw_hl", [E, 2 * ECH], dt.float16)
    fclb_sh = inp("fclb_sh", [ECH])
    fcew_hl = inp("fcew_hl", [BERT, 2 * ECH], dt.float16)
    fceb_sh = inp("fceb_sh", [ECH])
    fc2w_hl = inp("fc2w_hl", [2 * ECH, 2 * E], dt.float16)
    elp_hi = inp("elp_hi", [E], dt.float16)
    elp_lo = inp("elp_lo", [E], dt.float16)
    semb_hi = inp("semb_hi", [BERT], dt.float16)
    semb_lo = inp("semb_lo", [BERT], dt.float16)
    g_src = inp("g_src", [pad_g], dt.int32)
    g_dst = inp("g_dst", [pad_g], dt.int32)
    g_dstl = inp("g_dstl", [pad_g], dt.int32)
    g_lidx = inp("g_lidx", [pad_g], dt.int32)
    g_oh = inp("g_oh", [pad_g, NCHUNK], dt.float16)
    m_src = inp("m_src", [pad_m], dt.int32)
    m_dst = inp("m_dst", [pad_m], dt.int32)
    m_dstl = inp("m_dstl", [pad_m], dt.int32)
    m_lidx = inp("m_lidx", [pad_m], dt.int32)
    # outputs
    orig_out = nc.dram_tensor("orig_out", [E], dt.float32, kind="ExternalOutput")
    score_out = nc.dram_tensor("score_out", [pad_m], dt.float32,
                               kind="ExternalOutput")

    def bcast(dram_handle, cols, offset=0):
        """AP reading a [1, cols] DRAM row replicated over 128 partitions."""
        return bass.AP(tensor=dram_handle.ap().tensor, offset=offset,
                       ap=[[0, P], [1, cols]])

    def bcast_ap(ap_tile, cols, offset=0):
        a = ap_tile[:] if not isinstance(ap_tile, bass.AP) else ap_tile
        return bass.AP(tensor=a.tensor, offset=a.offset + offset,
                       ap=[[0, P], [1, cols]])

    with tile.TileContext(nc) as tc:
        with (
            tc.tile_pool(name="dram", bufs=1, space="DRAM") as dram,
            tc.tile_pool(name="single", bufs=1) as single,
            tc.tile_pool(name="sb", bufs=4) as sb,
            tc.tile_pool(name="keep", bufs=1) as keep,
        ):
            ident_h = single.tile([P, P], dt.float16)
            make_identity(nc, ident_h[:])

            # internal DRAM
            xp1_dram = dram.tile([N, XP1W], dt.float16)
            al1d_dram = dram.tile([N, 2 * H], dt.float16)
            fc2part = dram.tile([E], dt.float32)
            logits_dram = dram.tile([E], dt.float32, addr_space="Shared")
            lext_dram = dram.tile([E + 2, 1], dt.float32)
            sent_dram = dram.tile([S], dt.float32)
            gd_dram = dram.tile([2 * ECH], dt.float32)
            xp2_in = dram.tile([NCHUNK, XP2W], dt.float16)
            xp2_dram = dram.tile([N, XP2W], dt.float16, addr_space="Shared")
            h2_in = dram.tile([NCHUNK, F], dt.float16)
            h2_dram = dram.tile([N, F], dt.float16, addr_space="Shared")
            st1_in = dram.tile([520], dt.float32)
            st1_all = dram.tile([8 * 520], dt.float32, addr_space="Shared")
            st2_in = dram.tile([2 * HC2], dt.float32)
            st2_all = dram.tile([8 * 2 * HC2], dt.float32, addr_space="Shared")
            row_dram = dram.tile([4 * HC], dt.float32)  # gs/gb rows for bcast

            # =============== phase A: inputs + hT + xp1 (SP engine DMAs) ====
            # Phases A and B overlap at runtime, sharing 8 PSUM banks via two
            # tags: ps_big [128,2048] (sent-free; hT then xp1 cycle it) and
            # ps_row [1,2048] (sent, g1, g2, then fc2 blocks cycle it).
            esA = ExitStack()
            sbA = esA.enter_context(tc.tile_pool(name="sbA", bufs=2))
            psAB = esA.enter_context(
                tc.tile_pool(name="psAB", bufs=1, space="PSUM"))

            def row_ps():
                return psAB.tile([1, 2048], dt.float32, space="PSUM",
                                 tag="ps_row", bufs=1, name="ps_row")

            def big_ps():
                return psAB.tile([P, N], dt.float32, space="PSUM",
                                 tag="ps_big", bufs=1, name="ps_big")

            xT_t = [sbA.tile([P, N], dt.float16, tag=f"xT{k}", bufs=1,
                             name=f"xT{k}") for k in range(F // P)]
            for k in range(F // P):
                nc.sync.dma_start(xT_t[k][:], xT_in[k * P:(k + 1) * P, :])
            fc1w_t = [sbA.tile([P, S], dt.float16, tag=f"fc1w{k}", bufs=1,
                               name=f"fc1w{k}") for k in range(F // P)]
            for k in range(F // P):
                nc.sync.dma_start(fc1w_t[k][:], fc1_w[k * P:(k + 1) * P, :])
            fc0w_t = [sbA.tile([P, S], dt.float16, tag=f"fc0w{k}", bufs=1,
                               name=f"fc0w{k}") for k in range(BERT // P)]
            for k in range(BERT // P):
                nc.sync.dma_start(fc0w_t[k][:], fc0_w[k * P:(k + 1) * P, :])
            semb32 = single.tile([P, BERT // P], dt.float32)
            nc.sync.dma_start(semb32[:],
                              sent_in.ap().rearrange("(k p) -> p k", p=P))
            semb16 = single.tile([P, BERT // P], dt.float16)
            nc.vector.tensor_copy(semb16[:], semb32[:])
            elp_hc = single.tile([P, E // P], dt.float16)
            nc.sync.dma_start(elp_hc[:],
                              elp_hi.ap().rearrange("(k p) -> p k", p=P))
            elp_lc = single.tile([P, E // P], dt.float16)
            nc.sync.dma_start(elp_lc[:],
                              elp_lo.ap().rearrange("(k p) -> p k", p=P))
            semb_hc = single.tile([P, BERT // P], dt.float16)
            nc.sync.dma_start(semb_hc[:],
                              semb_hi.ap().rearrange("(k p) -> p k", p=P))
            semb_lc = single.tile([P, BERT // P], dt.float16)
            nc.sync.dma_start(semb_lc[:],
                              semb_lo.ap().rearrange("(k p) -> p k", p=P))

            # sent = relu(sent_emb @ fc0_w + fc0_b) as a [1, S] row
            ps_sent = row_ps()
            for k in range(BERT // P):
                nc.tensor.matmul(ps_sent[:, 0:S], lhsT=semb16[:, k:k + 1],
                                 rhs=fc0w_t[k][:],
                                 start=(k == 0), stop=(k == BERT // P - 1))
            sent_row = sbA.tile([1, S], dt.float32, tag="sentrow", bufs=1)
            b_row = sbA.tile([1, S], dt.float32, tag="fc0brow", bufs=1)
            nc.sync.dma_start(b_row[:], fc0_b[None, :])
            nc.vector.tensor_add(sent_row[:], ps_sent[:, 0:S], b_row[:])
            nc.scalar.activation(sent_row[:], sent_row[:], AF.Relu)
            nc.sync.dma_start(sent_dram[:][None, :], sent_row[:])
            sent_col = single.tile([P, S // P], dt.float32)
            nc.sync.dma_start(sent_col[:],
                              sent_dram[:].rearrange("(k p) -> p k", p=P))
            fc1b_col = single.tile([P, S // P], dt.float32)
            nc.sync.dma_start(fc1b_col[:],
                              fc1_b.ap().rearrange("(k p) -> p k", p=P))

            # hT[s, n] = relu(xT^T-free GEMM) + sent, fully transpose-free
            hT_t = [sbA.tile([P, N], dt.float16, tag=f"hT{s}", bufs=1,
                             name=f"hT{s}") for s in range(S // P)]
            for si in range(S // P):
                ps_hT = big_ps()
                for k in range(F // P):
                    for q0 in range(0, N, 512):
                        nc.tensor.matmul(
                            ps_hT[:, q0:q0 + 512],
                            lhsT=fc1w_t[k][:, si * P:(si + 1) * P],
                            rhs=xT_t[k][:, q0:q0 + 512],
                            start=(k == 0), stop=(k == F // P - 1))
                nc.vector.tensor_tensor(
                    hT_t[si][:], ps_hT[:],
                    fc1b_col[:, si:si + 1].to_broadcast([P, N]), op=ALU.add)
                nc.scalar.activation(hT_t[si][:], hT_t[si][:], AF.Relu)
                nc.vector.tensor_tensor(
                    hT_t[si][:], hT_t[si][:],
                    sent_col[:, si:si + 1].to_broadcast([P, N]), op=ALU.add)

            # W1aug = [conv1_W | W@a_src | W@a_dst] as 4 k-tiles [128, 1032] f16
            c1a_bc = sbA.tile([P, 2 * H * HC], dt.float16, tag="c1abc", bufs=1)
            nc.sync.dma_start(c1a_bc[:], bcast(c1a, 2 * H * HC))
            w1aug_h = [sbA.tile([P, XP1W], dt.float16, tag=f"w1aug{k}", bufs=1,
                                name=f"w1aug{k}") for k in range(S // P)]
            lp = nc.allow_low_precision(
                "fp16 a-vector projections only shape GAT softmax logits")
            lp.__enter__()
            for k in range(S // P):
                nc.sync.dma_start(w1aug_h[k][:, 0:H * HC],
                                  c1w[k * P:(k + 1) * P, :])
                tmp = sbA.tile([P, H * HC], dt.float16, tag="scratch4k")
                nc.vector.tensor_mul(tmp[:], w1aug_h[k][:, 0:H * HC],
                                     c1a_bc[:, 0:H * HC])
                for h in range(H):
                    nc.vector.reduce_sum(
                        w1aug_h[k][:, H * HC + h:H * HC + h + 1],
                        tmp[:, h * HC:(h + 1) * HC], axis=mybir.AxisListType.X)
                nc.vector.tensor_mul(tmp[:], w1aug_h[k][:, 0:H * HC],
                                     c1a_bc[:, H * HC:2 * H * HC])
                for h in range(H):
                    nc.vector.reduce_sum(
                        w1aug_h[k][:, H * HC + H + h:H * HC + H + h + 1],
                        tmp[:, h * HC:(h + 1) * HC], axis=mybir.AxisListType.X)

            # xp1 = h @ W1aug per node-tile; lhsT = hT slice (no transposes)

            def emit_xp1_tile(nt):
                ps_xp1 = big_ps()
                for si in range(S // P):
                    for s0, s1 in ((0, 512), (512, 1024), (1024, XP1W)):
                        nc.tensor.matmul(
                            ps_xp1[:, s0:s1],
                            lhsT=hT_t[si][:, nt * P:(nt + 1) * P],
                            rhs=w1aug_h[si][:, s0:s1],
                            start=(si == 0), stop=(si == S // P - 1))
                xp1_t = sbA.tile([P, XP1W], dt.float16, tag="xp1")
                nc.vector.tensor_copy(xp1_t[:], ps_xp1[:, 0:XP1W])
                nc.sync.dma_start(xp1_dram[nt * P:(nt + 1) * P, :], xp1_t[:])
                nc.sync.dma_start(al1d_dram[nt * P:(nt + 1) * P, :],
                                  xp1_t[:, H * HC:H * HC + 2 * H])

            # ======== phase B: weight streaming (Pool engine DMAs) ========
            sbB = esA.enter_context(tc.tile_pool(name="sbB", bufs=2))

            # g1 = relu(elp @ fcl_w + b): [1, ECH] psum row, hi/lo passes
            ps_g1 = row_ps()
            for k in range(E // P):
                wt = sbB.tile([P, 2 * ECH], dt.float16, tag="wfcl", bufs=3,
                              name="wfcl")
                nc.gpsimd.dma_start(wt[:], fclw_hl[k * P:(k + 1) * P, :])
                for q0 in range(0, ECH, 512):
                    nc.tensor.matmul(ps_g1[:, q0:q0 + 512],
                                     lhsT=elp_hc[:, k:k + 1],
                                     rhs=wt[:, q0:q0 + 512],
                                     start=(k == 0), stop=False)
                    nc.tensor.matmul(ps_g1[:, q0:q0 + 512],
                                     lhsT=elp_lc[:, k:k + 1],
                                     rhs=wt[:, q0:q0 + 512],
                                     start=False, stop=False)
                    nc.tensor.matmul(ps_g1[:, q0:q0 + 512],
                                     lhsT=elp_hc[:, k:k + 1],
                                     rhs=wt[:, ECH + q0:ECH + q0 + 512],
                                     start=False, stop=(k == E // P - 1))
            g1_row = sbB.tile([1, ECH], dt.float32, tag="g1row", bufs=1,
                              name="g1row")
            gb_row = sbB.tile([1, ECH], dt.float32, tag="gbrow", bufs=1)
            nc.sync.dma_start(gb_row[:], fclb_sh[None, :])
            nc.vector.tensor_add(g1_row[:], ps_g1[:, 0:ECH], gb_row[:])
            nc.scalar.activation(g1_row[:], g1_row[:], AF.Relu)
            nc.sync.dma_start(gd_dram[0:ECH][None, :], g1_row[:])

            # g2 = relu(sent_emb @ fce_w + b)
            ps_g2 = row_ps()
            for k in range(BERT // P):
                wt = sbB.tile([P, 2 * ECH], dt.float16, tag="wfcl", bufs=3,
                              name="wfce")
                nc.gpsimd.dma_start(wt[:], fcew_hl[k * P:(k + 1) * P, :])
                for q0 in range(0, ECH, 512):
                    nc.tensor.matmul(ps_g2[:, q0:q0 + 512],
                                     lhsT=semb_hc[:, k:k + 1],
                                     rhs=wt[:, q0:q0 + 512],
                                     start=(k == 0), stop=False)
                    nc.tensor.matmul(ps_g2[:, q0:q0 + 512],
                                     lhsT=semb_lc[:, k:k + 1],
                                     rhs=wt[:, q0:q0 + 512],
                                     start=False, stop=False)
                    nc.tensor.matmul(ps_g2[:, q0:q0 + 512],
                                     lhsT=semb_hc[:, k:k + 1],
                                     rhs=wt[:, ECH + q0:ECH + q0 + 512],
                                     start=False, stop=(k == BERT // P - 1))
            g2_row = sbB.tile([1, ECH], dt.float32, tag="g2row", bufs=1,
                              name="g2row")
            gb2_row = sbB.tile([1, ECH], dt.float32, tag="gbrow", bufs=1)
            nc.sync.dma_start(gb2_row[:], fceb_sh[None, :])
            nc.vector.tensor_add(g2_row[:], ps_g2[:, 0:ECH], gb2_row[:])
            nc.scalar.activation(g2_row[:], g2_row[:], AF.Relu)
            nc.sync.dma_start(gd_dram[ECH:2 * ECH][None, :], g2_row[:])

            # g hi/lo split done in tiny [128, 16] column space
            g_col32 = single.tile([P, 2 * ECH // P], dt.float32)
            nc.sync.dma_start(g_col32[:],
                              gd_dram[:].rearrange("(k p) -> p k", p=P))
            g_colh = single.tile([P, 2 * ECH // P], dt.float16)
            nc.vector.tensor_copy(g_colh[:], g_col32[:])
            g_colh32 = single.tile([P, 2 * ECH // P], dt.float32)
            nc.vector.tensor_copy(g_colh32[:], g_colh[:])
            g_coll32 = single.tile([P, 2 * ECH // P], dt.float32)
            nc.vector.tensor_sub(g_coll32[:], g_col32[:], g_colh32[:])
            g_coll = single.tile([P, 2 * ECH // P], dt.float16)
            nc.vector.tensor_copy(g_coll[:], g_coll32[:])

            # fc2 partial: column-block streaming, xp1 tiles interleaved so
            # the PE fills DMA-pacing slack without stalling the stream.
            CB = 2048
            NXB = (N // P) // (E // CB)
            for c in range(E // CB):
                ps_f2 = row_ps()
                for k in range(2 * ECH // P):
                    wt = sbB.tile([P, 2 * CB], dt.float16, tag="wfc2", bufs=3,
                                  name="wfc2")
                    nc.gpsimd.dma_start(
                        wt[:], fc2w_hl[k * P:(k + 1) * P,
                                       2 * c * CB:2 * (c + 1) * CB])
                    for q0 in range(0, CB, 512):
                        nc.tensor.matmul(ps_f2[:, q0:q0 + 512],
                                         lhsT=g_colh[:, k:k + 1],
                                         rhs=wt[:, q0:q0 + 512],
                                         start=(k == 0), stop=False)
                        nc.tensor.matmul(ps_f2[:, q0:q0 + 512],
                                         lhsT=g_coll[:, k:k + 1],
                                         rhs=wt[:, q0:q0 + 512],
                                         start=False, stop=False)
                        nc.tensor.matmul(ps_f2[:, q0:q0 + 512],
                                         lhsT=g_colh[:, k:k + 1],
                                         rhs=wt[:, CB + q0:CB + q0 + 512],
                                         start=False,
                                         stop=(k == 2 * ECH // P - 1))
                f2row = sbB.tile([1, CB], dt.float32, tag="f2row", bufs=1)
                nc.vector.tensor_copy(f2row[:], ps_f2[:, 0:CB])
                nc.sync.dma_start(fc2part[c * CB:(c + 1) * CB][None, :],
                                  f2row[:])
                for nt in range(c * NXB, (c + 1) * NXB):
                    emit_xp1_tile(nt)

            # ======== conv1 gather prefetch (during fc2 stream) ========
            gsrc_sb = single.tile([P, nt_g], dt.int32)
            nc.sync.dma_start(gsrc_sb[:],
                              g_src.ap().rearrange("(t p) -> p t", p=P))
            gdst_sb = single.tile([P, nt_g], dt.int32)
            nc.sync.dma_start(gdst_sb[:],
                              g_dst.ap().rearrange("(t p) -> p t", p=P))
            glidx_sb = single.tile([P, nt_g], dt.int32)
            nc.sync.dma_start(glidx_sb[:],
                              g_lidx.ap().rearrange("(t p) -> p t", p=P))
            gdstl_sb = single.tile([P, nt_g], dt.int32)
            nc.sync.dma_start(gdstl_sb[:],
                              g_dstl.ap().rearrange("(t p) -> p t", p=P))
            msrc_sb = single.tile([P, nt_m], dt.int32)
            nc.sync.dma_start(msrc_sb[:],
                              m_src.ap().rearrange("(t p) -> p t", p=P))
            mdstl_sb = single.tile([P, nt_m], dt.int32)
            nc.sync.dma_start(mdstl_sb[:],
                              m_dstl.ap().rearrange("(t p) -> p t", p=P))
            mlidx_sb = single.tile([P, nt_m], dt.int32)
            nc.sync.dma_start(mlidx_sb[:],
                              m_lidx.ap().rearrange("(t p) -> p t", p=P))
            oh_t = [keep.tile([P, NCHUNK], dt.float16, tag=f"oh{t}",
                              name=f"oh{t}") for t in range(nt_g)]
            xs_t = [keep.tile([P, XP1W], dt.float16, tag=f"xs{t}",
                              name=f"xs{t}") for t in range(nt_g)]
            msgu_t = [keep.tile([P, H * HC + H], dt.float16, tag=f"msgu{t}",
                               name=f"msgu{t}") for t in range(nt_g)]
            for t in range(nt_g):
                nc.sync.dma_start(oh_t[t][:], g_oh[t * P:(t + 1) * P, :])
                nc.gpsimd.indirect_dma_start(
                    out=xs_t[t][:], out_offset=None, in_=xp1_dram[:],
                    in_offset=bass.IndirectOffsetOnAxis(
                        ap=gsrc_sb[:, t:t + 1], axis=0))
                ad = sbA.tile([P, 2 * H], dt.float16, tag="gad", bufs=4)
                nc.gpsimd.indirect_dma_start(
                    out=ad[:], out_offset=None, in_=al1d_dram[:],
                    in_offset=bass.IndirectOffsetOnAxis(
                        ap=gdst_sb[:, t:t + 1], axis=0))
                # alpha/exp and the unmasked message xs*ex are mask-free:
                # compute them during the stream, leaving only *valid and the
                # one-hot matmuls for after the logits AllReduce.
                alpha = sbA.tile([P, H], dt.float32, tag="alpha", bufs=4)
                nc.vector.tensor_add(alpha[:],
                                     xs_t[t][:, H * HC:H * HC + H],
                                     ad[:, H:2 * H])
                nc.vector.scalar_tensor_tensor(alpha[:], alpha[:], 0.2,
                                               alpha[:],
                                               op0=ALU.mult, op1=ALU.max)
                ex = sbA.tile([P, H], dt.float32, tag="ex", bufs=4)
                nc.scalar.activation(ex[:], alpha[:], AF.Exp)
                for h in range(H):
                    nc.vector.tensor_tensor(
                        msgu_t[t][:, h * HC:(h + 1) * HC],
                        xs_t[t][:, h * HC:(h + 1) * HC],
                        ex[:, h:h + 1].to_broadcast([P, HC]), op=ALU.mult)
                nc.vector.tensor_copy(msgu_t[t][:, H * HC:H * HC + H], ex[:])
            esA.close()

            if stage >= 2:
                nc.gpsimd.collective_compute(
                    "AllReduce", ALU.add, replica_groups=RG,
                    ins=[fc2part[:]], outs=[logits_dram[:]])
                # logits += fc2_b ; orig_out ; logits_ext
                lg_pf = single.tile([P, E // P], dt.float32)
                nc.sync.dma_start(lg_pf[:],
                                  logits_dram[:].rearrange("(p f) -> p f", p=P))
                f2b_pf = single.tile([P, E // P], dt.float32)
                nc.sync.dma_start(f2b_pf[:],
                                  fc2_b.ap().rearrange("(p f) -> p f", p=P))
                nc.vector.tensor_add(lg_pf[:], lg_pf[:], f2b_pf[:])
                nc.sync.dma_start(orig_out.ap().rearrange("(p f) -> p f", p=P),
                                  lg_pf[:])
                nc.sync.dma_start(
                    lext_dram[0:E, :].rearrange("(p f) x -> p (f x)", p=P),
                    lg_pf[:])
                big_t = single.tile([1, 2], dt.float32)
                nc.vector.memset(big_t[:, 0:1], BIG)
                nc.vector.memset(big_t[:, 1:2], -BIG)
                nc.sync.dma_start(lext_dram[E:E + 2, 0][None, :], big_t[:])


                # ============ conv1 aggregation (mask-dependent part) =======
                valid_t = [keep.tile([P, 1], dt.float32, tag=f"valid{t}",
                                     name=f"valid{t}") for t in range(nt_g)]
                esC = ExitStack()
                sbC = esC.enter_context(tc.tile_pool(name="sbC", bufs=2))
                psC = esC.enter_context(
                    tc.tile_pool(name="psC", bufs=1, space="PSUM"))
                ps_msg = [psC.tile([P, H * HC], dt.float32, space="PSUM",
                                   tag=f"ps_msg{d}", name=f"ps_msg{d}")
                          for d in range(2)]
                ps_den = [psC.tile([P, H], dt.float32, space="PSUM",
                                   tag=f"ps_den{d}", name=f"ps_den{d}")
                          for d in range(2)]
                for t in range(nt_g):
                    lg = sbC.tile([P, 1], dt.float32, tag="glg", bufs=8)
                    nc.gpsimd.indirect_dma_start(
                        out=lg[:], out_offset=None, in_=lext_dram[:],
                        in_offset=bass.IndirectOffsetOnAxis(
                            ap=glidx_sb[:, t:t + 1], axis=0))
                    nc.vector.tensor_scalar(valid_t[t][:], lg[:], 0.0, None,
                                            op0=ALU.is_gt)
                for t in range(nt_g):
                    msg = sbC.tile([P, H * HC + H], dt.float16, tag="msg",
                                   bufs=4)
                    nc.vector.tensor_tensor(
                        msg[:], msgu_t[t][:],
                        valid_t[t][:].to_broadcast([P, H * HC + H]),
                        op=ALU.mult)
                    for d in range(2):
                        lhsT = oh_t[t][:, d * P:(d + 1) * P]
                        st, sp = (t == 0), (t == nt_g - 1)
                        nc.tensor.matmul(ps_msg[d][:, 0:512], lhsT=lhsT,
                                         rhs=msg[:, 0:512], start=st, stop=sp)
                        nc.tensor.matmul(ps_msg[d][:, 512:1024], lhsT=lhsT,
                                         rhs=msg[:, 512:1024], start=st,
                                         stop=sp)
                        nc.tensor.matmul(ps_den[d][:], lhsT=lhsT,
                                         rhs=msg[:, H * HC:H * HC + H],
                                         start=st, stop=sp)
                # prefetch MLP masks now that lext is final
                if stage >= 4:
                    mask_f = [keep.tile([P, 1], dt.float32, tag=f"maskf{t}",
                                        name=f"maskf{t}") for t in range(nt_m)]
                    mask_u8 = [keep.tile([P, 1], dt.uint8, tag=f"masku{t}",
                                         name=f"masku{t}") for t in range(nt_m)]
                    for t in range(nt_m):
                        mlg = sbC.tile([P, 1], dt.float32, tag="mlg", bufs=6)
                        nc.gpsimd.indirect_dma_start(
                            out=mlg[:], out_offset=None, in_=lext_dram[:],
                            in_offset=bass.IndirectOffsetOnAxis(
                                ap=mlidx_sb[:, t:t + 1], axis=0))
                        nc.vector.tensor_scalar(mask_f[t][:], mlg[:], 0.0,
                                                None, op0=ALU.is_gt)
                        nc.vector.tensor_copy(mask_u8[t][:], mask_f[t][:])

                # finalize conv1 (+elu) and xp2aug weights
                c1b_bc = sbC.tile([P, H * HC], dt.float32, tag="c1bbc", bufs=1)
                nc.sync.dma_start(c1b_bc[:], bcast(c1b, H * HC))
                c2a_bc = sbC.tile([P, 2 * F], dt.float16, tag="c2abc", bufs=1)
                nc.sync.dma_start(c2a_bc[:], bcast(c2a, 2 * F))
                w2aug_h = [keep.tile([P, XP2W], dt.float16, tag=f"w2aug{k}",
                                     name=f"w2aug{k}")
                           for k in range(H * HC // P)]
                for k in range(H * HC // P):
                    nc.sync.dma_start(w2aug_h[k][:, 0:F],
                                      c2w[k * P:(k + 1) * P, :])
                    tmp = sbC.tile([P, F], dt.float16, tag="w2tmp")
                    nc.vector.tensor_mul(tmp[:], w2aug_h[k][:, 0:F],
                                         c2a_bc[:, 0:F])
                    nc.vector.reduce_sum(w2aug_h[k][:, F:F + 1], tmp[:],
                                         axis=mybir.AxisListType.X)
                    nc.vector.tensor_mul(tmp[:], w2aug_h[k][:, 0:F],
                                         c2a_bc[:, F:2 * F])
                    nc.vector.reduce_sum(w2aug_h[k][:, F + 1:F + 2], tmp[:],
                                         axis=mybir.AxisListType.X)
                h1_keep = [keep.tile([P, H * HC], dt.float16, tag=f"h1k{d}",
                                     name=f"h1k{d}") for d in range(2)]
                for d in range(2):
                    denr = sbC.tile([P, H], dt.float32, tag="denr")
                    nc.vector.reciprocal(denr[:], ps_den[d][:])
                    h1_t = h1_keep[d]
                    h1f = sbC.tile([P, H * HC], dt.float32, tag="h1f")
                    for h in range(H):
                        nc.vector.scalar_tensor_tensor(
                            h1f[:, h * HC:(h + 1) * HC],
                            ps_msg[d][:, h * HC:(h + 1) * HC],
                            denr[:, h:h + 1],
                            c1b_bc[:, h * HC:(h + 1) * HC],
                            op0=ALU.mult, op1=ALU.add)
                    # elu = relu(x) + exp(min(x,0)) - 1
                    relu_t = sbC.tile([P, H * HC], dt.float32, tag="elu_r")
                    nc.scalar.activation(relu_t[:], h1f[:], AF.Relu)
                    nc.vector.tensor_scalar_min(h1f[:], h1f[:], 0.0)
                    nc.scalar.activation(h1f[:], h1f[:], AF.Exp)
                    nc.vector.scalar_tensor_tensor(h1_t[:], h1f[:], -1.0,
                                                   relu_t[:],
                                                   op0=ALU.add, op1=ALU.add)
                esC.close()
                esD = ExitStack()
                sbD = esD.enter_context(tc.tile_pool(name="sbD", bufs=2))
                psD = esD.enter_context(
                    tc.tile_pool(name="psD", bufs=1, space="PSUM"))
                for d in range(2):
                    h1_t = h1_keep[d]
                    ps_xp2 = psD.tile([P, XP2W], dt.float32, space="PSUM",
                                      tag="ps_xp2")
                    for k in range(H * HC // P):
                        ps_h1t = psD.tile([P, P], dt.float16, space="PSUM",
                                          tag="ps_xth", bufs=4)
                        nc.tensor.transpose(ps_h1t[:],
                                            h1_t[:, k * P:(k + 1) * P],
                                            ident_h[:])
                        h1T = sb.tile([P, P], dt.float16, tag="xTh")
                        nc.vector.tensor_copy(h1T[:], ps_h1t[:])
                        nc.tensor.matmul(ps_xp2[:], lhsT=h1T[:],
                                         rhs=w2aug_h[k][:],
                                         start=(k == 0),
                                         stop=(k == H * HC // P - 1))
                    xp2_t = sbD.tile([P, XP2W], dt.float16, tag="xp2")
                    nc.vector.tensor_copy(xp2_t[:], ps_xp2[:])
                    nc.sync.dma_start(xp2_in[d * P:(d + 1) * P, :], xp2_t[:])
                esD.close()
                if stage >= 3:
                    xd2_t = [keep.tile([P, XP2W], dt.float16, tag=f"xd2{t}",
                                       name=f"xd2{t}") for t in range(nt_g)]
                    for t in range(nt_g):
                        nc.gpsimd.indirect_dma_start(
                            out=xd2_t[t][:], out_offset=None, in_=xp2_in[:],
                            in_offset=bass.IndirectOffsetOnAxis(
                                ap=gdstl_sb[:, t:t + 1], axis=0))
                nc.gpsimd.collective_compute(
                    "AllGather", ALU.bypass, replica_groups=RG,
                    ins=[xp2_in[:]], outs=[xp2_dram[:]])

            if stage >= 3:
                # ============ conv2 aggregation (reuses oh tiles) ============
                esE = ExitStack()
                sbE = esE.enter_context(tc.tile_pool(name="sbE", bufs=2))
                psE = esE.enter_context(
                    tc.tile_pool(name="psE", bufs=1, space="PSUM"))
                ps_m2 = [psE.tile([P, F + 1], dt.float32, space="PSUM",
                                  tag=f"ps_m2{d}", name=f"ps_m2{d}")
                         for d in range(2)]
                for t in range(nt_g):
                    xs2 = sbE.tile([P, XP2W], dt.float16, tag="xs2", bufs=4)
                    nc.gpsimd.indirect_dma_start(
                        out=xs2[:], out_offset=None, in_=xp2_dram[:],
                        in_offset=bass.IndirectOffsetOnAxis(
                            ap=gsrc_sb[:, t:t + 1], axis=0))
                    alpha2 = sbE.tile([P, 1], dt.float32, tag="alpha2", bufs=4)
                    nc.vector.tensor_add(alpha2[:], xs2[:, F:F + 1],
                                         xd2_t[t][:, F + 1:F + 2])
                    nc.vector.scalar_tensor_tensor(alpha2[:], alpha2[:], 0.2,
                                                   alpha2[:],
                                                   op0=ALU.mult, op1=ALU.max)
                    ex2 = sbE.tile([P, 1], dt.float32, tag="ex2", bufs=4)
                    nc.scalar.activation(ex2[:], alpha2[:], AF.Exp)
                    nc.vector.tensor_mul(ex2[:], ex2[:], valid_t[t][:])
                    msg2 = sbE.tile([P, F + 1], dt.float16, tag="msg2", bufs=4)
                    nc.vector.tensor_tensor(msg2[:, 0:F], xs2[:, 0:F],
                                            ex2[:].to_broadcast([P, F]),
                                            op=ALU.mult)
                    nc.vector.tensor_copy(msg2[:, F:F + 1], ex2[:])
                    for d in range(2):
                        lhsT = oh_t[t][:, d * P:(d + 1) * P]
                        st, sp = (t == 0), (t == nt_g - 1)
                        nc.tensor.matmul(ps_m2[d][:], lhsT=lhsT, rhs=msg2[:],
                                         start=st, stop=sp)
                c2b_bc = sbE.tile([P, F], dt.float32, tag="c2bbc", bufs=1)
                nc.sync.dma_start(c2b_bc[:], bcast(c2b, F))
                for d in range(2):
                    d2r = sbE.tile([P, 1], dt.float32, tag="d2r")
                    nc.vector.reciprocal(d2r[:], ps_m2[d][:, F:F + 1])
                    h2_t = sbE.tile([P, F], dt.float16, tag="h2")
                    nc.vector.tensor_tensor(h2_t[:], ps_m2[d][:, 0:F],
                                            d2r[:].to_broadcast([P, F]),
                                            op=ALU.mult)
                    nc.vector.tensor_add(h2_t[:], h2_t[:], c2b_bc[:])
                    nc.sync.dma_start(h2_in[d * P:(d + 1) * P, :], h2_t[:])
                esE.close()
                if stage >= 4:
                    xj_t = [keep.tile([P, F], dt.float16, tag=f"xj{t}",
                                      name=f"xj{t}") for t in range(nt_m)]
                    for t in range(nt_m):
                        nc.gpsimd.indirect_dma_start(
                            out=xj_t[t][:], out_offset=None, in_=h2_in[:],
                            in_offset=bass.IndirectOffsetOnAxis(
                                ap=mdstl_sb[:, t:t + 1], axis=0))
                nc.gpsimd.collective_compute(
                    "AllGather", ALU.bypass, replica_groups=RG,
                    ins=[h2_in[:]], outs=[h2_dram[:]])

            if stage >= 4:
                # ============ edge MLP ============
                m1w_h = [keep.tile([P, HC], dt.float16, tag=f"m1w{k}",
                                   name=f"m1w{k}") for k in range(4 * F // P)]
                for k in range(4 * F // P):
                    nc.sync.dma_start(m1w_h[k][:], m1w[k * P:(k + 1) * P, :])
                m1b_bc = single.tile([P, HC], dt.float32)
                nc.sync.dma_start(m1b_bc[:], bcast(m1b, HC))
                z1_t = [keep.tile([P, HC], dt.float32, tag=f"z1_{t}",
                                  name=f"z1_{t}") for t in range(nt_m)]
                esF = ExitStack()
                sbF = esF.enter_context(tc.tile_pool(name="sbF", bufs=2))
                psF = esF.enter_context(
                    tc.tile_pool(name="psF", bufs=1, space="PSUM"))
                ps_s1 = psF.tile([1, HC], dt.float32, space="PSUM", tag="ps_s1")
                ps_q1 = psF.tile([1, HC], dt.float32, space="PSUM", tag="ps_q1")
                ps_cnt = psF.tile([1, 1], dt.float32, space="PSUM",
                                  tag="ps_cnt")
                for t in range(nt_m):
                    xi = sbF.tile([P, F], dt.float16, tag="xi", bufs=6)
                    nc.gpsimd.indirect_dma_start(
                        out=xi[:], out_offset=None, in_=h2_dram[:],
                        in_offset=bass.IndirectOffsetOnAxis(
                            ap=msrc_sb[:, t:t + 1], axis=0))
                    xj = xj_t[t]
                    dsub = sbF.tile([P, F], dt.float16, tag="dsub", bufs=4)
                    nc.vector.tensor_sub(dsub[:], xi[:], xj[:])
                    nc.scalar.activation(dsub[:], dsub[:], AF.Abs)
                    pmul = sbF.tile([P, F], dt.float16, tag="pmul", bufs=4)
                    nc.vector.tensor_mul(pmul[:], xi[:], xj[:])
                    ps_z1 = psF.tile([P, HC], dt.float32, space="PSUM",
                                     tag="ps_z1", bufs=2)
                    for pi, piece in enumerate((xi, xj, dsub, pmul)):
                        for hf in range(2):
                            ps_t = psF.tile([P, P], dt.float16, space="PSUM",
                                            tag="ps_xth", bufs=3)
                            nc.tensor.transpose(ps_t[:],
                                                piece[:, hf * P:(hf + 1) * P],
                                                ident_h[:])
                            efT = sb.tile([P, P], dt.float16, tag="xTh")
                            nc.vector.tensor_copy(efT[:], ps_t[:])
                            k = pi * 2 + hf
                            nc.tensor.matmul(ps_z1[:], lhsT=efT[:],
                                             rhs=m1w_h[k][:],
                                             start=(k == 0), stop=(k == 7))
                    nc.vector.tensor_add(z1_t[t][:], ps_z1[:], m1b_bc[:])
                    zsq = sbF.tile([P, HC], dt.float32, tag="zsq", bufs=4)
                    nc.vector.tensor_mul(zsq[:], z1_t[t][:], z1_t[t][:])
                    st, sp = (t == 0), (t == nt_m - 1)
                    nc.tensor.matmul(ps_s1[:], lhsT=mask_f[t][:],
                                     rhs=z1_t[t][:], start=st, stop=sp)
                    nc.tensor.matmul(ps_q1[:], lhsT=mask_f[t][:], rhs=zsq[:],
                                     start=st, stop=sp)
                    nc.tensor.matmul(ps_cnt[:], lhsT=mask_f[t][:],
                                     rhs=mask_f[t][:], start=st, stop=sp)
                # pack stats1, AllGather + local combine
                s_sb = sbF.tile([1, HC], dt.float32, tag="stat")
                nc.vector.tensor_copy(s_sb[:], ps_s1[:])
                nc.sync.dma_start(st1_in[None, 0:HC], s_sb[:])
                q_sb = sbF.tile([1, HC], dt.float32, tag="stat")
                nc.vector.tensor_copy(q_sb[:], ps_q1[:])
                nc.sync.dma_start(st1_in[None, HC:2 * HC], q_sb[:])
                c_sb = sbF.tile([1, 1], dt.float32, tag="statc")
                nc.vector.tensor_copy(c_sb[:], ps_cnt[:])
                nc.sync.dma_start(st1_in[None, 2 * HC:2 * HC + 1], c_sb[:])
                zpad = sbF.tile([1, 7], dt.float32, tag="statz")
                nc.vector.memset(zpad[:], 0.0)
                nc.sync.dma_start(st1_in[None, 2 * HC + 1:520], zpad[:])
                esF.close()
                nc.gpsimd.collective_compute(
                    "AllGather", ALU.bypass, replica_groups=RG,
                    ins=[st1_in[:]], outs=[st1_all[:]])

                esG = ExitStack()
                sbG = esG.enter_context(tc.tile_pool(name="sbG", bufs=2))
                psG = esG.enter_context(
                    tc.tile_pool(name="psG", bufs=1, space="PSUM"))

                ones8 = single.tile([8, 1], dt.float32)
                nc.vector.memset(ones8[:], 1.0)

                def combine_stats(st_all, width, tagn):
                    st8 = sbG.tile([8, width], dt.float32, tag=f"st8{tagn}",
                                   name=f"st8{tagn}")
                    nc.sync.dma_start(
                        st8[:], st_all[:].rearrange("(r c) -> r c", r=8))
                    ps_c = psG.tile([1, 520], dt.float32, space="PSUM",
                                    tag="pscmb", bufs=1, name=f"pscmb{tagn}")
                    for q0 in range(0, width, 512):
                        q1 = min(q0 + 512, width)
                        nc.tensor.matmul(ps_c[:, q0:q1], lhsT=ones8[:],
                                         rhs=st8[:, q0:q1], start=True,
                                         stop=True)
                    row = sbG.tile([1, width], dt.float32, tag=f"strow{tagn}",
                                   name=f"strow{tagn}")
                    nc.vector.tensor_copy(row[:], ps_c[:, 0:width])
                    return row

                st1_row = combine_stats(st1_all, 520, "a")

                def bn_rows(st_row, nch, g_in, b_in, row_off):
                    cnt_row = sbG.tile([1, 1], dt.float32, tag="bn_cnt",
                                       name="bn_cnt")
                    nc.vector.tensor_scalar_max(cnt_row[:],
                                                st1_row[:, 512:513], 1.0)
                    cr = sbG.tile([1, 1], dt.float32, tag="bn_cr", name="bn_cr")
                    nc.vector.reciprocal(cr[:], cnt_row[:])
                    mean = sbG.tile([1, nch], dt.float32, tag="bn_mean",
                                    name="bn_mean")
                    nc.vector.tensor_tensor(mean[:], st_row[:, 0:nch],
                                            cr[:].to_broadcast([1, nch]),
                                            op=ALU.mult)
                    var = sbG.tile([1, nch], dt.float32, tag="bn_var",
                                   name="bn_var")
                    nc.vector.tensor_tensor(var[:], st_row[:, nch:2 * nch],
                                            cr[:].to_broadcast([1, nch]),
                                            op=ALU.mult)
                    msq = sbG.tile([1, nch], dt.float32, tag="bn_msq",
                                   name="bn_msq")
                    nc.vector.tensor_mul(msq[:], mean[:], mean[:])
                    nc.vector.tensor_sub(var[:], var[:], msq[:])
                    nc.vector.tensor_scalar_add(var[:], var[:], 1e-5)
                    nc.scalar.activation(var[:], var[:], AF.Sqrt)
                    rstd = sbG.tile([1, nch], dt.float32, tag="bn_rstd",
                                    name="bn_rstd")
                    nc.vector.reciprocal(rstd[:], var[:])
                    g_row = sbG.tile([1, nch], dt.float32, tag="bn_g",
                                     name="bn_g")
                    nc.sync.dma_start(g_row[:], g_in[None, :])
                    b_row2 = sbG.tile([1, nch], dt.float32, tag="bn_b",
                                      name="bn_b")
                    nc.sync.dma_start(b_row2[:], b_in[None, :])
                    gs = sbG.tile([1, nch], dt.float32, tag="bn_gs",
                                  name="bn_gs")
                    nc.vector.tensor_mul(gs[:], g_row[:], rstd[:])
                    gb = sbG.tile([1, nch], dt.float32, tag="bn_gb",
                                  name="bn_gb")
                    nc.vector.tensor_mul(gb[:], mean[:], gs[:])
                    nc.vector.tensor_sub(gb[:], b_row2[:], gb[:])
                    nc.sync.dma_start(row_dram[None, row_off:row_off + nch],
                                      gs[:])
                    nc.sync.dma_start(
                        row_dram[None, row_off + nch:row_off + 2 * nch], gb[:])

                bn_rows(st1_row, HC, bn1g, bn1b, 0)
                gs1_bc = single.tile([P, HC], dt.float32)
                nc.sync.dma_start(gs1_bc[:], bcast_ap(row_dram, HC, 0))
                gb1_bc = single.tile([P, HC], dt.float32)
                nc.sync.dma_start(gb1_bc[:], bcast_ap(row_dram, HC, HC))
                m2w_h = [keep.tile([P, HC2], dt.float16, tag=f"m2w{k}",
                                   name=f"m2w{k}") for k in range(HC // P)]
                for k in range(HC // P):
                    nc.sync.dma_start(m2w_h[k][:], m2w[k * P:(k + 1) * P, :])
                m2b_bc = single.tile([P, HC2], dt.float32)
                nc.sync.dma_start(m2b_bc[:], bcast(m2b, HC2))
                z2_t = [keep.tile([P, HC2], dt.float32, tag=f"z2_{t}",
                                  name=f"z2_{t}") for t in range(nt_m)]
                ps_s2 = psG.tile([1, HC2], dt.float32, space="PSUM",
                                 tag="ps_s2")
                ps_q2 = psG.tile([1, HC2], dt.float32, space="PSUM",
                                 tag="ps_q2")
                for t in range(nt_m):
                    zn = sbG.tile([P, HC], dt.float16, tag="zn", bufs=4)
                    nc.vector.tensor_mul(zn[:], z1_t[t][:], gs1_bc[:])
                    nc.vector.tensor_add(zn[:], zn[:], gb1_bc[:])
                    nc.scalar.activation(zn[:], zn[:], AF.Relu)
                    ps_z2 = psG.tile([P, HC2], dt.float32, space="PSUM",
                                     tag="ps_z2", bufs=1)
                    for k in range(HC // P):
                        ps_t = psG.tile([P, P], dt.float16, space="PSUM",
                                        tag="ps_xth", bufs=2)
                        nc.tensor.transpose(ps_t[:], zn[:, k * P:(k + 1) * P],
                                            ident_h[:])
                        znT = sb.tile([P, P], dt.float16, tag="xTh")
                        nc.vector.tensor_copy(znT[:], ps_t[:])
                        nc.tensor.matmul(ps_z2[:], lhsT=znT[:], rhs=m2w_h[k][:],
                                         start=(k == 0), stop=(k == HC // P - 1))
                    nc.vector.tensor_add(z2_t[t][:], ps_z2[:], m2b_bc[:])
                    zsq2 = sbG.tile([P, HC2], dt.float32, tag="zsq2", bufs=4)
                    nc.vector.tensor_mul(zsq2[:], z2_t[t][:], z2_t[t][:])
                    st, sp = (t == 0), (t == nt_m - 1)
                    nc.tensor.matmul(ps_s2[:], lhsT=mask_f[t][:],
                                     rhs=z2_t[t][:], start=st, stop=sp)
                    nc.tensor.matmul(ps_q2[:], lhsT=mask_f[t][:], rhs=zsq2[:],
                                     start=st, stop=sp)
                s2_sb = sbG.tile([1, HC2], dt.float32, tag="stat2")
                nc.vector.tensor_copy(s2_sb[:], ps_s2[:])
                nc.sync.dma_start(st2_in[None, 0:HC2], s2_sb[:])
                q2_sb = sbG.tile([1, HC2], dt.float32, tag="stat2")
                nc.vector.tensor_copy(q2_sb[:], ps_q2[:])
                nc.sync.dma_start(st2_in[None, HC2:2 * HC2], q2_sb[:])
                nc.gpsimd.collective_compute(
                    "AllGather", ALU.bypass, replica_groups=RG,
                    ins=[st2_in[:]], outs=[st2_all[:]])
                st2_row = combine_stats(st2_all, 2 * HC2, "b")
                bn_rows(st2_row, HC2, bn2g, bn2b, 2 * HC)
                gs2_bc = single.tile([P, HC2], dt.float32)
                nc.sync.dma_start(gs2_bc[:], bcast_ap(row_dram, HC2, 2 * HC))
                gb2_bc = single.tile([P, HC2], dt.float32)
                nc.sync.dma_start(gb2_bc[:],
                                  bcast_ap(row_dram, HC2, 2 * HC + HC2))
                m3w_h = single.tile([P, 1], dt.float16)
                nc.sync.dma_start(m3w_h[:], m3w[:, :])
                m3b_bc = single.tile([P, 1], dt.float32)
                nc.sync.dma_start(m3b_bc[:], bcast(m3b, 1))
                neg25 = single.tile([P, 1], dt.float32)
                nc.vector.memset(neg25[:], -2.5)
                for t in range(nt_m):
                    zn2 = sbG.tile([P, HC2], dt.float16, tag="zn2", bufs=4)
                    nc.vector.tensor_mul(zn2[:], z2_t[t][:], gs2_bc[:])
                    nc.vector.tensor_add(zn2[:], zn2[:], gb2_bc[:])
                    nc.scalar.activation(zn2[:], zn2[:], AF.Relu)
                    ps_t = psG.tile([P, P], dt.float16, space="PSUM",
                                    tag="ps_xth", bufs=2)
                    nc.tensor.transpose(ps_t[:], zn2[:], ident_h[:])
                    znT2 = sb.tile([P, P], dt.float16, tag="xTh")
                    nc.vector.tensor_copy(znT2[:], ps_t[:])
                    ps_sc = psG.tile([P, 1], dt.float32, space="PSUM",
                                     tag="ps_sc", bufs=1)
                    nc.tensor.matmul(ps_sc[:], lhsT=znT2[:], rhs=m3w_h[:],
                                     start=True, stop=True)
                    score = sbG.tile([P, 1], dt.float32, tag="score", bufs=4)
                    nc.vector.tensor_add(score[:], ps_sc[:], m3b_bc[:])
                    sel = sbG.tile([P, 1], dt.float32, tag="sel", bufs=4)
                    nc.vector.select(sel[:], mask_u8[t][:], score[:], neg25[:])
                    nc.scalar.activation(sel[:], sel[:], AF.Sigmoid)
                    nc.sync.dma_start(score_out[t * P:(t + 1) * P][:, None],
                                      sel[:])
                esG.close()

    nc.compile()
    return nc


def kernel(**inputs):
    inputs = {k: np.asarray(v) for k, v in inputs.items()}
    src = inputs["edge_index"][0].astype(np.int64)
    dst = inputs["edge_index"][1].astype(np.int64)

    # --- edge partition by dst chunk (GAT set includes self loops) ---
    all_src = np.concatenate([src, np.arange(N, dtype=np.int64)])
    all_dst = np.concatenate([dst, np.arange(N, dtype=np.int64)])
    lidx_all = np.concatenate(
        [np.arange(E, dtype=np.int64), np.full(N, E, dtype=np.int64)])
    chunk_g = all_dst // NCHUNK
    gids = [np.where(chunk_g == c)[0] for c in range(NCORES)]
    nt_g = int(np.ceil(max(len(i) for i in gids) / P))
    pad_g = nt_g * P
    chunk_m = dst // NCHUNK
    mids = [np.where(chunk_m == c)[0] for c in range(NCORES)]
    nt_m = int(np.ceil(max(len(i) for i in mids) / P))
    pad_m = nt_m * P

    key = (nt_g, nt_m, DEBUG)
    if key not in _cache:
        _cache[key] = _build(nt_g, nt_m, debug=DEBUG)
    nc = _cache[key]

    def f32(v):
        return np.ascontiguousarray(v, dtype=np.float32)

    def f16(v):
        return np.ascontiguousarray(v, dtype=np.float16)

    shared = dict(
        xT=f16(inputs["x"].T),
        sent_emb=f32(inputs["sent_emb"]),
        elp=f32(inputs["edge_logits_param"]),
        elp_hi=f16(inputs["edge_logits_param"]),
        elp_lo=f16(inputs["edge_logits_param"]
                   - inputs["edge_logits_param"].astype(np.float16)
                     .astype(np.float32)),
        semb_hi=f16(inputs["sent_emb"]),
        semb_lo=f16(inputs["sent_emb"]
                    - inputs["sent_emb"].astype(np.float16)
                      .astype(np.float32)),
        fc0_w=f16(inputs["fc0_w"]), fc0_b=f32(inputs["fc0_b"]),
        fc1_w=f16(inputs["fc1_w"]), fc1_b=f32(inputs["fc1_b"]),
        conv1_W=f16(inputs["conv1_W"]),
        conv1_a=f16(np.concatenate([inputs["conv1_asrc"].reshape(-1),
                                    inputs["conv1_adst"].reshape(-1)])),
        conv1_b=f32(inputs["conv1_b"]),
        conv2_W=f16(inputs["conv2_W"]),
        conv2_a=f16(np.concatenate([inputs["conv2_asrc"].reshape(-1),
                                    inputs["conv2_adst"].reshape(-1)])),
        conv2_b=f32(inputs["conv2_b"]),
        mlp1_w=f16(inputs["mlp1_w"]), mlp1_b=f32(inputs["mlp1_b"]),
        bn1_g=f32(inputs["bn1_g"]), bn1_b=f32(inputs["bn1_b"]),
        mlp2_w=f16(inputs["mlp2_w"]), mlp2_b=f32(inputs["mlp2_b"]),
        bn2_g=f32(inputs["bn2_g"]), bn2_b=f32(inputs["bn2_b"]),
        mlp3_w=f16(inputs["mlp3_w"]), mlp3_b=f32(inputs["mlp3_b"]),
        fc2_b=f32(inputs["fc2_b"]),
    )

    fcl_w, fce_w, fc2_w = inputs["fcl_w"], inputs["fce_w"], inputs["fc2_w"]
    in_maps = []
    for c in range(NCORES):
        gi = gids[c]
        mi = mids[c]
        gsrc = np.zeros(pad_g, np.int32); gsrc[:len(gi)] = all_src[gi]
        gdst = np.zeros(pad_g, np.int32); gdst[:len(gi)] = all_dst[gi]
        gdstl = np.zeros(pad_g, np.int32)
        gdstl[:len(gi)] = all_dst[gi] - c * NCHUNK
        glidx = np.full(pad_g, E + 1, np.int32); glidx[:len(gi)] = lidx_all[gi]
        goh = np.zeros((pad_g, NCHUNK), np.float16)
        goh[np.arange(len(gi)), all_dst[gi] - c * NCHUNK] = 1.0
        msrc = np.zeros(pad_m, np.int32); msrc[:len(mi)] = src[mi]
        mdst = np.zeros(pad_m, np.int32); mdst[:len(mi)] = dst[mi]
        mdstl = np.zeros(pad_m, np.int32)
        mdstl[:len(mi)] = dst[mi] - c * NCHUNK
        mlidx = np.full(pad_m, E + 1, np.int32); mlidx[:len(mi)] = mi
        m = dict(shared)
        def hilo(w, blk=None):
            w = np.asarray(w, np.float32)
            h = w.astype(np.float16)
            l = (w - h.astype(np.float32)).astype(np.float16)
            if blk is None:
                return np.ascontiguousarray(np.concatenate([h, l], axis=1))
            # interleave per column-block: [h0|l0|h1|l1|...]
            parts = []
            for b in range(w.shape[1] // blk):
                parts.append(h[:, b * blk:(b + 1) * blk])
                parts.append(l[:, b * blk:(b + 1) * blk])
            return np.ascontiguousarray(np.concatenate(parts, axis=1))

        fc2w_c = np.asarray(np.concatenate(
            [fc2_w[c * ECH:(c + 1) * ECH],
             fc2_w[E + c * ECH:E + (c + 1) * ECH]], axis=0), np.float32)
        m.update(
            fclw_hl=hilo(fcl_w[:, c * ECH:(c + 1) * ECH]),
            fclb_sh=f32(inputs["fcl_b"][c * ECH:(c + 1) * ECH]),
            fcew_hl=hilo(fce_w[:, c * ECH:(c + 1) * ECH]),
            fceb_sh=f32(inputs["fce_b"][c * ECH:(c + 1) * ECH]),
            fc2w_hl=hilo(fc2w_c, blk=2048),
            g_src=gsrc, g_dst=gdst, g_dstl=gdstl, g_lidx=glidx, g_oh=goh,
            m_src=msrc, m_dst=mdst, m_dstl=mdstl, m_lidx=mlidx,
        )
        in_maps.append(m)

    global last_results, last_in_maps
    last_in_maps = in_maps
    res = run_bass_kernel_spmd(nc, in_maps, core_ids=list(range(NCORES)),
                               trace=TRACE)
    last_results = res
    orig = res.results[0]["orig_out"].reshape(E).astype(np.float32)
    sig = np.empty(E, np.float32)
    for c in range(NCORES):
        mi = mids[c]
        sig[mi] = res.results[c]["score_out"].reshape(pad_m)[:len(mi)]
    return sig, orig


# revision 39
# speedup vs baseline: 1.0877x; 1.0126x over previous
"""GATWithSentenceEmbedding Trainium2 kernel (8 NeuronCores, SPMD + collectives).

V2 restructure vs baseline:
  - Phase B (fcl/fce/fc2 streaming) uses rhs-side weight tiles with [1,N]
    matvec psums: weights stream row-major straight from DRAM (no rearrange,
    no per-j Ldweights storm).  Stream DMAs dispatch on the Pool engine so
    they never stall behind compute-gated dispatches.
  - Phase A: host stages x^T, so h^T and xp1 are computed with zero on-device
    transposes.  Small weights are host-cast to fp16 (halves DMA + removes
    DVE convert passes).
  - conv1 gather tiles (xs/ad/oh) are prefetched into SBUF during the weight
    stream; only the mask-dependent work runs after the logits AllReduce.
  - BN stats use AllGather + local ones-matmul combine instead of AllReduce
    (15.4us vs 28.2us each).
  - conv2 reuses conv1's one-hot tiles.
"""

import numpy as np
from contextlib import ExitStack

import concourse.bass as bass
import concourse.mybir as mybir
import concourse.tile as tile
from concourse import bacc
from concourse.bass_utils import run_bass_kernel_spmd
from concourse.masks import make_identity

N, F, HC, S, H, E, BERT = 2048, 256, 256, 512, 4, 8192, 768
NCORES = 8
P = 128
NCHUNK = N // NCORES          # 256 dst nodes per core
ECH = E // NCORES             # 1024 g1/g2 columns per core
XP1W = H * HC + 2 * H         # 1032 = xp1 | al_s | al_d
XP2W = F + 2                  # 258  = xp2 | al_s | al_d
HC2 = HC // 2                 # 128
BIG = 1.0e9

dt = mybir.dt
AF = mybir.ActivationFunctionType
ALU = mybir.AluOpType
RG = [list(range(NCORES))]

_cache = {}
last_in_maps = None
DEBUG = False
TRACE = False
last_results = None


def _build(nt_g: int, nt_m: int, debug: bool = False, stage: int = 4):
    pad_g = nt_g * P
    pad_m = nt_m * P
    nc = bacc.Bacc("TRN2", target_bir_lowering=False, debug=False)

    def inp(name, shape, dtype=dt.float32):
        return nc.dram_tensor(name, shape, dtype, kind="ExternalInput")

    # shared inputs (host-cast fp16 where precision allows)
    xT_in = inp("xT", [F, N], dt.float16)
    sent_in = inp("sent_emb", [BERT])
    elp_in = inp("elp", [E])
    fc0_w = inp("fc0_w", [BERT, S], dt.float16)
    fc0_b = inp("fc0_b", [S])
    fc1_w = inp("fc1_w", [F, S], dt.float16)
    fc1_b = inp("fc1_b", [S])
    c1w = inp("conv1_W", [S, H * HC], dt.float16)
    c1a = inp("conv1_a", [2 * H * HC], dt.float16)
    c1b = inp("conv1_b", [H * HC])
    c2w = inp("conv2_W", [H * HC, F], dt.float16)
    c2a = inp("conv2_a", [2 * F], dt.float16)
    c2b = inp("conv2_b", [F])
    m1w = inp("mlp1_w", [4 * F, HC], dt.float16)
    m1b = inp("mlp1_b", [HC])
    bn1g = inp("bn1_g", [HC]); bn1b = inp("bn1_b", [HC])
    m2w = inp("mlp2_w", [HC, HC2], dt.float16)
    m2b = inp("mlp2_b", [HC2])
    bn2g = inp("bn2_g", [HC2]); bn2b = inp("bn2_b", [HC2])
    m3w = inp("mlp3_w", [HC2, 1], dt.float16)
    m3b = inp("mlp3_b", [1])
    fc2_b = inp("fc2_b", [E])
    # per-core inputs (row-major, streamed as rhs tiles).  Each big matrix is
    # host-split into fp16 hi + fp16 lo (lo = x - fp16(x)); the matvec runs
    # hi*Whi + lo*Whi + hi*Wlo at fp16 PE rate (4x the fp32 rate) with
    # ~2^-22 effective precision (lo*Wlo dropped).
    fclw_hl = inp("fclw_hl", [E, 2 * ECH], dt.float16)
    fclb_sh = inp("fclb_sh", [ECH])
    fcew_hl = inp("fcew_hl", [BERT, 2 * ECH], dt.float16)
    fceb_sh = inp("fceb_sh", [ECH])
    fc2w_hl = inp("fc2w_hl", [2 * ECH, 2 * E], dt.float16)
    elp_hi = inp("elp_hi", [E], dt.float16)
    elp_lo = inp("elp_lo", [E], dt.float16)
    semb_hi = inp("semb_hi", [BERT], dt.float16)
    semb_lo = inp("semb_lo", [BERT], dt.float16)
    g_src = inp("g_src", [pad_g], dt.int32)
    g_dst = inp("g_dst", [pad_g], dt.int32)
    g_dstl = inp("g_dstl", [pad_g], dt.int32)
    g_lidx = inp("g_lidx", [pad_g], dt.int32)
    g_oh = inp("g_oh", [pad_g, NCHUNK], dt.float16)
    m_src = inp("m_src", [pad_m], dt.int32)
    m_dst = inp("m_dst", [pad_m], dt.int32)
    m_dstl = inp("m_dstl", [pad_m], dt.int32)
    m_lidx = inp("m_lidx", [pad_m], dt.int32)
    # outputs
    orig_out = nc.dram_tensor("orig_out", [E], dt.float32, kind="ExternalOutput")
    score_out = nc.dram_tensor("score_out", [pad_m], dt.float32,
                               kind="ExternalOutput")

    def bcast(dram_handle, cols, offset=0):
        """AP reading a [1, cols] DRAM row replicated over 128 partitions."""
        return bass.AP(tensor=dram_handle.ap().tensor, offset=offset,
                       ap=[[0, P], [1, cols]])

    def bcast_ap(ap_tile, cols, offset=0):
        a = ap_tile[:] if not isinstance(ap_tile, bass.AP) else ap_tile
        return bass.AP(tensor=a.tensor, offset=a.offset + offset,
                       ap=[[0, P], [1, cols]])

    with tile.TileContext(nc) as tc:
        with (
            tc.tile_pool(name="dram", bufs=1, space="DRAM") as dram,
            tc.tile_pool(name="single", bufs=1) as single,
            tc.tile_pool(name="sb", bufs=4) as sb,
            tc.tile_pool(name="keep", bufs=1) as keep,
        ):
            ident_h = single.tile([P, P], dt.float16)
            make_identity(nc, ident_h[:])

            # internal DRAM
            xp1_dram = dram.tile([N, XP1W], dt.float16)
            al1d_dram = dram.tile([N, 2 * H], dt.float16)
            fc2part = dram.tile([E], dt.float32)
            logits_dram = dram.tile([E], dt.float32, addr_space="Shared")
            lext_dram = dram.tile([E + 2, 1], dt.float32)
            sent_dram = dram.tile([S], dt.float32)
            gd_dram = dram.tile([2 * ECH], dt.float32)
            xp2_in = dram.tile([NCHUNK, XP2W], dt.float16)
            xp2_dram = dram.tile([N, XP2W], dt.float16, addr_space="Shared")
            h2_in = dram.tile([NCHUNK, F], dt.float16)
            h2_dram = dram.tile([N, F], dt.float16, addr_space="Shared")
            st1_in = dram.tile([520], dt.float32)
            st1_all = dram.tile([8 * 520], dt.float32, addr_space="Shared")
            st2_in = dram.tile([2 * HC2], dt.float32)
            st2_all = dram.tile([8 * 2 * HC2], dt.float32, addr_space="Shared")
            row_dram = dram.tile([4 * HC], dt.float32)  # gs/gb rows for bcast

            # =============== phase A: inputs + hT + xp1 (SP engine DMAs) ====
            # Phases A and B overlap at runtime, sharing 8 PSUM banks via two
            # tags: ps_big [128,2048] (sent-free; hT then xp1 cycle it) and
            # ps_row [1,2048] (sent, g1, g2, then fc2 blocks cycle it).
            esA = ExitStack()
            sbA = esA.enter_context(tc.tile_pool(name="sbA", bufs=2))
            psAB = esA.enter_context(
                tc.tile_pool(name="psAB", bufs=1, space="PSUM"))

            def row_ps():
                return psAB.tile([1, 2048], dt.float32, space="PSUM",
                                 tag="ps_row", bufs=1, name="ps_row")

            def big_ps():
                return psAB.tile([P, N], dt.float32, space="PSUM",
                                 tag="ps_big", bufs=1, name="ps_big")

            xT_t = [sbA.tile([P, N], dt.float16, tag=f"xT{k}", bufs=1,
                             name=f"xT{k}") for k in range(F // P)]
            for k in range(F // P):
                nc.sync.dma_start(xT_t[k][:], xT_in[k * P:(k + 1) * P, :])
            fc1w_t = [sbA.tile([P, S], dt.float16, tag=f"fc1w{k}", bufs=1,
                               name=f"fc1w{k}") for k in range(F // P)]
            for k in range(F // P):
                nc.sync.dma_start(fc1w_t[k][:], fc1_w[k * P:(k + 1) * P, :])
            fc0w_t = [sbA.tile([P, S], dt.float16, tag=f"fc0w{k}", bufs=1,
                               name=f"fc0w{k}") for k in range(BERT // P)]
            for k in range(BERT // P):
                nc.sync.dma_start(fc0w_t[k][:], fc0_w[k * P:(k + 1) * P, :])
            semb32 = single.tile([P, BERT // P], dt.float32)
            nc.sync.dma_start(semb32[:],
                              sent_in.ap().rearrange("(k p) -> p k", p=P))
            semb16 = single.tile([P, BERT // P], dt.float16)
            nc.vector.tensor_copy(semb16[:], semb32[:])
            elp_hc = single.tile([P, E // P], dt.float16)
            nc.sync.dma_start(elp_hc[:],
                              elp_hi.ap().rearrange("(k p) -> p k", p=P))
            elp_lc = single.tile([P, E // P], dt.float16)
            nc.sync.dma_start(elp_lc[:],
                              elp_lo.ap().rearrange("(k p) -> p k", p=P))
            semb_hc = single.tile([P, BERT // P], dt.float16)
            nc.sync.dma_start(semb_hc[:],
                              semb_hi.ap().rearrange("(k p) -> p k", p=P))
            semb_lc = single.tile([P, BERT // P], dt.float16)
            nc.sync.dma_start(semb_lc[:],
                              semb_lo.ap().rearrange("(k p) -> p k", p=P))

            # sent = relu(sent_emb @ fc0_w + fc0_b) as a [1, S] row
            ps_sent = row_ps()
            for k in range(BERT // P):
                nc.tensor.matmul(ps_sent[:, 0:S], lhsT=semb16[:, k:k + 1],
                                 rhs=fc0w_t[k][:],
                                 start=(k == 0), stop=(k == BERT // P - 1))
            sent_row = sbA.tile([1, S], dt.float32, tag="sentrow", bufs=1)
            b_row = sbA.tile([1, S], dt.float32, tag="fc0brow", bufs=1)
            nc.sync.dma_start(b_row[:], fc0_b[None, :])
            nc.vector.tensor_add(sent_row[:], ps_sent[:, 0:S], b_row[:])
            nc.scalar.activation(sent_row[:], sent_row[:], AF.Relu)
            nc.sync.dma_start(sent_dram[:][None, :], sent_row[:])
            sent_col = single.tile([P, S // P], dt.float32)
            nc.sync.dma_start(sent_col[:],
                              sent_dram[:].rearrange("(k p) -> p k", p=P))
            fc1b_col = single.tile([P, S // P], dt.float32)
            nc.sync.dma_start(fc1b_col[:],
                              fc1_b.ap().rearrange("(k p) -> p k", p=P))

            # hT[s, n] = relu(xT^T-free GEMM) + sent, fully transpose-free
            hT_t = [sbA.tile([P, N], dt.float16, tag=f"hT{s}", bufs=1,
                             name=f"hT{s}") for s in range(S // P)]
            for si in range(S // P):
                ps_hT = big_ps()
                for k in range(F // P):
                    for q0 in range(0, N, 512):
                        nc.tensor.matmul(
                            ps_hT[:, q0:q0 + 512],
                            lhsT=fc1w_t[k][:, si * P:(si + 1) * P],
                            rhs=xT_t[k][:, q0:q0 + 512],
                            start=(k == 0), stop=(k == F // P - 1))
                nc.vector.tensor_tensor(
                    hT_t[si][:], ps_hT[:],
                    fc1b_col[:, si:si + 1].to_broadcast([P, N]), op=ALU.add)
                nc.scalar.activation(hT_t[si][:], hT_t[si][:], AF.Relu)
                nc.vector.tensor_tensor(
                    hT_t[si][:], hT_t[si][:],
                    sent_col[:, si:si + 1].to_broadcast([P, N]), op=ALU.add)

            # W1aug = [conv1_W | W@a_src | W@a_dst] as 4 k-tiles [128, 1032] f16
            c1a_bc = sbA.tile([P, 2 * H * HC], dt.float16, tag="c1abc", bufs=1)
            nc.sync.dma_start(c1a_bc[:], bcast(c1a, 2 * H * HC))
            w1aug_h = [sbA.tile([P, XP1W], dt.float16, tag=f"w1aug{k}", bufs=1,
                                name=f"w1aug{k}") for k in range(S // P)]
            lp = nc.allow_low_precision(
                "fp16 a-vector projections only shape GAT softmax logits")
            lp.__enter__()
            for k in range(S // P):
                nc.sync.dma_start(w1aug_h[k][:, 0:H * HC],
                                  c1w[k * P:(k + 1) * P, :])
                tmp = sbA.tile([P, H * HC], dt.float16, tag="scratch4k")
                nc.vector.tensor_mul(tmp[:], w1aug_h[k][:, 0:H * HC],
                                     c1a_bc[:, 0:H * HC])
                for h in range(H):
                    nc.vector.reduce_sum(
                        w1aug_h[k][:, H * HC + h:H * HC + h + 1],
                        tmp[:, h * HC:(h + 1) * HC], axis=mybir.AxisListType.X)
                nc.vector.tensor_mul(tmp[:], w1aug_h[k][:, 0:H * HC],
                                     c1a_bc[:, H * HC:2 * H * HC])
                for h in range(H):
                    nc.vector.reduce_sum(
                        w1aug_h[k][:, H * HC + H + h:H * HC + H + h + 1],
                        tmp[:, h * HC:(h + 1) * HC], axis=mybir.AxisListType.X)

            # xp1 = h @ W1aug per node-tile; lhsT = hT slice (no transposes)

            def emit_xp1_tile(nt):
                ps_xp1 = big_ps()
                for si in range(S // P):
                    for s0, s1 in ((0, 512), (512, 1024), (1024, XP1W)):
                        nc.tensor.matmul(
                            ps_xp1[:, s0:s1],
                            lhsT=hT_t[si][:, nt * P:(nt + 1) * P],
                            rhs=w1aug_h[si][:, s0:s1],
                            start=(si == 0), stop=(si == S // P - 1))
                xp1_t = sbA.tile([P, XP1W], dt.float16, tag="xp1")
                nc.vector.tensor_copy(xp1_t[:], ps_xp1[:, 0:XP1W])
                nc.sync.dma_start(xp1_dram[nt * P:(nt + 1) * P, :], xp1_t[:])
                nc.sync.dma_start(al1d_dram[nt * P:(nt + 1) * P, :],
                                  xp1_t[:, H * HC:H * HC + 2 * H])

            # ======== phase B: weight streaming (Pool engine DMAs) ========
            sbB = esA.enter_context(tc.tile_pool(name="sbB", bufs=2))

            # g1 = relu(elp @ fcl_w + b): [1, ECH] psum row, hi/lo passes
            ps_g1 = row_ps()
            for k in range(E // P):
                wt = sbB.tile([P, 2 * ECH], dt.float16, tag="wfcl", bufs=3,
                              name="wfcl")
                nc.gpsimd.dma_start(wt[:], fclw_hl[k * P:(k + 1) * P, :])
                for q0 in range(0, ECH, 512):
                    nc.tensor.matmul(ps_g1[:, q0:q0 + 512],
                                     lhsT=elp_hc[:, k:k + 1],
                                     rhs=wt[:, q0:q0 + 512],
                                     start=(k == 0), stop=False)
                    nc.tensor.matmul(ps_g1[:, q0:q0 + 512],
                                     lhsT=elp_lc[:, k:k + 1],
                                     rhs=wt[:, q0:q0 + 512],
                                     start=False, stop=False)
                    nc.tensor.matmul(ps_g1[:, q0:q0 + 512],
                                     lhsT=elp_hc[:, k:k + 1],
                                     rhs=wt[:, ECH + q0:ECH + q0 + 512],
                                     start=False, stop=(k == E // P - 1))
            g1_row = sbB.tile([1, ECH], dt.float32, tag="g1row", bufs=1,
                              name="g1row")
            gb_row = sbB.tile([1, ECH], dt.float32, tag="gbrow", bufs=1)
            nc.sync.dma_start(gb_row[:], fclb_sh[None, :])
            nc.vector.tensor_add(g1_row[:], ps_g1[:, 0:ECH], gb_row[:])
            nc.scalar.activation(g1_row[:], g1_row[:], AF.Relu)
            nc.sync.dma_start(gd_dram[0:ECH][None, :], g1_row[:])

            # g2 = relu(sent_emb @ fce_w + b)
            ps_g2 = row_ps()
            for k in range(BERT // P):
                wt = sbB.tile([P, 2 * ECH], dt.float16, tag="wfcl", bufs=3,
                              name="wfce")
                nc.gpsimd.dma_start(wt[:], fcew_hl[k * P:(k + 1) * P, :])
                for q0 in range(0, ECH, 512):
                    nc.tensor.matmul(ps_g2[:, q0:q0 + 512],
                                     lhsT=semb_hc[:, k:k + 1],
                                     rhs=wt[:, q0:q0 + 512],
                                     start=(k == 0), stop=False)
                    nc.tensor.matmul(ps_g2[:, q0:q0 + 512],
                                     lhsT=semb_lc[:, k:k + 1],
                                     rhs=wt[:, q0:q0 + 512],
                                     start=False, stop=False)
                    nc.tensor.matmul(ps_g2[:, q0:q0 + 512],
                                     lhsT=semb_hc[:, k:k + 1],
                                     rhs=wt[:, ECH + q0:ECH + q0 + 512],
                                     start=False, stop=(k == BERT // P - 1))
            g2_row = sbB.tile([1, ECH], dt.float32, tag="g2row", bufs=1,
                              name="g2row")
            gb2_row = sbB.tile([1, ECH], dt.float32, tag="gbrow", bufs=1)
            nc.sync.dma_start(gb2_row[:], fceb_sh[None, :])
            nc.vector.tensor_add(g2_row[:], ps_g2[:, 0:ECH], gb2_row[:])
            nc.scalar.activation(g2_row[:], g2_row[:], AF.Relu)
            nc.sync.dma_start(gd_dram[ECH:2 * ECH][None, :], g2_row[:])

            # g hi/lo split done in tiny [128, 16] column space
            g_col32 = single.tile([P, 2 * ECH // P], dt.float32)
            nc.sync.dma_start(g_col32[:],
                              gd_dram[:].rearrange("(k p) -> p k", p=P))
            g_colh = single.tile([P, 2 * ECH // P], dt.float16)
            nc.vector.tensor_copy(g_colh[:], g_col32[:])
            g_colh32 = single.tile([P, 2 * ECH // P], dt.float32)
            nc.vector.tensor_copy(g_colh32[:], g_colh[:])
            g_coll32 = single.tile([P, 2 * ECH // P], dt.float32)
            nc.vector.tensor_sub(g_coll32[:], g_col32[:], g_colh32[:])
            g_coll = single.tile([P, 2 * ECH // P], dt.float16)
            nc.vector.tensor_copy(g_coll[:], g_coll32[:])

            # fc2 partial: column-block streaming, xp1 tiles interleaved so
            # the PE fills DMA-pacing slack without stalling the stream.
            CB = 2048
            NXB = (N // P) // (E // CB)
            for c in range(E // CB):
                ps_f2 = row_ps()
                for k in range(2 * ECH // P):
                    wt = sbB.tile([P, 2 * CB], dt.float16, tag="wfc2", bufs=3,
                                  name="wfc2")
                    nc.gpsimd.dma_start(
                        wt[:], fc2w_hl[k * P:(k + 1) * P,
                                       2 * c * CB:2 * (c + 1) * CB])
                    for q0 in range(0, CB, 512):
                        nc.tensor.matmul(ps_f2[:, q0:q0 + 512],
                                         lhsT=g_colh[:, k:k + 1],
                                         rhs=wt[:, q0:q0 + 512],
                                         start=(k == 0), stop=False)
                        nc.tensor.matmul(ps_f2[:, q0:q0 + 512],
                                         lhsT=g_coll[:, k:k + 1],
                                         rhs=wt[:, q0:q0 + 512],
                                         start=False, stop=False)
                        nc.tensor.matmul(ps_f2[:, q0:q0 + 512],
                                         lhsT=g_colh[:, k:k + 1],
                                         rhs=wt[:, CB + q0:CB + q0 + 512],
                                         start=False,
                                         stop=(k == 2 * ECH // P - 1))
                f2row = sbB.tile([1, CB], dt.float32, tag="f2row", bufs=1)
                nc.vector.tensor_copy(f2row[:], ps_f2[:, 0:CB])
                nc.sync.dma_start(fc2part[c * CB:(c + 1) * CB][None, :],
                                  f2row[:])
                for nt in range(c * NXB, (c + 1) * NXB):
                    emit_xp1_tile(nt)

            # ======== conv1 gather prefetch (during fc2 stream) ========
            gsrc_sb = single.tile([P, nt_g], dt.int32)
            nc.sync.dma_start(gsrc_sb[:],
                              g_src.ap().rearrange("(t p) -> p t", p=P))
            gdst_sb = single.tile([P, nt_g], dt.int32)
            nc.sync.dma_start(gdst_sb[:],
                              g_dst.ap().rearrange("(t p) -> p t", p=P))
            glidx_sb = single.tile([P, nt_g], dt.int32)
            nc.sync.dma_start(glidx_sb[:],
                              g_lidx.ap().rearrange("(t p) -> p t", p=P))
            gdstl_sb = single.tile([P, nt_g], dt.int32)
            nc.sync.dma_start(gdstl_sb[:],
                              g_dstl.ap().rearrange("(t p) -> p t", p=P))
            msrc_sb = single.tile([P, nt_m], dt.int32)
            nc.sync.dma_start(msrc_sb[:],
                              m_src.ap().rearrange("(t p) -> p t", p=P))
            mdstl_sb = single.tile([P, nt_m], dt.int32)
            nc.sync.dma_start(mdstl_sb[:],
                              m_dstl.ap().rearrange("(t p) -> p t", p=P))
            mlidx_sb = single.tile([P, nt_m], dt.int32)
            nc.sync.dma_start(mlidx_sb[:],
                              m_lidx.ap().rearrange("(t p) -> p t", p=P))
            oh_t = [keep.tile([P, NCHUNK], dt.float16, tag=f"oh{t}",
                              name=f"oh{t}") for t in range(nt_g)]
            xs_t = [keep.tile([P, XP1W], dt.float16, tag=f"xs{t}",
                              name=f"xs{t}") for t in range(nt_g)]
            msgu_t = [keep.tile([P, H * HC + H], dt.float16, tag=f"msgu{t}",
                               name=f"msgu{t}") for t in range(nt_g)]
            for t in range(nt_g):
                nc.sync.dma_start(oh_t[t][:], g_oh[t * P:(t + 1) * P, :])
                nc.gpsimd.indirect_dma_start(
                    out=xs_t[t][:], out_offset=None, in_=xp1_dram[:],
                    in_offset=bass.IndirectOffsetOnAxis(
                        ap=gsrc_sb[:, t:t + 1], axis=0))
                ad = sbA.tile([P, 2 * H], dt.float16, tag="gad", bufs=4)
                nc.gpsimd.indirect_dma_start(
                    out=ad[:], out_offset=None, in_=al1d_dram[:],
                    in_offset=bass.IndirectOffsetOnAxis(
                        ap=gdst_sb[:, t:t + 1], axis=0))
                # alpha/exp and the unmasked message xs*ex are mask-free:
                # compute them during the stream, leaving only *valid and the
                # one-hot matmuls for after the logits AllReduce.
                alpha = sbA.tile([P, H], dt.float32, tag="alpha", bufs=4)
                nc.vector.tensor_add(alpha[:],
                                     xs_t[t][:, H * HC:H * HC + H],
                                     ad[:, H:2 * H])
                nc.vector.scalar_tensor_tensor(alpha[:], alpha[:], 0.2,
                                               alpha[:],
                                               op0=ALU.mult, op1=ALU.max)
                ex = sbA.tile([P, H], dt.float32, tag="ex", bufs=4)
                nc.scalar.activation(ex[:], alpha[:], AF.Exp)
                for h in range(H):
                    nc.vector.tensor_tensor(
                        msgu_t[t][:, h * HC:(h + 1) * HC],
                        xs_t[t][:, h * HC:(h + 1) * HC],
                        ex[:, h:h + 1].to_broadcast([P, HC]), op=ALU.mult)
                nc.vector.tensor_copy(msgu_t[t][:, H * HC:H * HC + H], ex[:])
            esA.close()

            if stage >= 2:
                nc.gpsimd.collective_compute(
                    "AllReduce", ALU.add, replica_groups=RG,
                    ins=[fc2part[:]], outs=[logits_dram[:]])
                # logits += fc2_b ; orig_out ; logits_ext
                lg_pf = single.tile([P, E // P], dt.float32)
                nc.sync.dma_start(lg_pf[:],
                                  logits_dram[:].rearrange("(p f) -> p f", p=P))
                f2b_pf = single.tile([P, E // P], dt.float32)
                nc.sync.dma_start(f2b_pf[:],
                                  fc2_b.ap().rearrange("(p f) -> p f", p=P))
                nc.vector.tensor_add(lg_pf[:], lg_pf[:], f2b_pf[:])
                nc.sync.dma_start(orig_out.ap().rearrange("(p f) -> p f", p=P),
                                  lg_pf[:])
                nc.sync.dma_start(
                    lext_dram[0:E, :].rearrange("(p f) x -> p (f x)", p=P),
                    lg_pf[:])
                big_t = single.tile([1, 2], dt.float32)
                nc.vector.memset(big_t[:, 0:1], BIG)
                nc.vector.memset(big_t[:, 1:2], -BIG)
                nc.sync.dma_start(lext_dram[E:E + 2, 0][None, :], big_t[:])


                # ============ conv1 aggregation (mask-dependent part) =======
                valid_t = [keep.tile([P, 1], dt.float32, tag=f"valid{t}",
                                     name=f"valid{t}") for t in range(nt_g)]
                esC = ExitStack()
                sbC = esC.enter_context(tc.tile_pool(name="sbC", bufs=2))
                psC = esC.enter_context(
                    tc.tile_pool(name="psC", bufs=1, space="PSUM"))
                ps_msg = [psC.tile([P, H * HC], dt.float32, space="PSUM",
                                   tag=f"ps_msg{d}", name=f"ps_msg{d}")
                          for d in range(2)]
                ps_den = [psC.tile([P, H], dt.float32, space="PSUM",
                                   tag=f"ps_den{d}", name=f"ps_den{d}")
                          for d in range(2)]
                for t in range(nt_g):
                    lg = sbC.tile([P, 1], dt.float32, tag="glg", bufs=8)
                    nc.gpsimd.indirect_dma_start(
                        out=lg[:], out_offset=None, in_=lext_dram[:],
                        in_offset=bass.IndirectOffsetOnAxis(
                            ap=glidx_sb[:, t:t + 1], axis=0))
                    nc.vector.tensor_scalar(valid_t[t][:], lg[:], 0.0, None,
                                            op0=ALU.is_gt)
                for t in range(nt_g):
                    msg = sbC.tile([P, H * HC + H], dt.float16, tag="msg",
                                   bufs=4)
                    nc.vector.tensor_tensor(
                        msg[:], msgu_t[t][:],
                        valid_t[t][:].to_broadcast([P, H * HC + H]),
                        op=ALU.mult)
                    for d in range(2):
                        lhsT = oh_t[t][:, d * P:(d + 1) * P]
                        st, sp = (t == 0), (t == nt_g - 1)
                        nc.tensor.matmul(ps_msg[d][:, 0:512], lhsT=lhsT,
                                         rhs=msg[:, 0:512], start=st, stop=sp)
                        nc.tensor.matmul(ps_msg[d][:, 512:1024], lhsT=lhsT,
                                         rhs=msg[:, 512:1024], start=st,
                                         stop=sp)
                        nc.tensor.matmul(ps_den[d][:], lhsT=lhsT,
                                         rhs=msg[:, H * HC:H * HC + H],
                                         start=st, stop=sp)
                # prefetch MLP masks now that lext is final
                if stage >= 4:
                    mask_f = [keep.tile([P, 1], dt.float32, tag=f"maskf{t}",
                                        name=f"maskf{t}") for t in range(nt_m)]
                    mask_u8 = [keep.tile([P, 1], dt.uint8, tag=f"masku{t}",
                                         name=f"masku{t}") for t in range(nt_m)]
                    for t in range(nt_m):
                        mlg = sbC.tile([P, 1], dt.float32, tag="mlg", bufs=6)
                        nc.gpsimd.indirect_dma_start(
                            out=mlg[:], out_offset=None, in_=lext_dram[:],
                            in_offset=bass.IndirectOffsetOnAxis(
                                ap=mlidx_sb[:, t:t + 1], axis=0))
                        nc.vector.tensor_scalar(mask_f[t][:], mlg[:], 0.0,
                                                None, op0=ALU.is_gt)
                        nc.vector.tensor_copy(mask_u8[t][:], mask_f[t][:])

                # finalize conv1 (+elu) and xp2aug weights
                c1b_bc = sbC.tile([P, H * HC], dt.float32, tag="c1bbc", bufs=1)
                nc.sync.dma_start(c1b_bc[:], bcast(c1b, H * HC))
                c2a_bc = sbC.tile([P, 2 * F], dt.float16, tag="c2abc", bufs=1)
                nc.sync.dma_start(c2a_bc[:], bcast(c2a, 2 * F))
                w2aug_h = [keep.tile([P, XP2W], dt.float16, tag=f"w2aug{k}",
                                     name=f"w2aug{k}")
                           for k in range(H * HC // P)]
                for k in range(H * HC // P):
                    nc.sync.dma_start(w2aug_h[k][:, 0:F],
                                      c2w[k * P:(k + 1) * P, :])
                    tmp = sbC.tile([P, F], dt.float16, tag="w2tmp")
                    nc.vector.tensor_mul(tmp[:], w2aug_h[k][:, 0:F],
                                         c2a_bc[:, 0:F])
                    nc.vector.reduce_sum(w2aug_h[k][:, F:F + 1], tmp[:],
                                         axis=mybir.AxisListType.X)
                    nc.vector.tensor_mul(tmp[:], w2aug_h[k][:, 0:F],
                                         c2a_bc[:, F:2 * F])
                    nc.vector.reduce_sum(w2aug_h[k][:, F + 1:F + 2], tmp[:],
                                         axis=mybir.AxisListType.X)
                h1_keep = [keep.tile([P, H * HC], dt.float16, tag=f"h1k{d}",
                                     name=f"h1k{d}") for d in range(2)]
                for d in range(2):
                    denr = sbC.tile([P, H], dt.float32, tag="denr")
                    nc.vector.reciprocal(denr[:], ps_den[d][:])
                    h1_t = h1_keep[d]
                    h1f = sbC.tile([P, H * HC], dt.float32, tag="h1f")
                    for h in range(H):
                        nc.vector.scalar_tensor_tensor(
                            h1f[:, h * HC:(h + 1) * HC],
                            ps_msg[d][:, h * HC:(h + 1) * HC],
                            denr[:, h:h + 1],
                            c1b_bc[:, h * HC:(h + 1) * HC],
                            op0=ALU.mult, op1=ALU.add)
                    # elu = relu(x) + exp(min(x,0)) - 1
                    relu_t = sbC.tile([P, H * HC], dt.float32, tag="elu_r")
                    nc.scalar.activation(relu_t[:], h1f[:], AF.Relu)
                    nc.vector.tensor_scalar_min(h1f[:], h1f[:], 0.0)
                    nc.scalar.activation(h1f[:], h1f[:], AF.Exp)
                    nc.vector.scalar_tensor_tensor(h1_t[:], h1f[:], -1.0,
                                                   relu_t[:],
                                                   op0=ALU.add, op1=ALU.add)
                esC.close()
                esD = ExitStack()
                sbD = esD.enter_context(tc.tile_pool(name="sbD", bufs=2))
                psD = esD.enter_context(
                    tc.tile_pool(name="psD", bufs=1, space="PSUM"))
                for d in range(2):
                    h1_t = h1_keep[d]
                    ps_xp2 = psD.tile([P, XP2W], dt.float32, space="PSUM",
                                      tag="ps_xp2")
                    for k in range(H * HC // P):
                        ps_h1t = psD.tile([P, P], dt.float16, space="PSUM",
                                          tag="ps_xth", bufs=4)
                        nc.tensor.transpose(ps_h1t[:],
                                            h1_t[:, k * P:(k + 1) * P],
                                            ident_h[:])
                        h1T = sb.tile([P, P], dt.float16, tag="xTh")
                        nc.vector.tensor_copy(h1T[:], ps_h1t[:])
                        nc.tensor.matmul(ps_xp2[:], lhsT=h1T[:],
                                         rhs=w2aug_h[k][:],
                                         start=(k == 0),
                                         stop=(k == H * HC // P - 1))
                    xp2_t = sbD.tile([P, XP2W], dt.float16, tag="xp2")
                    nc.vector.tensor_copy(xp2_t[:], ps_xp2[:])
                    nc.sync.dma_start(xp2_in[d * P:(d + 1) * P, :], xp2_t[:])
                esD.close()
                if stage >= 3:
                    xd2_t = [keep.tile([P, XP2W], dt.float16, tag=f"xd2{t}",
                                       name=f"xd2{t}") for t in range(nt_g)]
                    for t in range(nt_g):
                        nc.gpsimd.indirect_dma_start(
                            out=xd2_t[t][:], out_offset=None, in_=xp2_in[:],
                            in_offset=bass.IndirectOffsetOnAxis(
                                ap=gdstl_sb[:, t:t + 1], axis=0))
                nc.gpsimd.collective_compute(
                    "AllGather", ALU.bypass, replica_groups=RG,
                    ins=[xp2_in[:]], outs=[xp2_dram[:]])

            if stage >= 3:
                # ============ conv2 aggregation (reuses oh tiles) ============
                esE = ExitStack()
                sbE = esE.enter_context(tc.tile_pool(name="sbE", bufs=2))
                psE = esE.enter_context(
                    tc.tile_pool(name="psE", bufs=1, space="PSUM"))
                ps_m2 = [psE.tile([P, F + 1], dt.float32, space="PSUM",
                                  tag=f"ps_m2{d}", name=f"ps_m2{d}")
                         for d in range(2)]
                for t in range(nt_g):
                    xs2 = sbE.tile([P, XP2W], dt.float16, tag="xs2", bufs=4)
                    nc.gpsimd.indirect_dma_start(
                        out=xs2[:], out_offset=None, in_=xp2_dram[:],
                        in_offset=bass.IndirectOffsetOnAxis(
                            ap=gsrc_sb[:, t:t + 1], axis=0))
                    alpha2 = sbE.tile([P, 1], dt.float32, tag="alpha2", bufs=4)
                    nc.vector.tensor_add(alpha2[:], xs2[:, F:F + 1],
                                         xd2_t[t][:, F + 1:F + 2])
                    nc.vector.scalar_tensor_tensor(alpha2[:], alpha2[:], 0.2,
                                                   alpha2[:],
                                                   op0=ALU.mult, op1=ALU.max)
                    ex2 = sbE.tile([P, 1], dt.float32, tag="ex2", bufs=4)
                    nc.scalar.activation(ex2[:], alpha2[:], AF.Exp)
                    nc.vector.tensor_mul(ex2[:], ex2[:], valid_t[t][:])
                    msg2 = sbE.tile([P, F + 1], dt.float16, tag="msg2", bufs=4)
                    nc.vector.tensor_tensor(msg2[:, 0:F], xs2[:, 0:F],
                                            ex2[:].to_broadcast([P, F]),
                                            op=ALU.mult)
                    nc.vector.tensor_copy(msg2[:, F:F + 1], ex2[:])
                    for d in range(2):
                        lhsT = oh_t[t][:, d * P:(d + 1) * P]
                        st, sp = (t == 0), (t == nt_g - 1)
                        nc.tensor.matmul(ps_m2[d][:], lhsT=lhsT, rhs=msg2[:],
                                         start=st, stop=sp)
                c2b_bc = sbE.tile([P, F], dt.float32, tag="c2bbc", bufs=1)
                nc.sync.dma_start(c2b_bc[:], bcast(c2b, F))
                for d in range(2):
                    d2r = sbE.tile([P, 1], dt.float32, tag="d2r")
                    nc.vector.reciprocal(d2r[:], ps_m2[d][:, F:F + 1])
                    h2_t = sbE.tile([P, F], dt.float16, tag="h2")
                    nc.vector.tensor_tensor(h2_t[:], ps_m2[d][:, 0:F],
                                            d2r[:].to_broadcast([P, F]),
                                            op=ALU.mult)
                    nc.vector.tensor_add(h2_t[:], h2_t[:], c2b_bc[:])
                    nc.sync.dma_start(h2_in[d * P:(d + 1) * P, :], h2_t[:])
                esE.close()
                if stage >= 4:
                    xj_t = [keep.tile([P, F], dt.float16, tag=f"xj{t}",
                                      name=f"xj{t}") for t in range(nt_m)]
                    for t in range(nt_m):
                        nc.gpsimd.indirect_dma_start(
                            out=xj_t[t][:], out_offset=None, in_=h2_in[:],
                            in_offset=bass.IndirectOffsetOnAxis(
                                ap=mdstl_sb[:, t:t + 1], axis=0))
                nc.gpsimd.collective_compute(
                    "AllGather", ALU.bypass, replica_groups=RG,
                    ins=[h2_in[:]], outs=[h2_dram[:]])

            if stage >= 4:
                # ============ edge MLP ============
                m1w_h = [keep.tile([P, HC], dt.float16, tag=f"m1w{k}",
                                   name=f"m1w{k}") for k in range(4 * F // P)]
                for k in range(4 * F // P):
                    nc.sync.dma_start(m1w_h[k][:], m1w[k * P:(k + 1) * P, :])
                m1b_bc = single.tile([P, HC], dt.float32)
                nc.sync.dma_start(m1b_bc[:], bcast(m1b, HC))
                z1_t = [keep.tile([P, HC], dt.float16, tag=f"z1_{t}",
                                  name=f"z1_{t}") for t in range(nt_m)]
                mask_h = [keep.tile([P, 1], dt.float16, tag=f"maskh{t}",
                                    name=f"maskh{t}") for t in range(nt_m)]
                for t in range(nt_m):
                    nc.vector.tensor_copy(mask_h[t][:], mask_f[t][:])
                esF = ExitStack()
                sbF = esF.enter_context(tc.tile_pool(name="sbF", bufs=2))
                psF = esF.enter_context(
                    tc.tile_pool(name="psF", bufs=1, space="PSUM"))
                ps_s1 = psF.tile([1, HC], dt.float32, space="PSUM", tag="ps_s1")
                ps_q1 = psF.tile([1, HC], dt.float32, space="PSUM", tag="ps_q1")
                ps_cnt = psF.tile([1, 1], dt.float32, space="PSUM",
                                  tag="ps_cnt")
                for t in range(nt_m):
                    xi = sbF.tile([P, F], dt.float16, tag="xi", bufs=6)
                    nc.gpsimd.indirect_dma_start(
                        out=xi[:], out_offset=None, in_=h2_dram[:],
                        in_offset=bass.IndirectOffsetOnAxis(
                            ap=msrc_sb[:, t:t + 1], axis=0))
                    xj = xj_t[t]
                    dsub = sbF.tile([P, F], dt.float16, tag="dsub", bufs=4)
                    nc.vector.tensor_sub(dsub[:], xi[:], xj[:])
                    nc.scalar.activation(dsub[:], dsub[:], AF.Abs)
                    pmul = sbF.tile([P, F], dt.float16, tag="pmul", bufs=4)
                    nc.vector.tensor_mul(pmul[:], xi[:], xj[:])
                    ps_z1 = psF.tile([P, HC], dt.float32, space="PSUM",
                                     tag="ps_z1", bufs=2)
                    for pi, piece in enumerate((xi, xj, dsub, pmul)):
                        for hf in range(2):
                            ps_t = psF.tile([P, P], dt.float16, space="PSUM",
                                            tag="ps_xth", bufs=3)
                            nc.tensor.transpose(ps_t[:],
                                                piece[:, hf * P:(hf + 1) * P],
                                                ident_h[:])
                            efT = sb.tile([P, P], dt.float16, tag="xTh")
                            nc.vector.tensor_copy(efT[:], ps_t[:])
                            k = pi * 2 + hf
                            nc.tensor.matmul(ps_z1[:], lhsT=efT[:],
                                             rhs=m1w_h[k][:],
                                             start=(k == 0), stop=(k == 7))
                    nc.vector.tensor_add(z1_t[t][:], ps_z1[:], m1b_bc[:])
                    zsq = sbF.tile([P, HC], dt.float16, tag="zsq", bufs=4)
                    nc.vector.tensor_mul(zsq[:], z1_t[t][:], z1_t[t][:])
                    st, sp = (t == 0), (t == nt_m - 1)
                    nc.tensor.matmul(ps_s1[:], lhsT=mask_h[t][:],
                                     rhs=z1_t[t][:], start=st, stop=sp)
                    nc.tensor.matmul(ps_q1[:], lhsT=mask_h[t][:], rhs=zsq[:],
                                     start=st, stop=sp)
                    nc.tensor.matmul(ps_cnt[:], lhsT=mask_h[t][:],
                                     rhs=mask_h[t][:], start=st, stop=sp)
                # pack stats1, AllGather + local combine
                s_sb = sbF.tile([1, HC], dt.float32, tag="stat")
                nc.vector.tensor_copy(s_sb[:], ps_s1[:])
                nc.sync.dma_start(st1_in[None, 0:HC], s_sb[:])
                q_sb = sbF.tile([1, HC], dt.float32, tag="stat")
                nc.vector.tensor_copy(q_sb[:], ps_q1[:])
                nc.sync.dma_start(st1_in[None, HC:2 * HC], q_sb[:])
                c_sb = sbF.tile([1, 1], dt.float32, tag="statc")
                nc.vector.tensor_copy(c_sb[:], ps_cnt[:])
                nc.sync.dma_start(st1_in[None, 2 * HC:2 * HC + 1], c_sb[:])
                zpad = sbF.tile([1, 7], dt.float32, tag="statz")
                nc.vector.memset(zpad[:], 0.0)
                nc.sync.dma_start(st1_in[None, 2 * HC + 1:520], zpad[:])
                esF.close()
                nc.gpsimd.collective_compute(
                    "AllGather", ALU.bypass, replica_groups=RG,
                    ins=[st1_in[:]], outs=[st1_all[:]])

                esG = ExitStack()
                sbG = esG.enter_context(tc.tile_pool(name="sbG", bufs=2))
                psG = esG.enter_context(
                    tc.tile_pool(name="psG", bufs=1, space="PSUM"))

                # transpose z1 into [hc, e] banks while the AllGather runs;
                # the bn1 affine is then per-partition and z2 needs no
                # in-loop transposes.
                z1T_t = [keep.tile([P, HC], dt.float16, tag=f"z1T{t}",
                                   name=f"z1T{t}") for t in range(nt_m)]
                for t in range(nt_m):
                    for b in range(2):
                        ps_t = psG.tile([P, P], dt.float16, space="PSUM",
                                        tag="ps_xth", bufs=2)
                        nc.tensor.transpose(ps_t[:],
                                            z1_t[t][:, b * P:(b + 1) * P],
                                            ident_h[:])
                        nc.vector.tensor_copy(z1T_t[t][:, b * P:(b + 1) * P],
                                              ps_t[:])

                ones8 = single.tile([8, 1], dt.float32)
                nc.vector.memset(ones8[:], 1.0)

                def combine_stats(st_all, width, tagn):
                    st8 = sbG.tile([8, width], dt.float32, tag=f"st8{tagn}",
                                   name=f"st8{tagn}")
                    nc.sync.dma_start(
                        st8[:], st_all[:].rearrange("(r c) -> r c", r=8))
                    ps_c = psG.tile([1, 520], dt.float32, space="PSUM",
                                    tag="pscmb", bufs=1, name=f"pscmb{tagn}")
                    for q0 in range(0, width, 512):
                        q1 = min(q0 + 512, width)
                        nc.tensor.matmul(ps_c[:, q0:q1], lhsT=ones8[:],
                                         rhs=st8[:, q0:q1], start=True,
                                         stop=True)
                    row = sbG.tile([1, width], dt.float32, tag=f"strow{tagn}",
                                   name=f"strow{tagn}")
                    nc.vector.tensor_copy(row[:], ps_c[:, 0:width])
                    return row

                st1_row = combine_stats(st1_all, 520, "a")

                def bn_rows(st_row, nch, g_in, b_in, row_off):
                    cnt_row = sbG.tile([1, 1], dt.float32, tag="bn_cnt",
                                       name="bn_cnt")
                    nc.vector.tensor_scalar_max(cnt_row[:],
                                                st1_row[:, 512:513], 1.0)
                    cr = sbG.tile([1, 1], dt.float32, tag="bn_cr", name="bn_cr")
                    nc.vector.reciprocal(cr[:], cnt_row[:])
                    mean = sbG.tile([1, nch], dt.float32, tag="bn_mean",
                                    name="bn_mean")
                    nc.vector.tensor_tensor(mean[:], st_row[:, 0:nch],
                                            cr[:].to_broadcast([1, nch]),
                                            op=ALU.mult)
                    var = sbG.tile([1, nch], dt.float32, tag="bn_var",
                                   name="bn_var")
                    nc.vector.tensor_tensor(var[:], st_row[:, nch:2 * nch],
                                            cr[:].to_broadcast([1, nch]),
                                            op=ALU.mult)
                    msq = sbG.tile([1, nch], dt.float32, tag="bn_msq",
                                   name="bn_msq")
                    nc.vector.tensor_mul(msq[:], mean[:], mean[:])
                    nc.vector.tensor_sub(var[:], var[:], msq[:])
                    nc.vector.tensor_scalar_add(var[:], var[:], 1e-5)
                    nc.scalar.activation(var[:], var[:], AF.Sqrt)
                    rstd = sbG.tile([1, nch], dt.float32, tag="bn_rstd",
                                    name="bn_rstd")
                    nc.vector.reciprocal(rstd[:], var[:])
                    g_row = sbG.tile([1, nch], dt.float32, tag="bn_g",
                                     name="bn_g")
                    nc.sync.dma_start(g_row[:], g_in[None, :])
                    b_row2 = sbG.tile([1, nch], dt.float32, tag="bn_b",
                                      name="bn_b")
                    nc.sync.dma_start(b_row2[:], b_in[None, :])
                    gs = sbG.tile([1, nch], dt.float32, tag="bn_gs",
                                  name="bn_gs")
                    nc.vector.tensor_mul(gs[:], g_row[:], rstd[:])
                    gb = sbG.tile([1, nch], dt.float32, tag="bn_gb",
                                  name="bn_gb")
                    nc.vector.tensor_mul(gb[:], mean[:], gs[:])
                    nc.vector.tensor_sub(gb[:], b_row2[:], gb[:])
                    nc.sync.dma_start(row_dram[None, row_off:row_off + nch],
                                      gs[:])
                    nc.sync.dma_start(
                        row_dram[None, row_off + nch:row_off + 2 * nch], gb[:])

                bn_rows(st1_row, HC, bn1g, bn1b, 0)
                gs1_c32 = single.tile([P, HC // P], dt.float32)
                nc.sync.dma_start(
                    gs1_c32[:], bass.AP(tensor=row_dram[:].tensor,
                                        offset=row_dram[:].offset,
                                        ap=[[1, P], [P, HC // P]]))
                gs1_c = single.tile([P, HC // P], dt.float16)
                nc.vector.tensor_copy(gs1_c[:], gs1_c32[:])
                gb1_c32 = single.tile([P, HC // P], dt.float32)
                nc.sync.dma_start(
                    gb1_c32[:], bass.AP(tensor=row_dram[:].tensor,
                                        offset=row_dram[:].offset + HC,
                                        ap=[[1, P], [P, HC // P]]))
                gb1_c = single.tile([P, HC // P], dt.float16)
                nc.vector.tensor_copy(gb1_c[:], gb1_c32[:])
                m2w_h = [keep.tile([P, HC2], dt.float16, tag=f"m2w{k}",
                                   name=f"m2w{k}") for k in range(HC // P)]
                for k in range(HC // P):
                    nc.sync.dma_start(m2w_h[k][:], m2w[k * P:(k + 1) * P, :])
                m2b_bc = single.tile([P, HC2], dt.float32)
                nc.sync.dma_start(m2b_bc[:], bcast(m2b, HC2))
                z2_t = [keep.tile([P, HC2], dt.float16, tag=f"z2_{t}",
                                  name=f"z2_{t}") for t in range(nt_m)]
                ps_s2 = psG.tile([1, HC2], dt.float32, space="PSUM",
                                 tag="ps_s2")
                ps_q2 = psG.tile([1, HC2], dt.float32, space="PSUM",
                                 tag="ps_q2")
                for t in range(nt_m):
                    znT = sbG.tile([P, HC], dt.float16, tag="znT", bufs=4)
                    for b in range(HC // P):
                        nc.vector.tensor_tensor(
                            znT[:, b * P:(b + 1) * P],
                            z1T_t[t][:, b * P:(b + 1) * P],
                            gs1_c[:, b:b + 1].to_broadcast([P, P]),
                            op=ALU.mult)
                        nc.vector.tensor_tensor(
                            znT[:, b * P:(b + 1) * P],
                            znT[:, b * P:(b + 1) * P],
                            gb1_c[:, b:b + 1].to_broadcast([P, P]),
                            op=ALU.add)
                    nc.scalar.activation(znT[:], znT[:], AF.Relu)
                    ps_z2 = psG.tile([P, HC2], dt.float32, space="PSUM",
                                     tag="ps_z2", bufs=1)
                    for b in range(HC // P):
                        nc.tensor.matmul(ps_z2[:],
                                         lhsT=znT[:, b * P:(b + 1) * P],
                                         rhs=m2w_h[b][:],
                                         start=(b == 0), stop=(b == HC // P - 1))
                    nc.vector.tensor_add(z2_t[t][:], ps_z2[:], m2b_bc[:])
                    zsq2 = sbG.tile([P, HC2], dt.float16, tag="zsq2", bufs=4)
                    nc.vector.tensor_mul(zsq2[:], z2_t[t][:], z2_t[t][:])
                    st, sp = (t == 0), (t == nt_m - 1)
                    nc.tensor.matmul(ps_s2[:], lhsT=mask_h[t][:],
                                     rhs=z2_t[t][:], start=st, stop=sp)
                    nc.tensor.matmul(ps_q2[:], lhsT=mask_h[t][:], rhs=zsq2[:],
                                     start=st, stop=sp)
                s2_sb = sbG.tile([1, HC2], dt.float32, tag="stat2")
                nc.vector.tensor_copy(s2_sb[:], ps_s2[:])
                nc.sync.dma_start(st2_in[None, 0:HC2], s2_sb[:])
                q2_sb = sbG.tile([1, HC2], dt.float32, tag="stat2")
                nc.vector.tensor_copy(q2_sb[:], ps_q2[:])
                nc.sync.dma_start(st2_in[None, HC2:2 * HC2], q2_sb[:])
                nc.gpsimd.collective_compute(
                    "AllGather", ALU.bypass, replica_groups=RG,
                    ins=[st2_in[:]], outs=[st2_all[:]])
                z2T_t = [keep.tile([P, HC2], dt.float16, tag=f"z2T{t}",
                                   name=f"z2T{t}") for t in range(nt_m)]
                for t in range(nt_m):
                    ps_t = psG.tile([P, P], dt.float16, space="PSUM",
                                    tag="ps_xth", bufs=2)
                    nc.tensor.transpose(ps_t[:], z2_t[t][:], ident_h[:])
                    nc.vector.tensor_copy(z2T_t[t][:], ps_t[:])
                st2_row = combine_stats(st2_all, 2 * HC2, "b")
                bn_rows(st2_row, HC2, bn2g, bn2b, 2 * HC)
                gs2_c32 = single.tile([P, 1], dt.float32)
                nc.sync.dma_start(gs2_c32[:],
                                  row_dram[2 * HC:2 * HC + HC2][:, None])
                gs2_c = single.tile([P, 1], dt.float16)
                nc.vector.tensor_copy(gs2_c[:], gs2_c32[:])
                gb2_c32 = single.tile([P, 1], dt.float32)
                nc.sync.dma_start(
                    gb2_c32[:],
                    row_dram[2 * HC + HC2:2 * HC + 2 * HC2][:, None])
                gb2_c = single.tile([P, 1], dt.float16)
                nc.vector.tensor_copy(gb2_c[:], gb2_c32[:])
                m3w_h = single.tile([P, 1], dt.float16)
                nc.sync.dma_start(m3w_h[:], m3w[:, :])
                m3b_bc = single.tile([P, 1], dt.float32)
                nc.sync.dma_start(m3b_bc[:], bcast(m3b, 1))
                neg25 = single.tile([P, 1], dt.float32)
                nc.vector.memset(neg25[:], -2.5)
                for t in range(nt_m):
                    zn2T = sbG.tile([P, HC2], dt.float16, tag="zn2T", bufs=4)
                    nc.vector.tensor_tensor(
                        zn2T[:], z2T_t[t][:],
                        gs2_c[:].to_broadcast([P, HC2]), op=ALU.mult)
                    nc.vector.tensor_tensor(
                        zn2T[:], zn2T[:],
                        gb2_c[:].to_broadcast([P, HC2]), op=ALU.add)
                    nc.scalar.activation(zn2T[:], zn2T[:], AF.Relu)
                    ps_sc = psG.tile([P, 1], dt.float32, space="PSUM",
                                     tag="ps_sc", bufs=1)
                    nc.tensor.matmul(ps_sc[:], lhsT=zn2T[:], rhs=m3w_h[:],
                                     start=True, stop=True)
                    score = sbG.tile([P, 1], dt.float32, tag="score", bufs=4)
                    nc.vector.tensor_add(score[:], ps_sc[:], m3b_bc[:])
                    sel = sbG.tile([P, 1], dt.float32, tag="sel", bufs=4)
                    nc.vector.select(sel[:], mask_u8[t][:], score[:], neg25[:])
                    nc.scalar.activation(sel[:], sel[:], AF.Sigmoid)
                    nc.sync.dma_start(score_out[t * P:(t + 1) * P][:, None],
                                      sel[:])
                esG.close()

    nc.compile()
    return nc


def kernel(**inputs):
    inputs = {k: np.asarray(v) for k, v in inputs.items()}
    src = inputs["edge_index"][0].astype(np.int64)
    dst = inputs["edge_index"][1].astype(np.int64)

    # --- edge partition by dst chunk (GAT set includes self loops) ---
    all_src = np.concatenate([src, np.arange(N, dtype=np.int64)])
    all_dst = np.concatenate([dst, np.arange(N, dtype=np.int64)])
    lidx_all = np.concatenate(
        [np.arange(E, dtype=np.int64), np.full(N, E, dtype=np.int64)])
    chunk_g = all_dst // NCHUNK
    gids = [np.where(chunk_g == c)[0] for c in range(NCORES)]
    nt_g = int(np.ceil(max(len(i) for i in gids) / P))
    pad_g = nt_g * P
    chunk_m = dst // NCHUNK
    mids = [np.where(chunk_m == c)[0] for c in range(NCORES)]
    nt_m = int(np.ceil(max(len(i) for i in mids) / P))
    pad_m = nt_m * P

    key = (nt_g, nt_m, DEBUG)
    if key not in _cache:
        _cache[key] = _build(nt_g, nt_m, debug=DEBUG)
    nc = _cache[key]

    def f32(v):
        return np.ascontiguousarray(v, dtype=np.float32)

    def f16(v):
        return np.ascontiguousarray(v, dtype=np.float16)

    shared = dict(
        xT=f16(inputs["x"].T),
        sent_emb=f32(inputs["sent_emb"]),
        elp=f32(inputs["edge_logits_param"]),
        elp_hi=f16(inputs["edge_logits_param"]),
        elp_lo=f16(inputs["edge_logits_param"]
                   - inputs["edge_logits_param"].astype(np.float16)
                     .astype(np.float32)),
        semb_hi=f16(inputs["sent_emb"]),
        semb_lo=f16(inputs["sent_emb"]
                    - inputs["sent_emb"].astype(np.float16)
                      .astype(np.float32)),
        fc0_w=f16(inputs["fc0_w"]), fc0_b=f32(inputs["fc0_b"]),
        fc1_w=f16(inputs["fc1_w"]), fc1_b=f32(inputs["fc1_b"]),
        conv1_W=f16(inputs["conv1_W"]),
        conv1_a=f16(np.concatenate([inputs["conv1_asrc"].reshape(-1),
                                    inputs["conv1_adst"].reshape(-1)])),
        conv1_b=f32(inputs["conv1_b"]),
        conv2_W=f16(inputs["conv2_W"]),
        conv2_a=f16(np.concatenate([inputs["conv2_asrc"].reshape(-1),
                                    inputs["conv2_adst"].reshape(-1)])),
        conv2_b=f32(inputs["conv2_b"]),
        mlp1_w=f16(inputs["mlp1_w"]), mlp1_b=f32(inputs["mlp1_b"]),
        bn1_g=f32(inputs["bn1_g"]), bn1_b=f32(inputs["bn1_b"]),
        mlp2_w=f16(inputs["mlp2_w"]), mlp2_b=f32(inputs["mlp2_b"]),
        bn2_g=f32(inputs["bn2_g"]), bn2_b=f32(inputs["bn2_b"]),
        mlp3_w=f16(inputs["mlp3_w"]), mlp3_b=f32(inputs["mlp3_b"]),
        fc2_b=f32(inputs["fc2_b"]),
    )

    fcl_w, fce_w, fc2_w = inputs["fcl_w"], inputs["fce_w"], inputs["fc2_w"]
    in_maps = []
    for c in range(NCORES):
        gi = gids[c]
        mi = mids[c]
        gsrc = np.zeros(pad_g, np.int32); gsrc[:len(gi)] = all_src[gi]
        gdst = np.zeros(pad_g, np.int32); gdst[:len(gi)] = all_dst[gi]
        gdstl = np.zeros(pad_g, np.int32)
        gdstl[:len(gi)] = all_dst[gi] - c * NCHUNK
        glidx = np.full(pad_g, E + 1, np.int32); glidx[:len(gi)] = lidx_all[gi]
        goh = np.zeros((pad_g, NCHUNK), np.float16)
        goh[np.arange(len(gi)), all_dst[gi] - c * NCHUNK] = 1.0
        msrc = np.zeros(pad_m, np.int32); msrc[:len(mi)] = src[mi]
        mdst = np.zeros(pad_m, np.int32); mdst[:len(mi)] = dst[mi]
        mdstl = np.zeros(pad_m, np.int32)
        mdstl[:len(mi)] = dst[mi] - c * NCHUNK
        mlidx = np.full(pad_m, E + 1, np.int32); mlidx[:len(mi)] = mi
        m = dict(shared)
        def hilo(w, blk=None):
            w = np.asarray(w, np.float32)
            h = w.astype(np.float16)
            l = (w - h.astype(np.float32)).astype(np.float16)
            if blk is None:
                return np.ascontiguousarray(np.concatenate([h, l], axis=1))
            # interleave per column-block: [h0|l0|h1|l1|...]
            parts = []
            for b in range(w.shape[1] // blk):
                parts.append(h[:, b * blk:(b + 1) * blk])
                parts.append(l[:, b * blk:(b + 1) * blk])
            return np.ascontiguousarray(np.concatenate(parts, axis=1))

        fc2w_c = np.asarray(np.concatenate(
            [fc2_w[c * ECH:(c + 1) * ECH],
             fc2_w[E + c * ECH:E + (c + 1) * ECH]], axis=0), np.float32)
        m.update(
            fclw_hl=hilo(fcl_w[:, c * ECH:(c + 1) * ECH]),
            fclb_sh=f32(inputs["fcl_b"][c * ECH:(c + 1) * ECH]),
            fcew_hl=hilo(fce_w[:, c * ECH:(c + 1) * ECH]),
            fceb_sh=f32(inputs["fce_b"][c * ECH:(c + 1) * ECH]),
            fc2w_hl=hilo(fc2w_c, blk=2048),
            g_src=gsrc, g_dst=gdst, g_dstl=gdstl, g_lidx=glidx, g_oh=goh,
            m_src=msrc, m_dst=mdst, m_dstl=mdstl, m_lidx=mlidx,
        )
        in_maps.append(m)

    global last_results, last_in_maps
    last_in_maps = in_maps
    res = run_bass_kernel_spmd(nc, in_maps, core_ids=list(range(NCORES)),
                               trace=TRACE)
    last_results = res
    orig = res.results[0]["orig_out"].reshape(E).astype(np.float32)
    sig = np.empty(E, np.float32)
    for c in range(NCORES):
        mi = mids[c]
        sig[mi] = res.results[c]["score_out"].reshape(pad_m)[:len(mi)]
    return sig, orig


# revision 41
# speedup vs baseline: 1.1143x; 1.0244x over previous
"""GATWithSentenceEmbedding Trainium2 kernel (8 NeuronCores, SPMD + collectives).

V2 restructure vs baseline:
  - Phase B (fcl/fce/fc2 streaming) uses rhs-side weight tiles with [1,N]
    matvec psums: weights stream row-major straight from DRAM (no rearrange,
    no per-j Ldweights storm).  Stream DMAs dispatch on the Pool engine so
    they never stall behind compute-gated dispatches.
  - Phase A: host stages x^T, so h^T and xp1 are computed with zero on-device
    transposes.  Small weights are host-cast to fp16 (halves DMA + removes
    DVE convert passes).
  - conv1 gather tiles (xs/ad/oh) are prefetched into SBUF during the weight
    stream; only the mask-dependent work runs after the logits AllReduce.
  - BN stats use AllGather + local ones-matmul combine instead of AllReduce
    (15.4us vs 28.2us each).
  - conv2 reuses conv1's one-hot tiles.
"""

import numpy as np
from contextlib import ExitStack

import concourse.bass as bass
import concourse.mybir as mybir
import concourse.tile as tile
from concourse import bacc
from concourse.bass_utils import run_bass_kernel_spmd
from concourse.masks import make_identity

N, F, HC, S, H, E, BERT = 2048, 256, 256, 512, 4, 8192, 768
NCORES = 8
P = 128
NCHUNK = N // NCORES          # 256 dst nodes per core
ECH = E // NCORES             # 1024 g1/g2 columns per core
XP1W = H * HC + 2 * H         # 1032 = xp1 | al_s | al_d
XP2W = F + 2                  # 258  = xp2 | al_s | al_d
HC2 = HC // 2                 # 128
BIG = 1.0e9

dt = mybir.dt
AF = mybir.ActivationFunctionType
ALU = mybir.AluOpType
RG = [list(range(NCORES))]

_cache = {}
last_in_maps = None
DEBUG = False
TRACE = False
last_results = None


def _build(nt_g: int, nt_m: int, debug: bool = False, stage: int = 4):
    pad_g = nt_g * P
    pad_m = nt_m * P
    nc = bacc.Bacc("TRN2", target_bir_lowering=False, debug=False)

    def inp(name, shape, dtype=dt.float32):
        return nc.dram_tensor(name, shape, dtype, kind="ExternalInput")

    # shared inputs (host-cast fp16 where precision allows)
    xT_in = inp("xT", [F, N], dt.float16)
    sent_in = inp("sent_emb", [BERT])
    elp_in = inp("elp", [E])
    fc0_w = inp("fc0_w", [BERT, S], dt.float16)
    fc0_b = inp("fc0_b", [S])
    fc1_w = inp("fc1_w", [F, S], dt.float16)
    fc1_b = inp("fc1_b", [S])
    c1w = inp("conv1_W", [S, H * HC], dt.float16)
    c1a = inp("conv1_a", [2 * H * HC], dt.float16)
    c1b = inp("conv1_b", [H * HC])
    c2w = inp("conv2_W", [H * HC, F], dt.float16)
    c2a = inp("conv2_a", [2 * F], dt.float16)
    c2b = inp("conv2_b", [F])
    m1w = inp("mlp1_w", [4 * F, HC], dt.float16)
    m1b = inp("mlp1_b", [HC])
    bn1g = inp("bn1_g", [HC]); bn1b = inp("bn1_b", [HC])
    m2w = inp("mlp2_w", [HC, HC2], dt.float16)
    m2b = inp("mlp2_b", [HC2])
    bn2g = inp("bn2_g", [HC2]); bn2b = inp("bn2_b", [HC2])
    m3w = inp("mlp3_w", [HC2, 1], dt.float16)
    m3b = inp("mlp3_b", [1])
    fc2_b = inp("fc2_b", [E])
    # per-core inputs (row-major, streamed as rhs tiles).  Each big matrix is
    # host-split into fp16 hi + fp16 lo (lo = x - fp16(x)); the matvec runs
    # hi*Whi + lo*Whi + hi*Wlo at fp16 PE rate (4x the fp32 rate) with
    # ~2^-22 effective precision (lo*Wlo dropped).
    fclw_hl = inp("fclw_hl", [E, 2 * ECH], dt.float16)
    fclb_sh = inp("fclb_sh", [ECH])
    fcew_hl = inp("fcew_hl", [BERT, 2 * ECH], dt.float16)
    fceb_sh = inp("fceb_sh", [ECH])
    fc2w_hl = inp("fc2w_hl", [2 * ECH, 2 * E], dt.float16)
    elp_hi = inp("elp_hi", [E], dt.float16)
    elp_lo = inp("elp_lo", [E], dt.float16)
    semb_hi = inp("semb_hi", [BERT], dt.float16)
    semb_lo = inp("semb_lo", [BERT], dt.float16)
    g_src = inp("g_src", [pad_g], dt.int32)
    g_dst = inp("g_dst", [pad_g], dt.int32)
    g_dstl = inp("g_dstl", [pad_g], dt.int32)
    g_lidx = inp("g_lidx", [pad_g], dt.int32)
    g_oh = inp("g_oh", [pad_g, NCHUNK], dt.float16)
    m_src = inp("m_src", [pad_m], dt.int32)
    m_dst = inp("m_dst", [pad_m], dt.int32)
    m_dstl = inp("m_dstl", [pad_m], dt.int32)
    m_lidx = inp("m_lidx", [pad_m], dt.int32)
    # outputs
    orig_out = nc.dram_tensor("orig_out", [E], dt.float32, kind="ExternalOutput")
    score_out = nc.dram_tensor("score_out", [pad_m], dt.float32,
                               kind="ExternalOutput")

    def bcast(dram_handle, cols, offset=0):
        """AP reading a [1, cols] DRAM row replicated over 128 partitions."""
        return bass.AP(tensor=dram_handle.ap().tensor, offset=offset,
                       ap=[[0, P], [1, cols]])

    def bcast_ap(ap_tile, cols, offset=0):
        a = ap_tile[:] if not isinstance(ap_tile, bass.AP) else ap_tile
        return bass.AP(tensor=a.tensor, offset=a.offset + offset,
                       ap=[[0, P], [1, cols]])

    with tile.TileContext(nc) as tc:
        with (
            tc.tile_pool(name="dram", bufs=1, space="DRAM") as dram,
            tc.tile_pool(name="single", bufs=1) as single,
            tc.tile_pool(name="sb", bufs=4) as sb,
            tc.tile_pool(name="keep", bufs=1) as keep,
        ):
            ident_h = single.tile([P, P], dt.float16)
            make_identity(nc, ident_h[:])

            # internal DRAM
            xp1_dram = dram.tile([N, XP1W], dt.float16)
            al1d_dram = dram.tile([N, 2 * H], dt.float16)
            fc2part = dram.tile([E], dt.float32)
            logits_dram = dram.tile([E], dt.float32, addr_space="Shared")
            lext_dram = dram.tile([E + 2, 1], dt.float32)
            sent_dram = dram.tile([S], dt.float32)
            gd_dram = dram.tile([2 * ECH], dt.float32)
            xp2_in = dram.tile([NCHUNK, XP2W], dt.float16)
            xp2_dram = dram.tile([N, XP2W], dt.float16, addr_space="Shared")
            h2_in = dram.tile([NCHUNK, F], dt.float16)
            h2_dram = dram.tile([N, F], dt.float16, addr_space="Shared")
            st1_in = dram.tile([520], dt.float32)
            st1_all = dram.tile([8 * 520], dt.float32, addr_space="Shared")
            st2_in = dram.tile([2 * HC2], dt.float32)
            st2_all = dram.tile([8 * 2 * HC2], dt.float32, addr_space="Shared")
            row_dram = dram.tile([4 * HC], dt.float32)  # gs/gb rows for bcast

            # =============== phase A: inputs + hT + xp1 (SP engine DMAs) ====
            # Phases A and B overlap at runtime, sharing 8 PSUM banks via two
            # tags: ps_big [128,2048] (sent-free; hT then xp1 cycle it) and
            # ps_row [1,2048] (sent, g1, g2, then fc2 blocks cycle it).
            esA = ExitStack()
            sbA = esA.enter_context(tc.tile_pool(name="sbA", bufs=2))
            psAB = esA.enter_context(
                tc.tile_pool(name="psAB", bufs=1, space="PSUM"))

            def row_ps():
                return psAB.tile([1, 2048], dt.float32, space="PSUM",
                                 tag="ps_row", bufs=1, name="ps_row")

            def big_ps():
                return psAB.tile([P, N], dt.float32, space="PSUM",
                                 tag="ps_big", bufs=1, name="ps_big")

            xT_t = [sbA.tile([P, N], dt.float16, tag=f"xT{k}", bufs=1,
                             name=f"xT{k}") for k in range(F // P)]
            for k in range(F // P):
                nc.sync.dma_start(xT_t[k][:], xT_in[k * P:(k + 1) * P, :])
            fc1w_t = [sbA.tile([P, S], dt.float16, tag=f"fc1w{k}", bufs=1,
                               name=f"fc1w{k}") for k in range(F // P)]
            for k in range(F // P):
                nc.sync.dma_start(fc1w_t[k][:], fc1_w[k * P:(k + 1) * P, :])
            fc0w_t = [sbA.tile([P, S], dt.float16, tag=f"fc0w{k}", bufs=1,
                               name=f"fc0w{k}") for k in range(BERT // P)]
            for k in range(BERT // P):
                nc.sync.dma_start(fc0w_t[k][:], fc0_w[k * P:(k + 1) * P, :])
            semb32 = single.tile([P, BERT // P], dt.float32)
            nc.sync.dma_start(semb32[:],
                              sent_in.ap().rearrange("(k p) -> p k", p=P))
            semb16 = single.tile([P, BERT // P], dt.float16)
            nc.vector.tensor_copy(semb16[:], semb32[:])
            elp_hc = single.tile([P, E // P], dt.float16)
            nc.sync.dma_start(elp_hc[:],
                              elp_hi.ap().rearrange("(k p) -> p k", p=P))
            elp_lc = single.tile([P, E // P], dt.float16)
            nc.sync.dma_start(elp_lc[:],
                              elp_lo.ap().rearrange("(k p) -> p k", p=P))
            semb_hc = single.tile([P, BERT // P], dt.float16)
            nc.sync.dma_start(semb_hc[:],
                              semb_hi.ap().rearrange("(k p) -> p k", p=P))
            semb_lc = single.tile([P, BERT // P], dt.float16)
            nc.sync.dma_start(semb_lc[:],
                              semb_lo.ap().rearrange("(k p) -> p k", p=P))

            # sent = relu(sent_emb @ fc0_w + fc0_b) as a [1, S] row
            ps_sent = row_ps()
            for k in range(BERT // P):
                nc.tensor.matmul(ps_sent[:, 0:S], lhsT=semb16[:, k:k + 1],
                                 rhs=fc0w_t[k][:],
                                 start=(k == 0), stop=(k == BERT // P - 1))
            sent_row = sbA.tile([1, S], dt.float32, tag="sentrow", bufs=1)
            b_row = sbA.tile([1, S], dt.float32, tag="fc0brow", bufs=1)
            nc.sync.dma_start(b_row[:], fc0_b[None, :])
            nc.vector.tensor_add(sent_row[:], ps_sent[:, 0:S], b_row[:])
            nc.scalar.activation(sent_row[:], sent_row[:], AF.Relu)
            nc.sync.dma_start(sent_dram[:][None, :], sent_row[:])
            sent_col = single.tile([P, S // P], dt.float32)
            nc.sync.dma_start(sent_col[:],
                              sent_dram[:].rearrange("(k p) -> p k", p=P))
            fc1b_col = single.tile([P, S // P], dt.float32)
            nc.sync.dma_start(fc1b_col[:],
                              fc1_b.ap().rearrange("(k p) -> p k", p=P))

            # hT[s, n] = relu(xT^T-free GEMM) + sent, fully transpose-free
            hT_t = [sbA.tile([P, N], dt.float16, tag=f"hT{s}", bufs=1,
                             name=f"hT{s}") for s in range(S // P)]
            for si in range(S // P):
                ps_hT = big_ps()
                for k in range(F // P):
                    for q0 in range(0, N, 512):
                        nc.tensor.matmul(
                            ps_hT[:, q0:q0 + 512],
                            lhsT=fc1w_t[k][:, si * P:(si + 1) * P],
                            rhs=xT_t[k][:, q0:q0 + 512],
                            start=(k == 0), stop=(k == F // P - 1))
                nc.scalar.activation(hT_t[si][:], ps_hT[:], AF.Relu,
                                      bias=fc1b_col[:, si:si + 1])
                nc.vector.tensor_tensor(
                    hT_t[si][:], hT_t[si][:],
                    sent_col[:, si:si + 1].to_broadcast([P, N]), op=ALU.add)

            # W1aug = [conv1_W | W@a_src | W@a_dst] as 4 k-tiles [128, 1032] f16
            c1a_bc = sbA.tile([P, 2 * H * HC], dt.float16, tag="c1abc", bufs=1)
            nc.sync.dma_start(c1a_bc[:], bcast(c1a, 2 * H * HC))
            w1aug_h = [sbA.tile([P, XP1W], dt.float16, tag=f"w1aug{k}", bufs=1,
                                name=f"w1aug{k}") for k in range(S // P)]
            lp = nc.allow_low_precision(
                "fp16 a-vector projections only shape GAT softmax logits")
            lp.__enter__()
            for k in range(S // P):
                nc.sync.dma_start(w1aug_h[k][:, 0:H * HC],
                                  c1w[k * P:(k + 1) * P, :])
                tmp = sbA.tile([P, H * HC], dt.float16, tag="scratch4k")
                nc.vector.tensor_mul(tmp[:], w1aug_h[k][:, 0:H * HC],
                                     c1a_bc[:, 0:H * HC])
                for h in range(H):
                    nc.vector.reduce_sum(
                        w1aug_h[k][:, H * HC + h:H * HC + h + 1],
                        tmp[:, h * HC:(h + 1) * HC], axis=mybir.AxisListType.X)
                nc.vector.tensor_mul(tmp[:], w1aug_h[k][:, 0:H * HC],
                                     c1a_bc[:, H * HC:2 * H * HC])
                for h in range(H):
                    nc.vector.reduce_sum(
                        w1aug_h[k][:, H * HC + H + h:H * HC + H + h + 1],
                        tmp[:, h * HC:(h + 1) * HC], axis=mybir.AxisListType.X)

            # xp1 = h @ W1aug per node-tile; lhsT = hT slice (no transposes)

            def emit_xp1_tile(nt):
                ps_xp1 = big_ps()
                for si in range(S // P):
                    for s0, s1 in ((0, 512), (512, 1024), (1024, XP1W)):
                        nc.tensor.matmul(
                            ps_xp1[:, s0:s1],
                            lhsT=hT_t[si][:, nt * P:(nt + 1) * P],
                            rhs=w1aug_h[si][:, s0:s1],
                            start=(si == 0), stop=(si == S // P - 1))
                xp1_t = sbA.tile([P, XP1W], dt.float16, tag="xp1")
                nc.vector.tensor_copy(xp1_t[:], ps_xp1[:, 0:XP1W])
                nc.sync.dma_start(xp1_dram[nt * P:(nt + 1) * P, :], xp1_t[:])
                nc.sync.dma_start(al1d_dram[nt * P:(nt + 1) * P, :],
                                  xp1_t[:, H * HC:H * HC + 2 * H])

            # ======== phase B: weight streaming (Pool engine DMAs) ========
            sbB = esA.enter_context(tc.tile_pool(name="sbB", bufs=2))

            # g1 = relu(elp @ fcl_w + b): [1, ECH] psum row, hi/lo passes
            ps_g1 = row_ps()
            for k in range(E // P):
                wt = sbB.tile([P, 2 * ECH], dt.float16, tag="wfcl", bufs=3,
                              name="wfcl")
                nc.gpsimd.dma_start(wt[:], fclw_hl[k * P:(k + 1) * P, :])
                for q0 in range(0, ECH, 512):
                    nc.tensor.matmul(ps_g1[:, q0:q0 + 512],
                                     lhsT=elp_hc[:, k:k + 1],
                                     rhs=wt[:, q0:q0 + 512],
                                     start=(k == 0), stop=False)
                    nc.tensor.matmul(ps_g1[:, q0:q0 + 512],
                                     lhsT=elp_lc[:, k:k + 1],
                                     rhs=wt[:, q0:q0 + 512],
                                     start=False, stop=False)
                    nc.tensor.matmul(ps_g1[:, q0:q0 + 512],
                                     lhsT=elp_hc[:, k:k + 1],
                                     rhs=wt[:, ECH + q0:ECH + q0 + 512],
                                     start=False, stop=(k == E // P - 1))
            g1_row = sbB.tile([1, ECH], dt.float32, tag="g1row", bufs=1,
                              name="g1row")
            gb_row = sbB.tile([1, ECH], dt.float32, tag="gbrow", bufs=1)
            nc.sync.dma_start(gb_row[:], fclb_sh[None, :])
            nc.vector.tensor_add(g1_row[:], ps_g1[:, 0:ECH], gb_row[:])
            nc.scalar.activation(g1_row[:], g1_row[:], AF.Relu)
            nc.sync.dma_start(gd_dram[0:ECH][None, :], g1_row[:])

            # g2 = relu(sent_emb @ fce_w + b)
            ps_g2 = row_ps()
            for k in range(BERT // P):
                wt = sbB.tile([P, 2 * ECH], dt.float16, tag="wfcl", bufs=3,
                              name="wfce")
                nc.gpsimd.dma_start(wt[:], fcew_hl[k * P:(k + 1) * P, :])
                for q0 in range(0, ECH, 512):
                    nc.tensor.matmul(ps_g2[:, q0:q0 + 512],
                                     lhsT=semb_hc[:, k:k + 1],
                                     rhs=wt[:, q0:q0 + 512],
                                     start=(k == 0), stop=False)
                    nc.tensor.matmul(ps_g2[:, q0:q0 + 512],
                                     lhsT=semb_lc[:, k:k + 1],
                                     rhs=wt[:, q0:q0 + 512],
                                     start=False, stop=False)
                    nc.tensor.matmul(ps_g2[:, q0:q0 + 512],
                                     lhsT=semb_hc[:, k:k + 1],
                                     rhs=wt[:, ECH + q0:ECH + q0 + 512],
                                     start=False, stop=(k == BERT // P - 1))
            g2_row = sbB.tile([1, ECH], dt.float32, tag="g2row", bufs=1,
                              name="g2row")
            gb2_row = sbB.tile([1, ECH], dt.float32, tag="gbrow", bufs=1)
            nc.sync.dma_start(gb2_row[:], fceb_sh[None, :])
            nc.vector.tensor_add(g2_row[:], ps_g2[:, 0:ECH], gb2_row[:])
            nc.scalar.activation(g2_row[:], g2_row[:], AF.Relu)
            nc.sync.dma_start(gd_dram[ECH:2 * ECH][None, :], g2_row[:])

            # g hi/lo split done in tiny [128, 16] column space
            g_col32 = single.tile([P, 2 * ECH // P], dt.float32)
            nc.sync.dma_start(g_col32[:],
                              gd_dram[:].rearrange("(k p) -> p k", p=P))
            g_colh = single.tile([P, 2 * ECH // P], dt.float16)
            nc.vector.tensor_copy(g_colh[:], g_col32[:])
            g_colh32 = single.tile([P, 2 * ECH // P], dt.float32)
            nc.vector.tensor_copy(g_colh32[:], g_colh[:])
            g_coll32 = single.tile([P, 2 * ECH // P], dt.float32)
            nc.vector.tensor_sub(g_coll32[:], g_col32[:], g_colh32[:])
            g_coll = single.tile([P, 2 * ECH // P], dt.float16)
            nc.vector.tensor_copy(g_coll[:], g_coll32[:])

            # fc2 partial: column-block streaming, xp1 tiles interleaved so
            # the PE fills DMA-pacing slack without stalling the stream.
            CB = 2048
            NXB = (N // P) // (E // CB)
            for c in range(E // CB):
                ps_f2 = row_ps()
                for k in range(2 * ECH // P):
                    wt = sbB.tile([P, 2 * CB], dt.float16, tag="wfc2", bufs=3,
                                  name="wfc2")
                    nc.gpsimd.dma_start(
                        wt[:], fc2w_hl[k * P:(k + 1) * P,
                                       2 * c * CB:2 * (c + 1) * CB])
                    for q0 in range(0, CB, 512):
                        nc.tensor.matmul(ps_f2[:, q0:q0 + 512],
                                         lhsT=g_colh[:, k:k + 1],
                                         rhs=wt[:, q0:q0 + 512],
                                         start=(k == 0), stop=False)
                        nc.tensor.matmul(ps_f2[:, q0:q0 + 512],
                                         lhsT=g_coll[:, k:k + 1],
                                         rhs=wt[:, q0:q0 + 512],
                                         start=False, stop=False)
                        nc.tensor.matmul(ps_f2[:, q0:q0 + 512],
                                         lhsT=g_colh[:, k:k + 1],
                                         rhs=wt[:, CB + q0:CB + q0 + 512],
                                         start=False,
                                         stop=(k == 2 * ECH // P - 1))
                f2row = sbB.tile([1, CB], dt.float32, tag="f2row", bufs=1)
                nc.vector.tensor_copy(f2row[:], ps_f2[:, 0:CB])
                nc.sync.dma_start(fc2part[c * CB:(c + 1) * CB][None, :],
                                  f2row[:])
                for nt in range(c * NXB, (c + 1) * NXB):
                    emit_xp1_tile(nt)

            # ======== conv1 gather prefetch (during fc2 stream) ========
            gsrc_sb = single.tile([P, nt_g], dt.int32)
            nc.sync.dma_start(gsrc_sb[:],
                              g_src.ap().rearrange("(t p) -> p t", p=P))
            gdst_sb = single.tile([P, nt_g], dt.int32)
            nc.sync.dma_start(gdst_sb[:],
                              g_dst.ap().rearrange("(t p) -> p t", p=P))
            glidx_sb = single.tile([P, nt_g], dt.int32)
            nc.sync.dma_start(glidx_sb[:],
                              g_lidx.ap().rearrange("(t p) -> p t", p=P))
            gdstl_sb = single.tile([P, nt_g], dt.int32)
            nc.sync.dma_start(gdstl_sb[:],
                              g_dstl.ap().rearrange("(t p) -> p t", p=P))
            msrc_sb = single.tile([P, nt_m], dt.int32)
            nc.sync.dma_start(msrc_sb[:],
                              m_src.ap().rearrange("(t p) -> p t", p=P))
            mdstl_sb = single.tile([P, nt_m], dt.int32)
            nc.sync.dma_start(mdstl_sb[:],
                              m_dstl.ap().rearrange("(t p) -> p t", p=P))
            mlidx_sb = single.tile([P, nt_m], dt.int32)
            nc.sync.dma_start(mlidx_sb[:],
                              m_lidx.ap().rearrange("(t p) -> p t", p=P))
            oh_t = [keep.tile([P, NCHUNK], dt.float16, tag=f"oh{t}",
                              name=f"oh{t}") for t in range(nt_g)]
            xs_t = [keep.tile([P, XP1W], dt.float16, tag=f"xs{t}",
                              name=f"xs{t}") for t in range(nt_g)]
            msgu_t = [keep.tile([P, H * HC + H], dt.float16, tag=f"msgu{t}",
                               name=f"msgu{t}") for t in range(nt_g)]
            for t in range(nt_g):
                nc.sync.dma_start(oh_t[t][:], g_oh[t * P:(t + 1) * P, :])
                nc.gpsimd.indirect_dma_start(
                    out=xs_t[t][:], out_offset=None, in_=xp1_dram[:],
                    in_offset=bass.IndirectOffsetOnAxis(
                        ap=gsrc_sb[:, t:t + 1], axis=0))
                ad = sbA.tile([P, 2 * H], dt.float16, tag="gad", bufs=4)
                nc.gpsimd.indirect_dma_start(
                    out=ad[:], out_offset=None, in_=al1d_dram[:],
                    in_offset=bass.IndirectOffsetOnAxis(
                        ap=gdst_sb[:, t:t + 1], axis=0))
                # alpha/exp and the unmasked message xs*ex are mask-free:
                # compute them during the stream, leaving only *valid and the
                # one-hot matmuls for after the logits AllReduce.
                alpha = sbA.tile([P, H], dt.float32, tag="alpha", bufs=4)
                nc.vector.tensor_add(alpha[:],
                                     xs_t[t][:, H * HC:H * HC + H],
                                     ad[:, H:2 * H])
                nc.vector.scalar_tensor_tensor(alpha[:], alpha[:], 0.2,
                                               alpha[:],
                                               op0=ALU.mult, op1=ALU.max)
                ex = sbA.tile([P, H], dt.float32, tag="ex", bufs=4)
                nc.scalar.activation(ex[:], alpha[:], AF.Exp)
                for h in range(H):
                    nc.vector.tensor_tensor(
                        msgu_t[t][:, h * HC:(h + 1) * HC],
                        xs_t[t][:, h * HC:(h + 1) * HC],
                        ex[:, h:h + 1].to_broadcast([P, HC]), op=ALU.mult)
                nc.vector.tensor_copy(msgu_t[t][:, H * HC:H * HC + H], ex[:])
            esA.close()

            if stage >= 2:
                nc.gpsimd.collective_compute(
                    "AllReduce", ALU.add, replica_groups=RG,
                    ins=[fc2part[:]], outs=[logits_dram[:]])
                # logits += fc2_b ; orig_out ; logits_ext
                lg_pf = single.tile([P, E // P], dt.float32)
                nc.sync.dma_start(lg_pf[:],
                                  logits_dram[:].rearrange("(p f) -> p f", p=P))
                f2b_pf = single.tile([P, E // P], dt.float32)
                nc.sync.dma_start(f2b_pf[:],
                                  fc2_b.ap().rearrange("(p f) -> p f", p=P))
                nc.vector.tensor_add(lg_pf[:], lg_pf[:], f2b_pf[:])
                nc.sync.dma_start(orig_out.ap().rearrange("(p f) -> p f", p=P),
                                  lg_pf[:])
                nc.sync.dma_start(
                    lext_dram[0:E, :].rearrange("(p f) x -> p (f x)", p=P),
                    lg_pf[:])
                big_t = single.tile([1, 2], dt.float32)
                nc.vector.memset(big_t[:, 0:1], BIG)
                nc.vector.memset(big_t[:, 1:2], -BIG)
                nc.sync.dma_start(lext_dram[E:E + 2, 0][None, :], big_t[:])


                # ============ conv1 aggregation (mask-dependent part) =======
                valid_t = [keep.tile([P, 1], dt.float32, tag=f"valid{t}",
                                     name=f"valid{t}") for t in range(nt_g)]
                esC = ExitStack()
                sbC = esC.enter_context(tc.tile_pool(name="sbC", bufs=2))
                psC = esC.enter_context(
                    tc.tile_pool(name="psC", bufs=1, space="PSUM"))
                ps_msg = [psC.tile([P, H * HC], dt.float32, space="PSUM",
                                   tag=f"ps_msg{d}", name=f"ps_msg{d}")
                          for d in range(2)]
                ps_den = [psC.tile([P, H], dt.float32, space="PSUM",
                                   tag=f"ps_den{d}", name=f"ps_den{d}")
                          for d in range(2)]
                for t in range(nt_g):
                    lg = sbC.tile([P, 1], dt.float32, tag="glg", bufs=8)
                    nc.gpsimd.indirect_dma_start(
                        out=lg[:], out_offset=None, in_=lext_dram[:],
                        in_offset=bass.IndirectOffsetOnAxis(
                            ap=glidx_sb[:, t:t + 1], axis=0))
                    nc.vector.tensor_scalar(valid_t[t][:], lg[:], 0.0, None,
                                            op0=ALU.is_gt)
                for t in range(nt_g):
                    msg = sbC.tile([P, H * HC + H], dt.float16, tag="msg",
                                   bufs=4)
                    nc.vector.tensor_tensor(
                        msg[:], msgu_t[t][:],
                        valid_t[t][:].to_broadcast([P, H * HC + H]),
                        op=ALU.mult)
                    for d in range(2):
                        lhsT = oh_t[t][:, d * P:(d + 1) * P]
                        st, sp = (t == 0), (t == nt_g - 1)
                        nc.tensor.matmul(ps_msg[d][:, 0:512], lhsT=lhsT,
                                         rhs=msg[:, 0:512], start=st, stop=sp)
                        nc.tensor.matmul(ps_msg[d][:, 512:1024], lhsT=lhsT,
                                         rhs=msg[:, 512:1024], start=st,
                                         stop=sp)
                        nc.tensor.matmul(ps_den[d][:], lhsT=lhsT,
                                         rhs=msg[:, H * HC:H * HC + H],
                                         start=st, stop=sp)
                # prefetch MLP masks now that lext is final
                if stage >= 4:
                    mask_f = [keep.tile([P, 1], dt.float32, tag=f"maskf{t}",
                                        name=f"maskf{t}") for t in range(nt_m)]
                    mask_u8 = [keep.tile([P, 1], dt.uint8, tag=f"masku{t}",
                                         name=f"masku{t}") for t in range(nt_m)]
                    for t in range(nt_m):
                        mlg = sbC.tile([P, 1], dt.float32, tag="mlg", bufs=6)
                        nc.gpsimd.indirect_dma_start(
                            out=mlg[:], out_offset=None, in_=lext_dram[:],
                            in_offset=bass.IndirectOffsetOnAxis(
                                ap=mlidx_sb[:, t:t + 1], axis=0))
                        nc.vector.tensor_scalar(mask_f[t][:], mlg[:], 0.0,
                                                None, op0=ALU.is_gt)
                        nc.vector.tensor_copy(mask_u8[t][:], mask_f[t][:])

                # finalize conv1 (+elu) and xp2aug weights
                c1b_bc = sbC.tile([P, H * HC], dt.float32, tag="c1bbc", bufs=1)
                nc.sync.dma_start(c1b_bc[:], bcast(c1b, H * HC))
                c2a_bc = sbC.tile([P, 2 * F], dt.float16, tag="c2abc", bufs=1)
                nc.sync.dma_start(c2a_bc[:], bcast(c2a, 2 * F))
                w2aug_h = [keep.tile([P, XP2W], dt.float16, tag=f"w2aug{k}",
                                     name=f"w2aug{k}")
                           for k in range(H * HC // P)]
                for k in range(H * HC // P):
                    nc.sync.dma_start(w2aug_h[k][:, 0:F],
                                      c2w[k * P:(k + 1) * P, :])
                    tmp = sbC.tile([P, F], dt.float16, tag="w2tmp")
                    nc.vector.tensor_mul(tmp[:], w2aug_h[k][:, 0:F],
                                         c2a_bc[:, 0:F])
                    nc.vector.reduce_sum(w2aug_h[k][:, F:F + 1], tmp[:],
                                         axis=mybir.AxisListType.X)
                    nc.vector.tensor_mul(tmp[:], w2aug_h[k][:, 0:F],
                                         c2a_bc[:, F:2 * F])
                    nc.vector.reduce_sum(w2aug_h[k][:, F + 1:F + 2], tmp[:],
                                         axis=mybir.AxisListType.X)
                h1_keep = [keep.tile([P, H * HC], dt.float16, tag=f"h1k{d}",
                                     name=f"h1k{d}") for d in range(2)]
                for d in range(2):
                    denr = sbC.tile([P, H], dt.float32, tag="denr")
                    nc.vector.reciprocal(denr[:], ps_den[d][:])
                    h1_t = h1_keep[d]
                    h1f = sbC.tile([P, H * HC], dt.float32, tag="h1f")
                    for h in range(H):
                        nc.vector.scalar_tensor_tensor(
                            h1f[:, h * HC:(h + 1) * HC],
                            ps_msg[d][:, h * HC:(h + 1) * HC],
                            denr[:, h:h + 1],
                            c1b_bc[:, h * HC:(h + 1) * HC],
                            op0=ALU.mult, op1=ALU.add)
                    # elu = relu(x) + exp(min(x,0)) - 1
                    relu_t = sbC.tile([P, H * HC], dt.float32, tag="elu_r")
                    nc.scalar.activation(relu_t[:], h1f[:], AF.Relu)
                    nc.vector.tensor_scalar_min(h1f[:], h1f[:], 0.0)
                    nc.scalar.activation(h1f[:], h1f[:], AF.Exp)
                    nc.vector.scalar_tensor_tensor(h1_t[:], h1f[:], -1.0,
                                                   relu_t[:],
                                                   op0=ALU.add, op1=ALU.add)
                esC.close()
                esD = ExitStack()
                sbD = esD.enter_context(tc.tile_pool(name="sbD", bufs=2))
                psD = esD.enter_context(
                    tc.tile_pool(name="psD", bufs=1, space="PSUM"))
                for d in range(2):
                    h1_t = h1_keep[d]
                    ps_xp2 = psD.tile([P, XP2W], dt.float32, space="PSUM",
                                      tag="ps_xp2")
                    for k in range(H * HC // P):
                        ps_h1t = psD.tile([P, P], dt.float16, space="PSUM",
                                          tag="ps_xth", bufs=4)
                        nc.tensor.transpose(ps_h1t[:],
                                            h1_t[:, k * P:(k + 1) * P],
                                            ident_h[:])
                        h1T = sb.tile([P, P], dt.float16, tag="xTh")
                        nc.vector.tensor_copy(h1T[:], ps_h1t[:])
                        nc.tensor.matmul(ps_xp2[:], lhsT=h1T[:],
                                         rhs=w2aug_h[k][:],
                                         start=(k == 0),
                                         stop=(k == H * HC // P - 1))
                    xp2_t = sbD.tile([P, XP2W], dt.float16, tag="xp2")
                    nc.vector.tensor_copy(xp2_t[:], ps_xp2[:])
                    nc.sync.dma_start(xp2_in[d * P:(d + 1) * P, :], xp2_t[:])
                esD.close()
                if stage >= 3:
                    xd2_t = [keep.tile([P, XP2W], dt.float16, tag=f"xd2{t}",
                                       name=f"xd2{t}") for t in range(nt_g)]
                    for t in range(nt_g):
                        nc.gpsimd.indirect_dma_start(
                            out=xd2_t[t][:], out_offset=None, in_=xp2_in[:],
                            in_offset=bass.IndirectOffsetOnAxis(
                                ap=gdstl_sb[:, t:t + 1], axis=0))
                nc.gpsimd.collective_compute(
                    "AllGather", ALU.bypass, replica_groups=RG,
                    ins=[xp2_in[:]], outs=[xp2_dram[:]])

            if stage >= 3:
                # ============ conv2 aggregation (reuses oh tiles) ============
                esE = ExitStack()
                sbE = esE.enter_context(tc.tile_pool(name="sbE", bufs=2))
                psE = esE.enter_context(
                    tc.tile_pool(name="psE", bufs=1, space="PSUM"))
                ps_m2 = [psE.tile([P, F + 1], dt.float32, space="PSUM",
                                  tag=f"ps_m2{d}", name=f"ps_m2{d}")
                         for d in range(2)]
                for t in range(nt_g):
                    xs2 = sbE.tile([P, XP2W], dt.float16, tag="xs2", bufs=4)
                    nc.gpsimd.indirect_dma_start(
                        out=xs2[:], out_offset=None, in_=xp2_dram[:],
                        in_offset=bass.IndirectOffsetOnAxis(
                            ap=gsrc_sb[:, t:t + 1], axis=0))
                    alpha2 = sbE.tile([P, 1], dt.float32, tag="alpha2", bufs=4)
                    nc.vector.tensor_add(alpha2[:], xs2[:, F:F + 1],
                                         xd2_t[t][:, F + 1:F + 2])
                    nc.vector.scalar_tensor_tensor(alpha2[:], alpha2[:], 0.2,
                                                   alpha2[:],
                                                   op0=ALU.mult, op1=ALU.max)
                    ex2 = sbE.tile([P, 1], dt.float32, tag="ex2", bufs=4)
                    nc.scalar.activation(ex2[:], alpha2[:], AF.Exp)
                    nc.vector.tensor_mul(ex2[:], ex2[:], valid_t[t][:])
                    msg2 = sbE.tile([P, F + 1], dt.float16, tag="msg2", bufs=4)
                    nc.vector.tensor_tensor(msg2[:, 0:F], xs2[:, 0:F],
                                            ex2[:].to_broadcast([P, F]),
                                            op=ALU.mult)
                    nc.vector.tensor_copy(msg2[:, F:F + 1], ex2[:])
                    for d in range(2):
                        lhsT = oh_t[t][:, d * P:(d + 1) * P]
                        st, sp = (t == 0), (t == nt_g - 1)
                        nc.tensor.matmul(ps_m2[d][:], lhsT=lhsT, rhs=msg2[:],
                                         start=st, stop=sp)
                c2b_bc = sbE.tile([P, F], dt.float32, tag="c2bbc", bufs=1)
                nc.sync.dma_start(c2b_bc[:], bcast(c2b, F))
                for d in range(2):
                    d2r = sbE.tile([P, 1], dt.float32, tag="d2r")
                    nc.vector.reciprocal(d2r[:], ps_m2[d][:, F:F + 1])
                    h2_t = sbE.tile([P, F], dt.float16, tag="h2")
                    nc.vector.tensor_tensor(h2_t[:], ps_m2[d][:, 0:F],
                                            d2r[:].to_broadcast([P, F]),
                                            op=ALU.mult)
                    nc.vector.tensor_add(h2_t[:], h2_t[:], c2b_bc[:])
                    nc.sync.dma_start(h2_in[d * P:(d + 1) * P, :], h2_t[:])
                esE.close()
                if stage >= 4:
                    xj_t = [keep.tile([P, F], dt.float16, tag=f"xj{t}",
                                      name=f"xj{t}") for t in range(nt_m)]
                    for t in range(nt_m):
                        nc.gpsimd.indirect_dma_start(
                            out=xj_t[t][:], out_offset=None, in_=h2_in[:],
                            in_offset=bass.IndirectOffsetOnAxis(
                                ap=mdstl_sb[:, t:t + 1], axis=0))
                nc.gpsimd.collective_compute(
                    "AllGather", ALU.bypass, replica_groups=RG,
                    ins=[h2_in[:]], outs=[h2_dram[:]])

            if stage >= 4:
                # ============ edge MLP ============
                m1w_h = [keep.tile([P, HC], dt.float16, tag=f"m1w{k}",
                                   name=f"m1w{k}") for k in range(4 * F // P)]
                for k in range(4 * F // P):
                    nc.sync.dma_start(m1w_h[k][:], m1w[k * P:(k + 1) * P, :])
                m1b_bc = single.tile([P, HC], dt.float32)
                nc.sync.dma_start(m1b_bc[:], bcast(m1b, HC))
                z1_t = [keep.tile([P, HC], dt.float16, tag=f"z1_{t}",
                                  name=f"z1_{t}") for t in range(nt_m)]
                mask_h = [keep.tile([P, 1], dt.float16, tag=f"maskh{t}",
                                    name=f"maskh{t}") for t in range(nt_m)]
                for t in range(nt_m):
                    nc.vector.tensor_copy(mask_h[t][:], mask_f[t][:])
                esF = ExitStack()
                sbF = esF.enter_context(tc.tile_pool(name="sbF", bufs=2))
                psF = esF.enter_context(
                    tc.tile_pool(name="psF", bufs=1, space="PSUM"))
                ps_s1 = psF.tile([1, HC], dt.float32, space="PSUM", tag="ps_s1")
                ps_q1 = psF.tile([1, HC], dt.float32, space="PSUM", tag="ps_q1")
                ps_cnt = psF.tile([1, 1], dt.float32, space="PSUM",
                                  tag="ps_cnt")
                for t in range(nt_m):
                    xi = sbF.tile([P, F], dt.float16, tag="xi", bufs=6)
                    nc.gpsimd.indirect_dma_start(
                        out=xi[:], out_offset=None, in_=h2_dram[:],
                        in_offset=bass.IndirectOffsetOnAxis(
                            ap=msrc_sb[:, t:t + 1], axis=0))
                    xj = xj_t[t]
                    dsub = sbF.tile([P, F], dt.float16, tag="dsub", bufs=4)
                    nc.vector.tensor_sub(dsub[:], xi[:], xj[:])
                    nc.scalar.activation(dsub[:], dsub[:], AF.Abs)
                    pmul = sbF.tile([P, F], dt.float16, tag="pmul", bufs=4)
                    nc.vector.tensor_mul(pmul[:], xi[:], xj[:])
                    ps_z1 = psF.tile([P, HC], dt.float32, space="PSUM",
                                     tag="ps_z1", bufs=2)
                    for pi, piece in enumerate((xi, xj, dsub, pmul)):
                        for hf in range(2):
                            ps_t = psF.tile([P, P], dt.float16, space="PSUM",
                                            tag="ps_xth", bufs=3)
                            nc.tensor.transpose(ps_t[:],
                                                piece[:, hf * P:(hf + 1) * P],
                                                ident_h[:])
                            efT = sb.tile([P, P], dt.float16, tag="xTh")
                            nc.vector.tensor_copy(efT[:], ps_t[:])
                            k = pi * 2 + hf
                            nc.tensor.matmul(ps_z1[:], lhsT=efT[:],
                                             rhs=m1w_h[k][:],
                                             start=(k == 0), stop=(k == 7))
                    nc.vector.tensor_add(z1_t[t][:], ps_z1[:], m1b_bc[:])
                    zsq = sbF.tile([P, HC], dt.float16, tag="zsq", bufs=4)
                    nc.vector.tensor_mul(zsq[:], z1_t[t][:], z1_t[t][:])
                    st, sp = (t == 0), (t == nt_m - 1)
                    nc.tensor.matmul(ps_s1[:], lhsT=mask_h[t][:],
                                     rhs=z1_t[t][:], start=st, stop=sp)
                    nc.tensor.matmul(ps_q1[:], lhsT=mask_h[t][:], rhs=zsq[:],
                                     start=st, stop=sp)
                    nc.tensor.matmul(ps_cnt[:], lhsT=mask_h[t][:],
                                     rhs=mask_h[t][:], start=st, stop=sp)
                # pack stats1, AllGather + local combine
                s_sb = sbF.tile([1, HC], dt.float32, tag="stat")
                nc.vector.tensor_copy(s_sb[:], ps_s1[:])
                nc.sync.dma_start(st1_in[None, 0:HC], s_sb[:])
                q_sb = sbF.tile([1, HC], dt.float32, tag="stat")
                nc.vector.tensor_copy(q_sb[:], ps_q1[:])
                nc.sync.dma_start(st1_in[None, HC:2 * HC], q_sb[:])
                c_sb = sbF.tile([1, 1], dt.float32, tag="statc")
                nc.vector.tensor_copy(c_sb[:], ps_cnt[:])
                nc.sync.dma_start(st1_in[None, 2 * HC:2 * HC + 1], c_sb[:])
                zpad = sbF.tile([1, 7], dt.float32, tag="statz")
                nc.vector.memset(zpad[:], 0.0)
                nc.sync.dma_start(st1_in[None, 2 * HC + 1:520], zpad[:])
                esF.close()
                nc.gpsimd.collective_compute(
                    "AllGather", ALU.bypass, replica_groups=RG,
                    ins=[st1_in[:]], outs=[st1_all[:]])

                esG = ExitStack()
                sbG = esG.enter_context(tc.tile_pool(name="sbG", bufs=2))
                psG = esG.enter_context(
                    tc.tile_pool(name="psG", bufs=1, space="PSUM"))

                # transpose z1 into [hc, e] banks while the AllGather runs;
                # the bn1 affine is then per-partition and z2 needs no
                # in-loop transposes.
                z1T_t = [keep.tile([P, HC], dt.float16, tag=f"z1T{t}",
                                   name=f"z1T{t}") for t in range(nt_m)]
                for t in range(nt_m):
                    for b in range(2):
                        ps_t = psG.tile([P, P], dt.float16, space="PSUM",
                                        tag="ps_xth", bufs=2)
                        nc.tensor.transpose(ps_t[:],
                                            z1_t[t][:, b * P:(b + 1) * P],
                                            ident_h[:])
                        nc.vector.tensor_copy(z1T_t[t][:, b * P:(b + 1) * P],
                                              ps_t[:])

                ones8 = single.tile([8, 1], dt.float32)
                nc.vector.memset(ones8[:], 1.0)
                cnt_dram = dram.tile([1], dt.float32)

                def combine_cols(st_all, nchunk, tagn):
                    """Sum the 8 gathered stat rows into [128, nchunk] columns
                    (chunk i = channels [i*128,(i+1)*128)) via per-chunk
                    matmuls with ones8 as rhs."""
                    st8 = sbG.tile([8, 520], dt.float32, tag=f"st8{tagn}",
                                   name=f"st8{tagn}")
                    nc.sync.dma_start(
                        st8[:, 0:st_all.shape[0] // 8],
                        st_all[:].rearrange("(r c) -> r c", r=8))
                    ps_c = psG.tile([P, 8], dt.float32, space="PSUM",
                                    tag="pscmb", bufs=1, name=f"pscmb{tagn}")
                    for i in range(nchunk):
                        nc.tensor.matmul(ps_c[:, i:i + 1],
                                         lhsT=st8[:, i * P:(i + 1) * P],
                                         rhs=ones8[:], start=True, stop=True)
                    cols = sbG.tile([P, nchunk], dt.float32,
                                    tag=f"stcol{tagn}", name=f"stcol{tagn}")
                    nc.vector.tensor_copy(cols[:], ps_c[:, 0:nchunk])
                    return st8, ps_c, cols

                # st1: chunks [s_b0, s_b1, q_b0, q_b1], cnt at flat col 512
                st8a, ps_ca, cols1 = combine_cols(st1_all, 4, "a")
                nc.tensor.matmul(ps_ca[0:8, 4:5], lhsT=st8a[:, 512:520],
                                 rhs=ones8[:], start=True, stop=True)
                cnt1 = sbG.tile([1, 1], dt.float32, tag="cnt1", name="cnt1")
                nc.vector.tensor_copy(cnt1[:], ps_ca[0:1, 4:5])
                nc.sync.dma_start(cnt_dram[:][None, :], cnt1[:])
                cnt_bc = sbG.tile([P, 1], dt.float32, tag="cntbc",
                                  name="cntbc")
                nc.sync.dma_start(cnt_bc[:], bcast_ap(cnt_dram, 1))
                nc.vector.tensor_scalar_max(cnt_bc[:], cnt_bc[:], 1.0)
                cr_bc = sbG.tile([P, 1], dt.float32, tag="crbc", name="crbc")
                nc.vector.reciprocal(cr_bc[:], cnt_bc[:])

                def bn_cols(cols, nb, g_in, b_in, tagn):
                    """cols = [s_0..s_{nb-1}, q_0..q_{nb-1}] -> gs/gb
                    [128, nb] column tiles, all math per-partition."""
                    mean = sbG.tile([P, nb], dt.float32, tag=f"bnm{tagn}",
                                    name=f"bnm{tagn}")
                    nc.vector.tensor_tensor(mean[:], cols[:, 0:nb],
                                            cr_bc[:].to_broadcast([P, nb]),
                                            op=ALU.mult)
                    var = sbG.tile([P, nb], dt.float32, tag=f"bnv{tagn}",
                                   name=f"bnv{tagn}")
                    nc.vector.tensor_tensor(var[:], cols[:, nb:2 * nb],
                                            cr_bc[:].to_broadcast([P, nb]),
                                            op=ALU.mult)
                    msq = sbG.tile([P, nb], dt.float32, tag=f"bnq{tagn}",
                                   name=f"bnq{tagn}")
                    nc.vector.tensor_mul(msq[:], mean[:], mean[:])
                    nc.vector.tensor_sub(var[:], var[:], msq[:])
                    nc.vector.tensor_scalar_add(var[:], var[:], 1e-5)
                    nc.scalar.activation(var[:], var[:], AF.Sqrt)
                    rstd = sbG.tile([P, nb], dt.float32, tag=f"bnr{tagn}",
                                    name=f"bnr{tagn}")
                    nc.vector.reciprocal(rstd[:], var[:])
                    gcol = sbG.tile([P, nb], dt.float32, tag=f"bng{tagn}",
                                    name=f"bng{tagn}")
                    nc.sync.dma_start(gcol[:],
                                      g_in.ap().rearrange("(k p) -> p k", p=P))
                    bcol = sbG.tile([P, nb], dt.float32, tag=f"bnb{tagn}",
                                    name=f"bnb{tagn}")
                    nc.sync.dma_start(bcol[:],
                                      b_in.ap().rearrange("(k p) -> p k", p=P))
                    gs = sbG.tile([P, nb], dt.float32, tag=f"bngs{tagn}",
                                  name=f"bngs{tagn}")
                    nc.vector.tensor_mul(gs[:], gcol[:], rstd[:])
                    gb = sbG.tile([P, nb], dt.float32, tag=f"bngb{tagn}",
                                  name=f"bngb{tagn}")
                    nc.vector.tensor_mul(gb[:], mean[:], gs[:])
                    nc.vector.tensor_sub(gb[:], bcol[:], gb[:])
                    return gs, gb

                gs1_c32, gb1_c32 = bn_cols(cols1, HC // P, bn1g, bn1b, "a")
                m2w_h = [keep.tile([P, HC2], dt.float16, tag=f"m2w{k}",
                                   name=f"m2w{k}") for k in range(HC // P)]
                for k in range(HC // P):
                    nc.sync.dma_start(m2w_h[k][:], m2w[k * P:(k + 1) * P, :])
                m2b_bc = single.tile([P, HC2], dt.float32)
                nc.sync.dma_start(m2b_bc[:], bcast(m2b, HC2))
                z2_t = [keep.tile([P, HC2], dt.float16, tag=f"z2_{t}",
                                  name=f"z2_{t}") for t in range(nt_m)]
                ps_s2 = psG.tile([1, HC2], dt.float32, space="PSUM",
                                 tag="ps_s2")
                ps_q2 = psG.tile([1, HC2], dt.float32, space="PSUM",
                                 tag="ps_q2")
                for t in range(nt_m):
                    znT = sbG.tile([P, HC], dt.float16, tag="znT", bufs=4)
                    for b in range(HC // P):
                        nc.scalar.activation(
                            znT[:, b * P:(b + 1) * P],
                            z1T_t[t][:, b * P:(b + 1) * P], AF.Relu,
                            bias=gb1_c32[:, b:b + 1],
                            scale=gs1_c32[:, b:b + 1])
                    ps_z2 = psG.tile([P, HC2], dt.float32, space="PSUM",
                                     tag="ps_z2", bufs=1)
                    for b in range(HC // P):
                        nc.tensor.matmul(ps_z2[:],
                                         lhsT=znT[:, b * P:(b + 1) * P],
                                         rhs=m2w_h[b][:],
                                         start=(b == 0), stop=(b == HC // P - 1))
                    nc.vector.tensor_add(z2_t[t][:], ps_z2[:], m2b_bc[:])
                    zsq2 = sbG.tile([P, HC2], dt.float16, tag="zsq2", bufs=4)
                    nc.vector.tensor_mul(zsq2[:], z2_t[t][:], z2_t[t][:])
                    st, sp = (t == 0), (t == nt_m - 1)
                    nc.tensor.matmul(ps_s2[:], lhsT=mask_h[t][:],
                                     rhs=z2_t[t][:], start=st, stop=sp)
                    nc.tensor.matmul(ps_q2[:], lhsT=mask_h[t][:], rhs=zsq2[:],
                                     start=st, stop=sp)
                s2_sb = sbG.tile([1, HC2], dt.float32, tag="stat2")
                nc.vector.tensor_copy(s2_sb[:], ps_s2[:])
                nc.sync.dma_start(st2_in[None, 0:HC2], s2_sb[:])
                q2_sb = sbG.tile([1, HC2], dt.float32, tag="stat2")
                nc.vector.tensor_copy(q2_sb[:], ps_q2[:])
                nc.sync.dma_start(st2_in[None, HC2:2 * HC2], q2_sb[:])
                nc.gpsimd.collective_compute(
                    "AllGather", ALU.bypass, replica_groups=RG,
                    ins=[st2_in[:]], outs=[st2_all[:]])
                z2T_t = [keep.tile([P, HC2], dt.float16, tag=f"z2T{t}",
                                   name=f"z2T{t}") for t in range(nt_m)]
                for t in range(nt_m):
                    ps_t = psG.tile([P, P], dt.float16, space="PSUM",
                                    tag="ps_xth", bufs=2)
                    nc.tensor.transpose(ps_t[:], z2_t[t][:], ident_h[:])
                    nc.vector.tensor_copy(z2T_t[t][:], ps_t[:])
                _, _, cols2 = combine_cols(st2_all, 2, "b")
                gs2_c32, gb2_c32 = bn_cols(cols2, 1, bn2g, bn2b, "b")
                m3w_h = single.tile([P, 1], dt.float16)
                nc.sync.dma_start(m3w_h[:], m3w[:, :])
                m3b_bc = single.tile([P, 1], dt.float32)
                nc.sync.dma_start(m3b_bc[:], bcast(m3b, 1))
                neg25 = single.tile([P, 1], dt.float32)
                nc.vector.memset(neg25[:], -2.5)
                for t in range(nt_m):
                    zn2T = sbG.tile([P, HC2], dt.float16, tag="zn2T", bufs=4)
                    nc.scalar.activation(zn2T[:], z2T_t[t][:], AF.Relu,
                                         bias=gb2_c32[:], scale=gs2_c32[:])
                    ps_sc = psG.tile([P, 1], dt.float32, space="PSUM",
                                     tag="ps_sc", bufs=1)
                    nc.tensor.matmul(ps_sc[:], lhsT=zn2T[:], rhs=m3w_h[:],
                                     start=True, stop=True)
                    score = sbG.tile([P, 1], dt.float32, tag="score", bufs=4)
                    nc.vector.tensor_add(score[:], ps_sc[:], m3b_bc[:])
                    sel = sbG.tile([P, 1], dt.float32, tag="sel", bufs=4)
                    nc.vector.select(sel[:], mask_u8[t][:], score[:], neg25[:])
                    nc.scalar.activation(sel[:], sel[:], AF.Sigmoid)
                    nc.sync.dma_start(score_out[t * P:(t + 1) * P][:, None],
                                      sel[:])
                esG.close()

    nc.compile()
    return nc


def kernel(**inputs):
    inputs = {k: np.asarray(v) for k, v in inputs.items()}
    src = inputs["edge_index"][0].astype(np.int64)
    dst = inputs["edge_index"][1].astype(np.int64)

    # --- edge partition by dst chunk (GAT set includes self loops) ---
    all_src = np.concatenate([src, np.arange(N, dtype=np.int64)])
    all_dst = np.concatenate([dst, np.arange(N, dtype=np.int64)])
    lidx_all = np.concatenate(
        [np.arange(E, dtype=np.int64), np.full(N, E, dtype=np.int64)])
    chunk_g = all_dst // NCHUNK
    gids = [np.where(chunk_g == c)[0] for c in range(NCORES)]
    nt_g = int(np.ceil(max(len(i) for i in gids) / P))
    pad_g = nt_g * P
    chunk_m = dst // NCHUNK
    mids = [np.where(chunk_m == c)[0] for c in range(NCORES)]
    nt_m = int(np.ceil(max(len(i) for i in mids) / P))
    pad_m = nt_m * P

    key = (nt_g, nt_m, DEBUG)
    if key not in _cache:
        _cache[key] = _build(nt_g, nt_m, debug=DEBUG)
    nc = _cache[key]

    def f32(v):
        return np.ascontiguousarray(v, dtype=np.float32)

    def f16(v):
        return np.ascontiguousarray(v, dtype=np.float16)

    shared = dict(
        xT=f16(inputs["x"].T),
        sent_emb=f32(inputs["sent_emb"]),
        elp=f32(inputs["edge_logits_param"]),
        elp_hi=f16(inputs["edge_logits_param"]),
        elp_lo=f16(inputs["edge_logits_param"]
                   - inputs["edge_logits_param"].astype(np.float16)
                     .astype(np.float32)),
        semb_hi=f16(inputs["sent_emb"]),
        semb_lo=f16(inputs["sent_emb"]
                    - inputs["sent_emb"].astype(np.float16)
                      .astype(np.float32)),
        fc0_w=f16(inputs["fc0_w"]), fc0_b=f32(inputs["fc0_b"]),
        fc1_w=f16(inputs["fc1_w"]), fc1_b=f32(inputs["fc1_b"]),
        conv1_W=f16(inputs["conv1_W"]),
        conv1_a=f16(np.concatenate([inputs["conv1_asrc"].reshape(-1),
                                    inputs["conv1_adst"].reshape(-1)])),
        conv1_b=f32(inputs["conv1_b"]),
        conv2_W=f16(inputs["conv2_W"]),
        conv2_a=f16(np.concatenate([inputs["conv2_asrc"].reshape(-1),
                                    inputs["conv2_adst"].reshape(-1)])),
        conv2_b=f32(inputs["conv2_b"]),
        mlp1_w=f16(inputs["mlp1_w"]), mlp1_b=f32(inputs["mlp1_b"]),
        bn1_g=f32(inputs["bn1_g"]), bn1_b=f32(inputs["bn1_b"]),
        mlp2_w=f16(inputs["mlp2_w"]), mlp2_b=f32(inputs["mlp2_b"]),
        bn2_g=f32(inputs["bn2_g"]), bn2_b=f32(inputs["bn2_b"]),
        mlp3_w=f16(inputs["mlp3_w"]), mlp3_b=f32(inputs["mlp3_b"]),
        fc2_b=f32(inputs["fc2_b"]),
    )

    fcl_w, fce_w, fc2_w = inputs["fcl_w"], inputs["fce_w"], inputs["fc2_w"]
    in_maps = []
    for c in range(NCORES):
        gi = gids[c]
        mi = mids[c]
        gsrc = np.zeros(pad_g, np.int32); gsrc[:len(gi)] = all_src[gi]
        gdst = np.zeros(pad_g, np.int32); gdst[:len(gi)] = all_dst[gi]
        gdstl = np.zeros(pad_g, np.int32)
        gdstl[:len(gi)] = all_dst[gi] - c * NCHUNK
        glidx = np.full(pad_g, E + 1, np.int32); glidx[:len(gi)] = lidx_all[gi]
        goh = np.zeros((pad_g, NCHUNK), np.float16)
        goh[np.arange(len(gi)), all_dst[gi] - c * NCHUNK] = 1.0
        msrc = np.zeros(pad_m, np.int32); msrc[:len(mi)] = src[mi]
        mdst = np.zeros(pad_m, np.int32); mdst[:len(mi)] = dst[mi]
        mdstl = np.zeros(pad_m, np.int32)
        mdstl[:len(mi)] = dst[mi] - c * NCHUNK
        mlidx = np.full(pad_m, E + 1, np.int32); mlidx[:len(mi)] = mi
        m = dict(shared)
        def hilo(w, blk=None):
            w = np.asarray(w, np.float32)
            h = w.astype(np.float16)
            l = (w - h.astype(np.float32)).astype(np.float16)
            if blk is None:
                return np.ascontiguousarray(np.concatenate([h, l], axis=1))
            # interleave per column-block: [h0|l0|h1|l1|...]
            parts = []
            for b in range(w.shape[1] // blk):
                parts.append(h[:, b * blk:(b + 1) * blk])
                parts.append(l[:, b * blk:(b + 1) * blk])
            return np.ascontiguousarray(np.concatenate(parts, axis=1))

        fc2w_c = np.asarray(np.concatenate(
            [fc2_w[c * ECH:(c + 1) * ECH],
             fc2_w[E + c * ECH:E + (c + 1) * ECH]], axis=0), np.float32)
        m.update(
            fclw_hl=hilo(fcl_w[:, c * ECH:(c + 1) * ECH]),
            fclb_sh=f32(inputs["fcl_b"][c * ECH:(c + 1) * ECH]),
            fcew_hl=hilo(fce_w[:, c * ECH:(c + 1) * ECH]),
            fceb_sh=f32(inputs["fce_b"][c * ECH:(c + 1) * ECH]),
            fc2w_hl=hilo(fc2w_c, blk=2048),
            g_src=gsrc, g_dst=gdst, g_dstl=gdstl, g_lidx=glidx, g_oh=goh,
            m_src=msrc, m_dst=mdst, m_dstl=mdstl, m_lidx=mlidx,
        )
        in_maps.append(m)

    global last_results, last_in_maps
    last_in_maps = in_maps
    res = run_bass_kernel_spmd(nc, in_maps, core_ids=list(range(NCORES)),
                               trace=TRACE)
    last_results = res
    orig = res.results[0]["orig_out"].reshape(E).astype(np.float32)
    sig = np.empty(E, np.float32)
    for c in range(NCORES):
        mi = mids[c]
        sig[mi] = res.results[c]["score_out"].reshape(pad_m)[:len(mi)]
    return sig, orig


# revision 45
# speedup vs baseline: 1.1174x; 1.0028x over previous
"""GATWithSentenceEmbedding Trainium2 kernel (8 NeuronCores, SPMD + collectives).

V2 restructure vs baseline:
  - Phase B (fcl/fce/fc2 streaming) uses rhs-side weight tiles with [1,N]
    matvec psums: weights stream row-major straight from DRAM (no rearrange,
    no per-j Ldweights storm).  Stream DMAs dispatch on the Pool engine so
    they never stall behind compute-gated dispatches.
  - Phase A: host stages x^T, so h^T and xp1 are computed with zero on-device
    transposes.  Small weights are host-cast to fp16 (halves DMA + removes
    DVE convert passes).
  - conv1 gather tiles (xs/ad/oh) are prefetched into SBUF during the weight
    stream; only the mask-dependent work runs after the logits AllReduce.
  - BN stats use AllGather + local ones-matmul combine instead of AllReduce
    (15.4us vs 28.2us each).
  - conv2 reuses conv1's one-hot tiles.
"""

import numpy as np
from contextlib import ExitStack

import concourse.bass as bass
import concourse.mybir as mybir
import concourse.tile as tile
from concourse import bacc
from concourse.bass_utils import run_bass_kernel_spmd
from concourse.masks import make_identity

N, F, HC, S, H, E, BERT = 2048, 256, 256, 512, 4, 8192, 768
NCORES = 8
P = 128
NCHUNK = N // NCORES          # 256 dst nodes per core
ECH = E // NCORES             # 1024 g1/g2 columns per core
XP1W = H * HC + 2 * H         # 1032 = xp1 | al_s | al_d
XP2W = F + 2                  # 258  = xp2 | al_s | al_d
HC2 = HC // 2                 # 128
BIG = 1.0e9

dt = mybir.dt
AF = mybir.ActivationFunctionType
ALU = mybir.AluOpType
RG = [list(range(NCORES))]

_cache = {}
last_in_maps = None
DEBUG = False
TRACE = False
last_results = None


def _build(nt_g: int, nt_m: int, debug: bool = False, stage: int = 4):
    pad_g = nt_g * P
    pad_m = nt_m * P
    nc = bacc.Bacc("TRN2", target_bir_lowering=False, debug=False)

    def inp(name, shape, dtype=dt.float32):
        return nc.dram_tensor(name, shape, dtype, kind="ExternalInput")

    # shared inputs (host-cast fp16 where precision allows)
    xT_in = inp("xT", [F, N], dt.float16)
    sent_in = inp("sent_emb", [BERT])
    elp_in = inp("elp", [E])
    fc0_w = inp("fc0_w", [BERT, S], dt.float16)
    fc0_b = inp("fc0_b", [S])
    fc1_w = inp("fc1_w", [F, S], dt.float16)
    fc1_b = inp("fc1_b", [S])
    c1w = inp("conv1_W", [S, H * HC], dt.float16)
    c1a = inp("conv1_a", [2 * H * HC], dt.float16)
    c1b = inp("conv1_b", [H * HC])
    c2w = inp("conv2_W", [H * HC, F], dt.float16)
    c2a = inp("conv2_a", [2 * F], dt.float16)
    c2b = inp("conv2_b", [F])
    m1w = inp("mlp1_w", [4 * F, HC], dt.float16)
    m1b = inp("mlp1_b", [HC])
    bn1g = inp("bn1_g", [HC]); bn1b = inp("bn1_b", [HC])
    m2w = inp("mlp2_w", [HC, HC2], dt.float16)
    m2b = inp("mlp2_b", [HC2])
    bn2g = inp("bn2_g", [HC2]); bn2b = inp("bn2_b", [HC2])
    m3w = inp("mlp3_w", [HC2, 1], dt.float16)
    m3b = inp("mlp3_b", [1])
    fc2_b = inp("fc2_b", [E])
    # per-core inputs (row-major, streamed as rhs tiles).  Each big matrix is
    # host-split into fp16 hi + fp16 lo (lo = x - fp16(x)); the matvec runs
    # hi*Whi + lo*Whi + hi*Wlo at fp16 PE rate (4x the fp32 rate) with
    # ~2^-22 effective precision (lo*Wlo dropped).
    fclw_hl = inp("fclw_hl", [E, 2 * ECH], dt.float16)
    fclb_sh = inp("fclb_sh", [ECH])
    fcew_hl = inp("fcew_hl", [BERT, 2 * ECH], dt.float16)
    fceb_sh = inp("fceb_sh", [ECH])
    fc2w_hl = inp("fc2w_hl", [2 * ECH, 2 * E], dt.float16)
    elp_hi = inp("elp_hi", [E], dt.float16)
    elp_lo = inp("elp_lo", [E], dt.float16)
    semb_hi = inp("semb_hi", [BERT], dt.float16)
    semb_lo = inp("semb_lo", [BERT], dt.float16)
    g_src = inp("g_src", [pad_g], dt.int32)
    g_dst = inp("g_dst", [pad_g], dt.int32)
    g_dstl = inp("g_dstl", [pad_g], dt.int32)
    g_lidx = inp("g_lidx", [pad_g], dt.int32)
    g_oh = inp("g_oh", [pad_g, NCHUNK], dt.float16)
    m_src = inp("m_src", [pad_m], dt.int32)
    m_dst = inp("m_dst", [pad_m], dt.int32)
    m_dstl = inp("m_dstl", [pad_m], dt.int32)
    m_lidx = inp("m_lidx", [pad_m], dt.int32)
    # outputs
    orig_out = nc.dram_tensor("orig_out", [E], dt.float32, kind="ExternalOutput")
    score_out = nc.dram_tensor("score_out", [pad_m], dt.float32,
                               kind="ExternalOutput")

    def bcast(dram_handle, cols, offset=0):
        """AP reading a [1, cols] DRAM row replicated over 128 partitions."""
        return bass.AP(tensor=dram_handle.ap().tensor, offset=offset,
                       ap=[[0, P], [1, cols]])

    def bcast_ap(ap_tile, cols, offset=0):
        a = ap_tile[:] if not isinstance(ap_tile, bass.AP) else ap_tile
        return bass.AP(tensor=a.tensor, offset=a.offset + offset,
                       ap=[[0, P], [1, cols]])

    with tile.TileContext(nc) as tc:
        with (
            tc.tile_pool(name="dram", bufs=1, space="DRAM") as dram,
            tc.tile_pool(name="single", bufs=1) as single,
            tc.tile_pool(name="sb", bufs=4) as sb,
            tc.tile_pool(name="keep", bufs=1) as keep,
        ):
            ident_h = single.tile([P, P], dt.float16)
            make_identity(nc, ident_h[:])

            # internal DRAM
            xp1_dram = dram.tile([N, XP1W], dt.float16)
            al1d_dram = dram.tile([N, 2 * H], dt.float16)
            fc2part = dram.tile([E], dt.float32)
            logits_dram = dram.tile([E], dt.float32, addr_space="Shared")
            lext_dram = dram.tile([E + 2, 1], dt.float32)
            sent_dram = dram.tile([S], dt.float32)
            gd_dram = dram.tile([2 * ECH], dt.float32)
            xp2_in = dram.tile([NCHUNK, XP2W], dt.float16)
            xp2_dram = dram.tile([N, XP2W], dt.float16, addr_space="Shared")
            h2_in = dram.tile([NCHUNK, F], dt.float16)
            h2_dram = dram.tile([N, F], dt.float16, addr_space="Shared")
            st1_in = dram.tile([520], dt.float32)
            st1_all = dram.tile([8 * 520], dt.float32, addr_space="Shared")
            st2_in = dram.tile([2 * HC2], dt.float32)
            st2_all = dram.tile([8 * 2 * HC2], dt.float32, addr_space="Shared")
            row_dram = dram.tile([4 * HC], dt.float32)  # gs/gb rows for bcast

            # =============== phase A: inputs + hT + xp1 (SP engine DMAs) ====
            # Phases A and B overlap at runtime, sharing 8 PSUM banks via two
            # tags: ps_big [128,2048] (sent-free; hT then xp1 cycle it) and
            # ps_row [1,2048] (sent, g1, g2, then fc2 blocks cycle it).
            esA = ExitStack()
            sbA = esA.enter_context(tc.tile_pool(name="sbA", bufs=2))
            psAB = esA.enter_context(
                tc.tile_pool(name="psAB", bufs=1, space="PSUM"))

            def row_ps():
                return psAB.tile([1, 2048], dt.float32, space="PSUM",
                                 tag="ps_row", bufs=1, name="ps_row")

            def big_ps():
                return psAB.tile([P, N], dt.float32, space="PSUM",
                                 tag="ps_big", bufs=1, name="ps_big")

            xT_t = [sbA.tile([P, N], dt.float16, tag=f"xT{k}", bufs=1,
                             name=f"xT{k}") for k in range(F // P)]
            for k in range(F // P):
                nc.sync.dma_start(xT_t[k][:], xT_in[k * P:(k + 1) * P, :])
            fc1w_t = [sbA.tile([P, S], dt.float16, tag=f"fc1w{k}", bufs=1,
                               name=f"fc1w{k}") for k in range(F // P)]
            for k in range(F // P):
                nc.sync.dma_start(fc1w_t[k][:], fc1_w[k * P:(k + 1) * P, :])
            es0 = ExitStack()
            sb0 = es0.enter_context(tc.tile_pool(name="sb0", bufs=1))
            fc0w_t = [sb0.tile([P, S], dt.float16, tag=f"fc0w{k}", bufs=1,
                               name=f"fc0w{k}") for k in range(BERT // P)]
            for k in range(BERT // P):
                nc.sync.dma_start(fc0w_t[k][:], fc0_w[k * P:(k + 1) * P, :])
            semb32 = single.tile([P, BERT // P], dt.float32)
            nc.sync.dma_start(semb32[:],
                              sent_in.ap().rearrange("(k p) -> p k", p=P))
            semb16 = single.tile([P, BERT // P], dt.float16)
            nc.vector.tensor_copy(semb16[:], semb32[:])
            elp_hc = single.tile([P, E // P], dt.float16)
            nc.sync.dma_start(elp_hc[:],
                              elp_hi.ap().rearrange("(k p) -> p k", p=P))
            elp_lc = single.tile([P, E // P], dt.float16)
            nc.sync.dma_start(elp_lc[:],
                              elp_lo.ap().rearrange("(k p) -> p k", p=P))
            semb_hc = single.tile([P, BERT // P], dt.float16)
            nc.sync.dma_start(semb_hc[:],
                              semb_hi.ap().rearrange("(k p) -> p k", p=P))
            semb_lc = single.tile([P, BERT // P], dt.float16)
            nc.sync.dma_start(semb_lc[:],
                              semb_lo.ap().rearrange("(k p) -> p k", p=P))

            # sent = relu(sent_emb @ fc0_w + fc0_b) as a [1, S] row
            ps_sent = row_ps()
            for k in range(BERT // P):
                nc.tensor.matmul(ps_sent[:, 0:S], lhsT=semb16[:, k:k + 1],
                                 rhs=fc0w_t[k][:],
                                 start=(k == 0), stop=(k == BERT // P - 1))
            es0.close()
            sent_row = sbA.tile([1, S], dt.float32, tag="sentrow", bufs=1)
            b_row = sbA.tile([1, S], dt.float32, tag="fc0brow", bufs=1)
            nc.sync.dma_start(b_row[:], fc0_b[None, :])
            nc.vector.tensor_add(sent_row[:], ps_sent[:, 0:S], b_row[:])
            nc.scalar.activation(sent_row[:], sent_row[:], AF.Relu)
            nc.sync.dma_start(sent_dram[:][None, :], sent_row[:])
            sent_col = single.tile([P, S // P], dt.float32)
            nc.sync.dma_start(sent_col[:],
                              sent_dram[:].rearrange("(k p) -> p k", p=P))
            fc1b_col = single.tile([P, S // P], dt.float32)
            nc.sync.dma_start(fc1b_col[:],
                              fc1_b.ap().rearrange("(k p) -> p k", p=P))

            # hT[s, n] = relu(xT^T-free GEMM) + sent, fully transpose-free
            hT_t = [sbA.tile([P, N], dt.float16, tag=f"hT{s}", bufs=1,
                             name=f"hT{s}") for s in range(S // P)]
            for si in range(S // P):
                ps_hT = big_ps()
                for k in range(F // P):
                    for q0 in range(0, N, 512):
                        nc.tensor.matmul(
                            ps_hT[:, q0:q0 + 512],
                            lhsT=fc1w_t[k][:, si * P:(si + 1) * P],
                            rhs=xT_t[k][:, q0:q0 + 512],
                            start=(k == 0), stop=(k == F // P - 1))
                nc.scalar.activation(hT_t[si][:], ps_hT[:], AF.Relu,
                                      bias=fc1b_col[:, si:si + 1])
                nc.vector.tensor_tensor(
                    hT_t[si][:], hT_t[si][:],
                    sent_col[:, si:si + 1].to_broadcast([P, N]), op=ALU.add)

            # W1aug = [conv1_W | W@a_src | W@a_dst] as 4 k-tiles [128, 1032] f16
            c1a_bc = sbA.tile([P, 2 * H * HC], dt.float16, tag="c1abc", bufs=1)
            nc.sync.dma_start(c1a_bc[:], bcast(c1a, 2 * H * HC))
            w1aug_h = [sbA.tile([P, XP1W], dt.float16, tag=f"w1aug{k}", bufs=1,
                                name=f"w1aug{k}") for k in range(S // P)]
            lp = nc.allow_low_precision(
                "fp16 a-vector projections only shape GAT softmax logits")
            lp.__enter__()
            for k in range(S // P):
                nc.sync.dma_start(w1aug_h[k][:, 0:H * HC],
                                  c1w[k * P:(k + 1) * P, :])
                tmp = sbA.tile([P, H * HC], dt.float16, tag="scratch4k")
                nc.vector.tensor_mul(tmp[:], w1aug_h[k][:, 0:H * HC],
                                     c1a_bc[:, 0:H * HC])
                for h in range(H):
                    nc.vector.reduce_sum(
                        w1aug_h[k][:, H * HC + h:H * HC + h + 1],
                        tmp[:, h * HC:(h + 1) * HC], axis=mybir.AxisListType.X)
                nc.vector.tensor_mul(tmp[:], w1aug_h[k][:, 0:H * HC],
                                     c1a_bc[:, H * HC:2 * H * HC])
                for h in range(H):
                    nc.vector.reduce_sum(
                        w1aug_h[k][:, H * HC + H + h:H * HC + H + h + 1],
                        tmp[:, h * HC:(h + 1) * HC], axis=mybir.AxisListType.X)

            # xp1 = h @ W1aug per node-tile; lhsT = hT slice (no transposes)

            def emit_xp1_tile(nt):
                ps_xp1 = big_ps()
                for si in range(S // P):
                    for s0, s1 in ((0, 512), (512, 1024), (1024, XP1W)):
                        nc.tensor.matmul(
                            ps_xp1[:, s0:s1],
                            lhsT=hT_t[si][:, nt * P:(nt + 1) * P],
                            rhs=w1aug_h[si][:, s0:s1],
                            start=(si == 0), stop=(si == S // P - 1))
                xp1_t = sbA.tile([P, XP1W], dt.float16, tag="xp1")
                nc.vector.tensor_copy(xp1_t[:], ps_xp1[:, 0:XP1W])
                nc.sync.dma_start(xp1_dram[nt * P:(nt + 1) * P, :], xp1_t[:])
                nc.sync.dma_start(al1d_dram[nt * P:(nt + 1) * P, :],
                                  xp1_t[:, H * HC:H * HC + 2 * H])

            # ======== phase B: weight streaming (Pool engine DMAs) ========
            sbB = esA.enter_context(tc.tile_pool(name="sbB", bufs=2))

            # g1 = relu(elp @ fcl_w + b): [1, ECH] psum row, hi/lo passes
            ps_g1 = row_ps()
            for k in range(E // P):
                wt = sbB.tile([P, 2 * ECH], dt.float16, tag="wfcl", bufs=3,
                              name="wfcl")
                nc.gpsimd.dma_start(wt[:], fclw_hl[k * P:(k + 1) * P, :])
                for q0 in range(0, ECH, 512):
                    nc.tensor.matmul(ps_g1[:, q0:q0 + 512],
                                     lhsT=elp_hc[:, k:k + 1],
                                     rhs=wt[:, q0:q0 + 512],
                                     start=(k == 0), stop=False)
                    nc.tensor.matmul(ps_g1[:, q0:q0 + 512],
                                     lhsT=elp_lc[:, k:k + 1],
                                     rhs=wt[:, q0:q0 + 512],
                                     start=False, stop=False)
                    nc.tensor.matmul(ps_g1[:, q0:q0 + 512],
                                     lhsT=elp_hc[:, k:k + 1],
                                     rhs=wt[:, ECH + q0:ECH + q0 + 512],
                                     start=False, stop=(k == E // P - 1))
            g1_row = sbB.tile([1, ECH], dt.float32, tag="g1row", bufs=1,
                              name="g1row")
            gb_row = sbB.tile([1, ECH], dt.float32, tag="gbrow", bufs=1)
            nc.sync.dma_start(gb_row[:], fclb_sh[None, :])
            nc.vector.tensor_add(g1_row[:], ps_g1[:, 0:ECH], gb_row[:])
            nc.scalar.activation(g1_row[:], g1_row[:], AF.Relu)
            nc.sync.dma_start(gd_dram[0:ECH][None, :], g1_row[:])

            # g2 = relu(sent_emb @ fce_w + b)
            ps_g2 = row_ps()
            for k in range(BERT // P):
                wt = sbB.tile([P, 2 * ECH], dt.float16, tag="wfcl", bufs=3,
                              name="wfce")
                nc.gpsimd.dma_start(wt[:], fcew_hl[k * P:(k + 1) * P, :])
                for q0 in range(0, ECH, 512):
                    nc.tensor.matmul(ps_g2[:, q0:q0 + 512],
                                     lhsT=semb_hc[:, k:k + 1],
                                     rhs=wt[:, q0:q0 + 512],
                                     start=(k == 0), stop=False)
                    nc.tensor.matmul(ps_g2[:, q0:q0 + 512],
                                     lhsT=semb_lc[:, k:k + 1],
                                     rhs=wt[:, q0:q0 + 512],
                                     start=False, stop=False)
                    nc.tensor.matmul(ps_g2[:, q0:q0 + 512],
                                     lhsT=semb_hc[:, k:k + 1],
                                     rhs=wt[:, ECH + q0:ECH + q0 + 512],
                                     start=False, stop=(k == BERT // P - 1))
            g2_row = sbB.tile([1, ECH], dt.float32, tag="g2row", bufs=1,
                              name="g2row")
            gb2_row = sbB.tile([1, ECH], dt.float32, tag="gbrow", bufs=1)
            nc.sync.dma_start(gb2_row[:], fceb_sh[None, :])
            nc.vector.tensor_add(g2_row[:], ps_g2[:, 0:ECH], gb2_row[:])
            nc.scalar.activation(g2_row[:], g2_row[:], AF.Relu)
            nc.sync.dma_start(gd_dram[ECH:2 * ECH][None, :], g2_row[:])

            # g hi/lo split done in tiny [128, 16] column space
            g_col32 = single.tile([P, 2 * ECH // P], dt.float32)
            nc.sync.dma_start(g_col32[:],
                              gd_dram[:].rearrange("(k p) -> p k", p=P))
            g_colh = single.tile([P, 2 * ECH // P], dt.float16)
            nc.vector.tensor_copy(g_colh[:], g_col32[:])
            g_colh32 = single.tile([P, 2 * ECH // P], dt.float32)
            nc.vector.tensor_copy(g_colh32[:], g_colh[:])
            g_coll32 = single.tile([P, 2 * ECH // P], dt.float32)
            nc.vector.tensor_sub(g_coll32[:], g_col32[:], g_colh32[:])
            g_coll = single.tile([P, 2 * ECH // P], dt.float16)
            nc.vector.tensor_copy(g_coll[:], g_coll32[:])

            # fc2 partial: column-block streaming, xp1 tiles interleaved so
            # the PE fills DMA-pacing slack without stalling the stream.
            CB = 2048
            NXB = (N // P) // (E // CB)
            for c in range(E // CB):
                ps_f2 = row_ps()
                for k in range(2 * ECH // P):
                    wt = sbB.tile([P, 2 * CB], dt.float16, tag="wfc2", bufs=3,
                                  name="wfc2")
                    nc.gpsimd.dma_start(
                        wt[:], fc2w_hl[k * P:(k + 1) * P,
                                       2 * c * CB:2 * (c + 1) * CB])
                    for q0 in range(0, CB, 512):
                        nc.tensor.matmul(ps_f2[:, q0:q0 + 512],
                                         lhsT=g_colh[:, k:k + 1],
                                         rhs=wt[:, q0:q0 + 512],
                                         start=(k == 0), stop=False)
                        nc.tensor.matmul(ps_f2[:, q0:q0 + 512],
                                         lhsT=g_coll[:, k:k + 1],
                                         rhs=wt[:, q0:q0 + 512],
                                         start=False, stop=False)
                        nc.tensor.matmul(ps_f2[:, q0:q0 + 512],
                                         lhsT=g_colh[:, k:k + 1],
                                         rhs=wt[:, CB + q0:CB + q0 + 512],
                                         start=False,
                                         stop=(k == 2 * ECH // P - 1))
                f2row = sbB.tile([1, CB], dt.float32, tag="f2row", bufs=1)
                nc.vector.tensor_copy(f2row[:], ps_f2[:, 0:CB])
                nc.sync.dma_start(fc2part[c * CB:(c + 1) * CB][None, :],
                                  f2row[:])
                for nt in range(c * NXB, (c + 1) * NXB):
                    emit_xp1_tile(nt)

            # ======== conv1 gather prefetch (during fc2 stream) ========
            gsrc_sb = single.tile([P, nt_g], dt.int32)
            nc.sync.dma_start(gsrc_sb[:],
                              g_src.ap().rearrange("(t p) -> p t", p=P))
            gdst_sb = single.tile([P, nt_g], dt.int32)
            nc.sync.dma_start(gdst_sb[:],
                              g_dst.ap().rearrange("(t p) -> p t", p=P))
            glidx_sb = single.tile([P, nt_g], dt.int32)
            nc.sync.dma_start(glidx_sb[:],
                              g_lidx.ap().rearrange("(t p) -> p t", p=P))
            gdstl_sb = single.tile([P, nt_g], dt.int32)
            nc.sync.dma_start(gdstl_sb[:],
                              g_dstl.ap().rearrange("(t p) -> p t", p=P))
            msrc_sb = single.tile([P, nt_m], dt.int32)
            nc.sync.dma_start(msrc_sb[:],
                              m_src.ap().rearrange("(t p) -> p t", p=P))
            mdstl_sb = single.tile([P, nt_m], dt.int32)
            nc.sync.dma_start(mdstl_sb[:],
                              m_dstl.ap().rearrange("(t p) -> p t", p=P))
            mlidx_sb = single.tile([P, nt_m], dt.int32)
            nc.sync.dma_start(mlidx_sb[:],
                              m_lidx.ap().rearrange("(t p) -> p t", p=P))
            oh_t = [keep.tile([P, NCHUNK], dt.float16, tag=f"oh{t}",
                              name=f"oh{t}") for t in range(nt_g)]
            xs_t = [keep.tile([P, XP1W], dt.float16, tag=f"xs{t}",
                              name=f"xs{t}") for t in range(nt_g)]
            msgu_t = [keep.tile([P, H * HC + H], dt.float16, tag=f"msgu{t}",
                               name=f"msgu{t}") for t in range(nt_g)]
            for t in range(nt_g):
                nc.sync.dma_start(oh_t[t][:], g_oh[t * P:(t + 1) * P, :])
                nc.gpsimd.indirect_dma_start(
                    out=xs_t[t][:], out_offset=None, in_=xp1_dram[:],
                    in_offset=bass.IndirectOffsetOnAxis(
                        ap=gsrc_sb[:, t:t + 1], axis=0))
                ad = sbA.tile([P, 2 * H], dt.float16, tag="gad", bufs=4)
                nc.gpsimd.indirect_dma_start(
                    out=ad[:], out_offset=None, in_=al1d_dram[:],
                    in_offset=bass.IndirectOffsetOnAxis(
                        ap=gdst_sb[:, t:t + 1], axis=0))
                # alpha/exp and the unmasked message xs*ex are mask-free:
                # compute them during the stream, leaving only *valid and the
                # one-hot matmuls for after the logits AllReduce.
                alpha = sbA.tile([P, H], dt.float32, tag="alpha", bufs=4)
                nc.vector.tensor_add(alpha[:],
                                     xs_t[t][:, H * HC:H * HC + H],
                                     ad[:, H:2 * H])
                nc.vector.scalar_tensor_tensor(alpha[:], alpha[:], 0.2,
                                               alpha[:],
                                               op0=ALU.mult, op1=ALU.max)
                ex = sbA.tile([P, H], dt.float32, tag="ex", bufs=4)
                nc.scalar.activation(ex[:], alpha[:], AF.Exp)
                for h in range(H):
                    nc.vector.tensor_tensor(
                        msgu_t[t][:, h * HC:(h + 1) * HC],
                        xs_t[t][:, h * HC:(h + 1) * HC],
                        ex[:, h:h + 1].to_broadcast([P, HC]), op=ALU.mult)
                nc.vector.tensor_copy(msgu_t[t][:, H * HC:H * HC + H], ex[:])
            esA.close()

            if stage >= 2:
                nc.gpsimd.collective_compute(
                    "AllReduce", ALU.add, replica_groups=RG,
                    ins=[fc2part[:]], outs=[logits_dram[:]])
                # logits += fc2_b ; orig_out ; logits_ext
                lg_pf = single.tile([P, E // P], dt.float32)
                nc.sync.dma_start(lg_pf[:],
                                  logits_dram[:].rearrange("(p f) -> p f", p=P))
                f2b_pf = single.tile([P, E // P], dt.float32)
                nc.sync.dma_start(f2b_pf[:],
                                  fc2_b.ap().rearrange("(p f) -> p f", p=P))
                nc.vector.tensor_add(lg_pf[:], lg_pf[:], f2b_pf[:])
                nc.sync.dma_start(orig_out.ap().rearrange("(p f) -> p f", p=P),
                                  lg_pf[:])
                nc.sync.dma_start(
                    lext_dram[0:E, :].rearrange("(p f) x -> p (f x)", p=P),
                    lg_pf[:])
                big_t = single.tile([1, 2], dt.float32)
                nc.vector.memset(big_t[:, 0:1], BIG)
                nc.vector.memset(big_t[:, 1:2], -BIG)
                nc.sync.dma_start(lext_dram[E:E + 2, 0][None, :], big_t[:])


                # ============ conv1 aggregation (mask-dependent part) =======
                valid_t = [keep.tile([P, 1], dt.float32, tag=f"valid{t}",
                                     name=f"valid{t}") for t in range(nt_g)]
                esC = ExitStack()
                sbC = esC.enter_context(tc.tile_pool(name="sbC", bufs=2))
                psC = esC.enter_context(
                    tc.tile_pool(name="psC", bufs=1, space="PSUM"))
                ps_msg = [psC.tile([P, H * HC], dt.float32, space="PSUM",
                                   tag=f"ps_msg{d}", name=f"ps_msg{d}")
                          for d in range(2)]
                ps_den = [psC.tile([P, H], dt.float32, space="PSUM",
                                   tag=f"ps_den{d}", name=f"ps_den{d}")
                          for d in range(2)]
                for t in range(nt_g):
                    lg = sbC.tile([P, 1], dt.float32, tag="glg", bufs=8)
                    nc.gpsimd.indirect_dma_start(
                        out=lg[:], out_offset=None, in_=lext_dram[:],
                        in_offset=bass.IndirectOffsetOnAxis(
                            ap=glidx_sb[:, t:t + 1], axis=0))
                    nc.vector.tensor_scalar(valid_t[t][:], lg[:], 0.0, None,
                                            op0=ALU.is_gt)
                for t in range(nt_g):
                    msg = sbC.tile([P, H * HC + H], dt.float16, tag="msg",
                                   bufs=4)
                    nc.vector.tensor_tensor(
                        msg[:], msgu_t[t][:],
                        valid_t[t][:].to_broadcast([P, H * HC + H]),
                        op=ALU.mult)
                    for d in range(2):
                        lhsT = oh_t[t][:, d * P:(d + 1) * P]
                        st, sp = (t == 0), (t == nt_g - 1)
                        nc.tensor.matmul(ps_msg[d][:, 0:512], lhsT=lhsT,
                                         rhs=msg[:, 0:512], start=st, stop=sp)
                        nc.tensor.matmul(ps_msg[d][:, 512:1024], lhsT=lhsT,
                                         rhs=msg[:, 512:1024], start=st,
                                         stop=sp)
                        nc.tensor.matmul(ps_den[d][:], lhsT=lhsT,
                                         rhs=msg[:, H * HC:H * HC + H],
                                         start=st, stop=sp)
                # prefetch MLP masks now that lext is final
                if stage >= 4:
                    mask_f = [keep.tile([P, 1], dt.float32, tag=f"maskf{t}",
                                        name=f"maskf{t}") for t in range(nt_m)]
                    mask_u8 = [keep.tile([P, 1], dt.uint8, tag=f"masku{t}",
                                         name=f"masku{t}") for t in range(nt_m)]
                    for t in range(nt_m):
                        mlg = sbC.tile([P, 1], dt.float32, tag="mlg", bufs=6)
                        nc.gpsimd.indirect_dma_start(
                            out=mlg[:], out_offset=None, in_=lext_dram[:],
                            in_offset=bass.IndirectOffsetOnAxis(
                                ap=mlidx_sb[:, t:t + 1], axis=0))
                        nc.vector.tensor_scalar(mask_f[t][:], mlg[:], 0.0,
                                                None, op0=ALU.is_gt)
                        nc.vector.tensor_copy(mask_u8[t][:], mask_f[t][:])

                # finalize conv1 (+elu) and xp2aug weights
                c1b_bc = sbC.tile([P, H * HC], dt.float32, tag="c1bbc", bufs=1)
                nc.sync.dma_start(c1b_bc[:], bcast(c1b, H * HC))
                c2a_bc = sbC.tile([P, 2 * F], dt.float16, tag="c2abc", bufs=1)
                nc.sync.dma_start(c2a_bc[:], bcast(c2a, 2 * F))
                w2aug_h = [keep.tile([P, XP2W], dt.float16, tag=f"w2aug{k}",
                                     name=f"w2aug{k}")
                           for k in range(H * HC // P)]
                for k in range(H * HC // P):
                    nc.sync.dma_start(w2aug_h[k][:, 0:F],
                                      c2w[k * P:(k + 1) * P, :])
                    tmp = sbC.tile([P, F], dt.float16, tag="w2tmp")
                    nc.vector.tensor_mul(tmp[:], w2aug_h[k][:, 0:F],
                                         c2a_bc[:, 0:F])
                    nc.vector.reduce_sum(w2aug_h[k][:, F:F + 1], tmp[:],
                                         axis=mybir.AxisListType.X)
                    nc.vector.tensor_mul(tmp[:], w2aug_h[k][:, 0:F],
                                         c2a_bc[:, F:2 * F])
                    nc.vector.reduce_sum(w2aug_h[k][:, F + 1:F + 2], tmp[:],
                                         axis=mybir.AxisListType.X)
                h1_keep = [keep.tile([P, H * HC], dt.float16, tag=f"h1k{d}",
                                     name=f"h1k{d}") for d in range(2)]
                for d in range(2):
                    denr = sbC.tile([P, H], dt.float32, tag="denr")
                    nc.vector.reciprocal(denr[:], ps_den[d][:])
                    h1_t = h1_keep[d]
                    h1f = sbC.tile([P, H * HC], dt.float32, tag="h1f")
                    for h in range(H):
                        nc.vector.scalar_tensor_tensor(
                            h1f[:, h * HC:(h + 1) * HC],
                            ps_msg[d][:, h * HC:(h + 1) * HC],
                            denr[:, h:h + 1],
                            c1b_bc[:, h * HC:(h + 1) * HC],
                            op0=ALU.mult, op1=ALU.add)
                    # elu = relu(x) + exp(min(x,0)) - 1
                    relu_t = sbC.tile([P, H * HC], dt.float32, tag="elu_r")
                    nc.scalar.activation(relu_t[:], h1f[:], AF.Relu)
                    nc.vector.tensor_scalar_min(h1f[:], h1f[:], 0.0)
                    nc.scalar.activation(h1f[:], h1f[:], AF.Exp)
                    nc.vector.scalar_tensor_tensor(h1_t[:], h1f[:], -1.0,
                                                   relu_t[:],
                                                   op0=ALU.add, op1=ALU.add)
                esC.close()
                esD = ExitStack()
                sbD = esD.enter_context(tc.tile_pool(name="sbD", bufs=2))
                psD = esD.enter_context(
                    tc.tile_pool(name="psD", bufs=1, space="PSUM"))
                for d in range(2):
                    h1_t = h1_keep[d]
                    ps_xp2 = psD.tile([P, XP2W], dt.float32, space="PSUM",
                                      tag="ps_xp2")
                    for k in range(H * HC // P):
                        ps_h1t = psD.tile([P, P], dt.float16, space="PSUM",
                                          tag="ps_xth", bufs=4)
                        nc.tensor.transpose(ps_h1t[:],
                                            h1_t[:, k * P:(k + 1) * P],
                                            ident_h[:])
                        h1T = sb.tile([P, P], dt.float16, tag="xTh")
                        nc.vector.tensor_copy(h1T[:], ps_h1t[:])
                        nc.tensor.matmul(ps_xp2[:], lhsT=h1T[:],
                                         rhs=w2aug_h[k][:],
                                         start=(k == 0),
                                         stop=(k == H * HC // P - 1))
                    xp2_t = sbD.tile([P, XP2W], dt.float16, tag="xp2")
                    nc.vector.tensor_copy(xp2_t[:], ps_xp2[:])
                    nc.sync.dma_start(xp2_in[d * P:(d + 1) * P, :], xp2_t[:])
                esD.close()
                if stage >= 3:
                    xd2_t = [keep.tile([P, XP2W], dt.float16, tag=f"xd2{t}",
                                       name=f"xd2{t}") for t in range(nt_g)]
                    for t in range(nt_g):
                        nc.gpsimd.indirect_dma_start(
                            out=xd2_t[t][:], out_offset=None, in_=xp2_in[:],
                            in_offset=bass.IndirectOffsetOnAxis(
                                ap=gdstl_sb[:, t:t + 1], axis=0))
                nc.gpsimd.collective_compute(
                    "AllGather", ALU.bypass, replica_groups=RG,
                    ins=[xp2_in[:]], outs=[xp2_dram[:]])

            if stage >= 3:
                # ============ conv2 aggregation (reuses oh tiles) ============
                esE = ExitStack()
                sbE = esE.enter_context(tc.tile_pool(name="sbE", bufs=2))
                psE = esE.enter_context(
                    tc.tile_pool(name="psE", bufs=1, space="PSUM"))
                ps_m2 = [psE.tile([P, F + 1], dt.float32, space="PSUM",
                                  tag=f"ps_m2{d}", name=f"ps_m2{d}")
                         for d in range(2)]
                for t in range(nt_g):
                    xs2 = sbE.tile([P, XP2W], dt.float16, tag="xs2", bufs=4)
                    nc.gpsimd.indirect_dma_start(
                        out=xs2[:], out_offset=None, in_=xp2_dram[:],
                        in_offset=bass.IndirectOffsetOnAxis(
                            ap=gsrc_sb[:, t:t + 1], axis=0))
                    alpha2 = sbE.tile([P, 1], dt.float32, tag="alpha2", bufs=4)
                    nc.vector.tensor_add(alpha2[:], xs2[:, F:F + 1],
                                         xd2_t[t][:, F + 1:F + 2])
                    ex2 = sbE.tile([P, 1], dt.float32, tag="ex2", bufs=4)
                    nc.scalar.activation(ex2[:], alpha2[:], AF.Lrelu,
                                         alpha=0.2)
                    nc.scalar.activation(ex2[:], ex2[:], AF.Exp)
                    nc.vector.tensor_mul(ex2[:], ex2[:], valid_t[t][:])
                    msg2 = sbE.tile([P, F + 1], dt.float16, tag="msg2", bufs=4)
                    nc.vector.tensor_tensor(msg2[:, 0:F], xs2[:, 0:F],
                                            ex2[:].to_broadcast([P, F]),
                                            op=ALU.mult)
                    nc.vector.tensor_copy(msg2[:, F:F + 1], ex2[:])
                    for d in range(2):
                        lhsT = oh_t[t][:, d * P:(d + 1) * P]
                        st, sp = (t == 0), (t == nt_g - 1)
                        nc.tensor.matmul(ps_m2[d][:], lhsT=lhsT, rhs=msg2[:],
                                         start=st, stop=sp)
                c2b_bc = sbE.tile([P, F], dt.float32, tag="c2bbc", bufs=1)
                nc.sync.dma_start(c2b_bc[:], bcast(c2b, F))
                for d in range(2):
                    d2r = sbE.tile([P, 1], dt.float32, tag="d2r")
                    nc.vector.reciprocal(d2r[:], ps_m2[d][:, F:F + 1])
                    h2_t = sbE.tile([P, F], dt.float16, tag="h2")
                    nc.vector.tensor_tensor(h2_t[:], ps_m2[d][:, 0:F],
                                            d2r[:].to_broadcast([P, F]),
                                            op=ALU.mult)
                    nc.vector.tensor_add(h2_t[:], h2_t[:], c2b_bc[:])
                    nc.sync.dma_start(h2_in[d * P:(d + 1) * P, :], h2_t[:])
                esE.close()
                if stage >= 4:
                    xj_t = [keep.tile([P, F], dt.float16, tag=f"xj{t}",
                                      name=f"xj{t}") for t in range(nt_m)]
                    for t in range(nt_m):
                        nc.gpsimd.indirect_dma_start(
                            out=xj_t[t][:], out_offset=None, in_=h2_in[:],
                            in_offset=bass.IndirectOffsetOnAxis(
                                ap=mdstl_sb[:, t:t + 1], axis=0))
                nc.gpsimd.collective_compute(
                    "AllGather", ALU.bypass, replica_groups=RG,
                    ins=[h2_in[:]], outs=[h2_dram[:]])
                if stage >= 4:
                    # transpose the (local) xj pieces while the AG runs
                    esT = ExitStack()
                    sbT = esT.enter_context(tc.tile_pool(name="sbT", bufs=2))
                    psT = esT.enter_context(
                        tc.tile_pool(name="psT", bufs=1, space="PSUM"))
                    xjT_t = [keep.tile([P, F], dt.float16, tag=f"xjT{t}",
                                       name=f"xjT{t}") for t in range(nt_m)]
                    for t in range(nt_m):
                        for hf in range(2):
                            ps_t = psT.tile([P, P], dt.float16, space="PSUM",
                                            tag="ps_xjt", bufs=2)
                            nc.tensor.transpose(
                                ps_t[:], xj_t[t][:, hf * P:(hf + 1) * P],
                                ident_h[:])
                            nc.vector.tensor_copy(
                                xjT_t[t][:, hf * P:(hf + 1) * P], ps_t[:])
                    esT.close()

            if stage >= 4:
                # ============ edge MLP ============
                m1w_h = [keep.tile([P, HC], dt.float16, tag=f"m1w{k}",
                                   name=f"m1w{k}") for k in range(4 * F // P)]
                for k in range(4 * F // P):
                    nc.sync.dma_start(m1w_h[k][:], m1w[k * P:(k + 1) * P, :])
                m1b_bc = single.tile([P, HC], dt.float32)
                nc.sync.dma_start(m1b_bc[:], bcast(m1b, HC))
                z1_t = [keep.tile([P, HC], dt.float16, tag=f"z1_{t}",
                                  name=f"z1_{t}") for t in range(nt_m)]
                mask_h = [keep.tile([P, 1], dt.float16, tag=f"maskh{t}",
                                    name=f"maskh{t}") for t in range(nt_m)]
                for t in range(nt_m):
                    nc.vector.tensor_copy(mask_h[t][:], mask_f[t][:])
                esF = ExitStack()
                sbF = esF.enter_context(tc.tile_pool(name="sbF", bufs=2))
                psF = esF.enter_context(
                    tc.tile_pool(name="psF", bufs=1, space="PSUM"))
                ps_s1 = psF.tile([1, HC], dt.float32, space="PSUM", tag="ps_s1")
                ps_q1 = psF.tile([1, HC], dt.float32, space="PSUM", tag="ps_q1")
                ps_cnt = psF.tile([1, 1], dt.float32, space="PSUM",
                                  tag="ps_cnt")
                for t in range(nt_m):
                    xi = sbF.tile([P, F], dt.float16, tag="xi", bufs=6)
                    nc.gpsimd.indirect_dma_start(
                        out=xi[:], out_offset=None, in_=h2_dram[:],
                        in_offset=bass.IndirectOffsetOnAxis(
                            ap=msrc_sb[:, t:t + 1], axis=0))
                    xj = xj_t[t]
                    dsub = sbF.tile([P, F], dt.float16, tag="dsub", bufs=4)
                    nc.vector.tensor_sub(dsub[:], xi[:], xj[:])
                    nc.scalar.activation(dsub[:], dsub[:], AF.Abs)
                    pmul = sbF.tile([P, F], dt.float16, tag="pmul", bufs=4)
                    nc.vector.tensor_mul(pmul[:], xi[:], xj[:])
                    ps_z1 = psF.tile([P, HC], dt.float32, space="PSUM",
                                     tag="ps_z1", bufs=2)
                    for pi, piece in enumerate((xi, None, dsub, pmul)):
                        for hf in range(2):
                            k = pi * 2 + hf
                            if piece is None:
                                nc.tensor.matmul(
                                    ps_z1[:],
                                    lhsT=xjT_t[t][:, hf * P:(hf + 1) * P],
                                    rhs=m1w_h[k][:],
                                    start=(k == 0), stop=(k == 7))
                                continue
                            ps_t = psF.tile([P, P], dt.float16, space="PSUM",
                                            tag="ps_xth", bufs=3)
                            nc.tensor.transpose(ps_t[:],
                                                piece[:, hf * P:(hf + 1) * P],
                                                ident_h[:])
                            efT = sb.tile([P, P], dt.float16, tag="xTh")
                            nc.vector.tensor_copy(efT[:], ps_t[:])
                            nc.tensor.matmul(ps_z1[:], lhsT=efT[:],
                                             rhs=m1w_h[k][:],
                                             start=(k == 0), stop=(k == 7))
                    nc.vector.tensor_add(z1_t[t][:], ps_z1[:], m1b_bc[:])
                    zsq = sbF.tile([P, HC], dt.float16, tag="zsq", bufs=4)
                    nc.vector.tensor_mul(zsq[:], z1_t[t][:], z1_t[t][:])
                    st, sp = (t == 0), (t == nt_m - 1)
                    nc.tensor.matmul(ps_s1[:], lhsT=mask_h[t][:],
                                     rhs=z1_t[t][:], start=st, stop=sp)
                    nc.tensor.matmul(ps_q1[:], lhsT=mask_h[t][:], rhs=zsq[:],
                                     start=st, stop=sp)
                    nc.tensor.matmul(ps_cnt[:], lhsT=mask_h[t][:],
                                     rhs=mask_h[t][:], start=st, stop=sp)
                # pack stats1, AllGather + local combine
                s_sb = sbF.tile([1, HC], dt.float32, tag="stat")
                nc.vector.tensor_copy(s_sb[:], ps_s1[:])
                nc.sync.dma_start(st1_in[None, 0:HC], s_sb[:])
                q_sb = sbF.tile([1, HC], dt.float32, tag="stat")
                nc.vector.tensor_copy(q_sb[:], ps_q1[:])
                nc.sync.dma_start(st1_in[None, HC:2 * HC], q_sb[:])
                c_sb = sbF.tile([1, 1], dt.float32, tag="statc")
                nc.vector.tensor_copy(c_sb[:], ps_cnt[:])
                nc.sync.dma_start(st1_in[None, 2 * HC:2 * HC + 1], c_sb[:])
                zpad = sbF.tile([1, 7], dt.float32, tag="statz")
                nc.vector.memset(zpad[:], 0.0)
                nc.sync.dma_start(st1_in[None, 2 * HC + 1:520], zpad[:])
                esF.close()
                nc.gpsimd.collective_compute(
                    "AllGather", ALU.bypass, replica_groups=RG,
                    ins=[st1_in[:]], outs=[st1_all[:]])

                esG = ExitStack()
                sbG = esG.enter_context(tc.tile_pool(name="sbG", bufs=2))
                psG = esG.enter_context(
                    tc.tile_pool(name="psG", bufs=1, space="PSUM"))

                # transpose z1 into [hc, e] banks while the AllGather runs;
                # the bn1 affine is then per-partition and z2 needs no
                # in-loop transposes.
                z1T_t = [keep.tile([P, HC], dt.float16, tag=f"z1T{t}",
                                   name=f"z1T{t}") for t in range(nt_m)]
                for t in range(nt_m):
                    for b in range(2):
                        ps_t = psG.tile([P, P], dt.float16, space="PSUM",
                                        tag="ps_xth", bufs=2)
                        nc.tensor.transpose(ps_t[:],
                                            z1_t[t][:, b * P:(b + 1) * P],
                                            ident_h[:])
                        nc.vector.tensor_copy(z1T_t[t][:, b * P:(b + 1) * P],
                                              ps_t[:])

                ones8 = single.tile([8, 1], dt.float32)
                nc.vector.memset(ones8[:], 1.0)
                cnt_dram = dram.tile([1], dt.float32)

                def combine_cols(st_all, nchunk, tagn):
                    """Sum the 8 gathered stat rows into [128, nchunk] columns
                    (chunk i = channels [i*128,(i+1)*128)) via per-chunk
                    matmuls with ones8 as rhs."""
                    st8 = sbG.tile([8, 520], dt.float32, tag=f"st8{tagn}",
                                   name=f"st8{tagn}")
                    nc.sync.dma_start(
                        st8[:, 0:st_all.shape[0] // 8],
                        st_all[:].rearrange("(r c) -> r c", r=8))
                    ps_c = psG.tile([P, 8], dt.float32, space="PSUM",
                                    tag="pscmb", bufs=1, name=f"pscmb{tagn}")
                    for i in range(nchunk):
                        nc.tensor.matmul(ps_c[:, i:i + 1],
                                         lhsT=st8[:, i * P:(i + 1) * P],
                                         rhs=ones8[:], start=True, stop=True)
                    cols = sbG.tile([P, nchunk], dt.float32,
                                    tag=f"stcol{tagn}", name=f"stcol{tagn}")
                    nc.vector.tensor_copy(cols[:], ps_c[:, 0:nchunk])
                    return st8, ps_c, cols

                # st1: chunks [s_b0, s_b1, q_b0, q_b1], cnt at flat col 512
                st8a, ps_ca, cols1 = combine_cols(st1_all, 4, "a")
                nc.tensor.matmul(ps_ca[0:8, 4:5], lhsT=st8a[:, 512:520],
                                 rhs=ones8[:], start=True, stop=True)
                cnt1 = sbG.tile([1, 1], dt.float32, tag="cnt1", name="cnt1")
                nc.vector.tensor_copy(cnt1[:], ps_ca[0:1, 4:5])
                nc.sync.dma_start(cnt_dram[:][None, :], cnt1[:])
                cnt_bc = sbG.tile([P, 1], dt.float32, tag="cntbc",
                                  name="cntbc")
                nc.sync.dma_start(cnt_bc[:], bcast_ap(cnt_dram, 1))
                nc.vector.tensor_scalar_max(cnt_bc[:], cnt_bc[:], 1.0)
                cr_bc = sbG.tile([P, 1], dt.float32, tag="crbc", name="crbc")
                nc.vector.reciprocal(cr_bc[:], cnt_bc[:])

                def bn_cols(cols, nb, g_in, b_in, tagn):
                    """cols = [s_0..s_{nb-1}, q_0..q_{nb-1}] -> gs/gb
                    [128, nb] column tiles, all math per-partition."""
                    mean = sbG.tile([P, nb], dt.float32, tag=f"bnm{tagn}",
                                    name=f"bnm{tagn}")
                    nc.vector.tensor_tensor(mean[:], cols[:, 0:nb],
                                            cr_bc[:].to_broadcast([P, nb]),
                                            op=ALU.mult)
                    var = sbG.tile([P, nb], dt.float32, tag=f"bnv{tagn}",
                                   name=f"bnv{tagn}")
                    nc.vector.tensor_tensor(var[:], cols[:, nb:2 * nb],
                                            cr_bc[:].to_broadcast([P, nb]),
                                            op=ALU.mult)
                    msq = sbG.tile([P, nb], dt.float32, tag=f"bnq{tagn}",
                                   name=f"bnq{tagn}")
                    nc.vector.tensor_mul(msq[:], mean[:], mean[:])
                    nc.vector.tensor_sub(var[:], var[:], msq[:])
                    nc.vector.tensor_scalar_add(var[:], var[:], 1e-5)
                    nc.scalar.activation(var[:], var[:], AF.Sqrt)
                    rstd = sbG.tile([P, nb], dt.float32, tag=f"bnr{tagn}",
                                    name=f"bnr{tagn}")
                    nc.vector.reciprocal(rstd[:], var[:])
                    gcol = sbG.tile([P, nb], dt.float32, tag=f"bng{tagn}",
                                    name=f"bng{tagn}")
                    nc.sync.dma_start(gcol[:],
                                      g_in.ap().rearrange("(k p) -> p k", p=P))
                    bcol = sbG.tile([P, nb], dt.float32, tag=f"bnb{tagn}",
                                    name=f"bnb{tagn}")
                    nc.sync.dma_start(bcol[:],
                                      b_in.ap().rearrange("(k p) -> p k", p=P))
                    gs = sbG.tile([P, nb], dt.float32, tag=f"bngs{tagn}",
                                  name=f"bngs{tagn}")
                    nc.vector.tensor_mul(gs[:], gcol[:], rstd[:])
                    gb = sbG.tile([P, nb], dt.float32, tag=f"bngb{tagn}",
                                  name=f"bngb{tagn}")
                    nc.vector.tensor_mul(gb[:], mean[:], gs[:])
                    nc.vector.tensor_sub(gb[:], bcol[:], gb[:])
                    return gs, gb

                gs1_c32, gb1_c32 = bn_cols(cols1, HC // P, bn1g, bn1b, "a")
                m2w_h = [keep.tile([P, HC2], dt.float16, tag=f"m2w{k}",
                                   name=f"m2w{k}") for k in range(HC // P)]
                for k in range(HC // P):
                    nc.sync.dma_start(m2w_h[k][:], m2w[k * P:(k + 1) * P, :])
                m2b_bc = single.tile([P, HC2], dt.float32)
                nc.sync.dma_start(m2b_bc[:], bcast(m2b, HC2))
                z2_t = [keep.tile([P, HC2], dt.float16, tag=f"z2_{t}",
                                  name=f"z2_{t}") for t in range(nt_m)]
                ps_s2 = psG.tile([1, HC2], dt.float32, space="PSUM",
                                 tag="ps_s2")
                ps_q2 = psG.tile([1, HC2], dt.float32, space="PSUM",
                                 tag="ps_q2")
                for t in range(nt_m):
                    znT = sbG.tile([P, HC], dt.float16, tag="znT", bufs=4)
                    for b in range(HC // P):
                        nc.scalar.activation(
                            znT[:, b * P:(b + 1) * P],
                            z1T_t[t][:, b * P:(b + 1) * P], AF.Relu,
                            bias=gb1_c32[:, b:b + 1],
                            scale=gs1_c32[:, b:b + 1])
                    ps_z2 = psG.tile([P, HC2], dt.float32, space="PSUM",
                                     tag="ps_z2", bufs=1)
                    for b in range(HC // P):
                        nc.tensor.matmul(ps_z2[:],
                                         lhsT=znT[:, b * P:(b + 1) * P],
                                         rhs=m2w_h[b][:],
                                         start=(b == 0), stop=(b == HC // P - 1))
                    nc.vector.tensor_add(z2_t[t][:], ps_z2[:], m2b_bc[:])
                    zsq2 = sbG.tile([P, HC2], dt.float16, tag="zsq2", bufs=4)
                    nc.vector.tensor_mul(zsq2[:], z2_t[t][:], z2_t[t][:])
                    st, sp = (t == 0), (t == nt_m - 1)
                    nc.tensor.matmul(ps_s2[:], lhsT=mask_h[t][:],
                                     rhs=z2_t[t][:], start=st, stop=sp)
                    nc.tensor.matmul(ps_q2[:], lhsT=mask_h[t][:], rhs=zsq2[:],
                                     start=st, stop=sp)
                s2_sb = sbG.tile([1, HC2], dt.float32, tag="stat2")
                nc.vector.tensor_copy(s2_sb[:], ps_s2[:])
                nc.sync.dma_start(st2_in[None, 0:HC2], s2_sb[:])
                q2_sb = sbG.tile([1, HC2], dt.float32, tag="stat2")
                nc.vector.tensor_copy(q2_sb[:], ps_q2[:])
                nc.sync.dma_start(st2_in[None, HC2:2 * HC2], q2_sb[:])
                nc.gpsimd.collective_compute(
                    "AllGather", ALU.bypass, replica_groups=RG,
                    ins=[st2_in[:]], outs=[st2_all[:]])
                z2T_t = [keep.tile([P, HC2], dt.float16, tag=f"z2T{t}",
                                   name=f"z2T{t}") for t in range(nt_m)]
                for t in range(nt_m):
                    ps_t = psG.tile([P, P], dt.float16, space="PSUM",
                                    tag="ps_xth", bufs=2)
                    nc.tensor.transpose(ps_t[:], z2_t[t][:], ident_h[:])
                    nc.vector.tensor_copy(z2T_t[t][:], ps_t[:])
                _, _, cols2 = combine_cols(st2_all, 2, "b")
                gs2_c32, gb2_c32 = bn_cols(cols2, 1, bn2g, bn2b, "b")
                m3w_h = single.tile([P, 1], dt.float16)
                nc.sync.dma_start(m3w_h[:], m3w[:, :])
                m3b_bc = single.tile([P, 1], dt.float32)
                nc.sync.dma_start(m3b_bc[:], bcast(m3b, 1))
                neg25 = single.tile([P, 1], dt.float32)
                nc.vector.memset(neg25[:], -2.5)
                for t in range(nt_m):
                    zn2T = sbG.tile([P, HC2], dt.float16, tag="zn2T", bufs=4)
                    nc.scalar.activation(zn2T[:], z2T_t[t][:], AF.Relu,
                                         bias=gb2_c32[:], scale=gs2_c32[:])
                    ps_sc = psG.tile([P, 1], dt.float32, space="PSUM",
                                     tag="ps_sc", bufs=1)
                    nc.tensor.matmul(ps_sc[:], lhsT=zn2T[:], rhs=m3w_h[:],
                                     start=True, stop=True)
                    score = sbG.tile([P, 1], dt.float32, tag="score", bufs=4)
                    nc.vector.tensor_add(score[:], ps_sc[:], m3b_bc[:])
                    sel = sbG.tile([P, 1], dt.float32, tag="sel", bufs=4)
                    nc.vector.select(sel[:], mask_u8[t][:], score[:], neg25[:])
                    nc.scalar.activation(sel[:], sel[:], AF.Sigmoid)
                    nc.sync.dma_start(score_out[t * P:(t + 1) * P][:, None],
                                      sel[:])
                esG.close()

    nc.compile()
    return nc


def kernel(**inputs):
    inputs = {k: np.asarray(v) for k, v in inputs.items()}
    src = inputs["edge_index"][0].astype(np.int64)
    dst = inputs["edge_index"][1].astype(np.int64)

    # --- edge partition by dst chunk (GAT set includes self loops) ---
    all_src = np.concatenate([src, np.arange(N, dtype=np.int64)])
    all_dst = np.concatenate([dst, np.arange(N, dtype=np.int64)])
    lidx_all = np.concatenate(
        [np.arange(E, dtype=np.int64), np.full(N, E, dtype=np.int64)])
    chunk_g = all_dst // NCHUNK
    gids = [np.where(chunk_g == c)[0] for c in range(NCORES)]
    nt_g = int(np.ceil(max(len(i) for i in gids) / P))
    pad_g = nt_g * P
    chunk_m = dst // NCHUNK
    mids = [np.where(chunk_m == c)[0] for c in range(NCORES)]
    nt_m = int(np.ceil(max(len(i) for i in mids) / P))
    pad_m = nt_m * P

    key = (nt_g, nt_m, DEBUG)
    if key not in _cache:
        _cache[key] = _build(nt_g, nt_m, debug=DEBUG)
    nc = _cache[key]

    def f32(v):
        return np.ascontiguousarray(v, dtype=np.float32)

    def f16(v):
        return np.ascontiguousarray(v, dtype=np.float16)

    shared = dict(
        xT=f16(inputs["x"].T),
        sent_emb=f32(inputs["sent_emb"]),
        elp=f32(inputs["edge_logits_param"]),
        elp_hi=f16(inputs["edge_logits_param"]),
        elp_lo=f16(inputs["edge_logits_param"]
                   - inputs["edge_logits_param"].astype(np.float16)
                     .astype(np.float32)),
        semb_hi=f16(inputs["sent_emb"]),
        semb_lo=f16(inputs["sent_emb"]
                    - inputs["sent_emb"].astype(np.float16)
                      .astype(np.float32)),
        fc0_w=f16(inputs["fc0_w"]), fc0_b=f32(inputs["fc0_b"]),
        fc1_w=f16(inputs["fc1_w"]), fc1_b=f32(inputs["fc1_b"]),
        conv1_W=f16(inputs["conv1_W"]),
        conv1_a=f16(np.concatenate([inputs["conv1_asrc"].reshape(-1),
                                    inputs["conv1_adst"].reshape(-1)])),
        conv1_b=f32(inputs["conv1_b"]),
        conv2_W=f16(inputs["conv2_W"]),
        conv2_a=f16(np.concatenate([inputs["conv2_asrc"].reshape(-1),
                                    inputs["conv2_adst"].reshape(-1)])),
        conv2_b=f32(inputs["conv2_b"]),
        mlp1_w=f16(inputs["mlp1_w"]), mlp1_b=f32(inputs["mlp1_b"]),
        bn1_g=f32(inputs["bn1_g"]), bn1_b=f32(inputs["bn1_b"]),
        mlp2_w=f16(inputs["mlp2_w"]), mlp2_b=f32(inputs["mlp2_b"]),
        bn2_g=f32(inputs["bn2_g"]), bn2_b=f32(inputs["bn2_b"]),
        mlp3_w=f16(inputs["mlp3_w"]), mlp3_b=f32(inputs["mlp3_b"]),
        fc2_b=f32(inputs["fc2_b"]),
    )

    fcl_w, fce_w, fc2_w = inputs["fcl_w"], inputs["fce_w"], inputs["fc2_w"]
    in_maps = []
    for c in range(NCORES):
        gi = gids[c]
        mi = mids[c]
        gsrc = np.zeros(pad_g, np.int32); gsrc[:len(gi)] = all_src[gi]
        gdst = np.zeros(pad_g, np.int32); gdst[:len(gi)] = all_dst[gi]
        gdstl = np.zeros(pad_g, np.int32)
        gdstl[:len(gi)] = all_dst[gi] - c * NCHUNK
        glidx = np.full(pad_g, E + 1, np.int32); glidx[:len(gi)] = lidx_all[gi]
        goh = np.zeros((pad_g, NCHUNK), np.float16)
        goh[np.arange(len(gi)), all_dst[gi] - c * NCHUNK] = 1.0
        msrc = np.zeros(pad_m, np.int32); msrc[:len(mi)] = src[mi]
        mdst = np.zeros(pad_m, np.int32); mdst[:len(mi)] = dst[mi]
        mdstl = np.zeros(pad_m, np.int32)
        mdstl[:len(mi)] = dst[mi] - c * NCHUNK
        mlidx = np.full(pad_m, E + 1, np.int32); mlidx[:len(mi)] = mi
        m = dict(shared)
        def hilo(w, blk=None):
            w = np.asarray(w, np.float32)
            h = w.astype(np.float16)
            l = (w - h.astype(np.float32)).astype(np.float16)
            if blk is None:
                return np.ascontiguousarray(np.concatenate([h, l], axis=1))
            # interleave per column-block: [h0|l0|h1|l1|...]
            parts = []
            for b in range(w.shape[1] // blk):
                parts.append(h[:, b * blk:(b + 1) * blk])
                parts.append(l[:, b * blk:(b + 1) * blk])
            return np.ascontiguousarray(np.concatenate(parts, axis=1))

        fc2w_c = np.asarray(np.concatenate(
            [fc2_w[c * ECH:(c + 1) * ECH],
             fc2_w[E + c * ECH:E + (c + 1) * ECH]], axis=0), np.float32)
        m.update(
            fclw_hl=hilo(fcl_w[:, c * ECH:(c + 1) * ECH]),
            fclb_sh=f32(inputs["fcl_b"][c * ECH:(c + 1) * ECH]),
            fcew_hl=hilo(fce_w[:, c * ECH:(c + 1) * ECH]),
            fceb_sh=f32(inputs["fce_b"][c * ECH:(c + 1) * ECH]),
            fc2w_hl=hilo(fc2w_c, blk=2048),
            g_src=gsrc, g_dst=gdst, g_dstl=gdstl, g_lidx=glidx, g_oh=goh,
            m_src=msrc, m_dst=mdst, m_dstl=mdstl, m_lidx=mlidx,
        )
        in_maps.append(m)

    global last_results, last_in_maps
    last_in_maps = in_maps
    res = run_bass_kernel_spmd(nc, in_maps, core_ids=list(range(NCORES)),
                               trace=TRACE)
    last_results = res
    orig = res.results[0]["orig_out"].reshape(E).astype(np.float32)
    sig = np.empty(E, np.float32)
    for c in range(NCORES):
        mi = mids[c]
        sig[mi] = res.results[c]["score_out"].reshape(pad_m)[:len(mi)]
    return sig, orig
